# revision 1
# baseline (speedup 1.0000x reference)
"""Trainium2 Bass kernel for nn_DecodeMoeOps (MoE decode: dispatch-quant,
grouped int8 GEMM1, SwiGLU, requant, grouped int8 GEMM2, weighted combine).

Expert-parallel across 8 NeuronCores: core c owns experts {2c, 2c+1} and
computes, for ALL 128 tokens, its 2 experts' contributions weighted by the
combine matrix; the host sums the 8 partial outputs. Combine weights are zero
for unrouted (token, expert) pairs, so this matches the reference's dense
compute exactly.
"""

import os
import sys

for _p in ("/opt/trn_rl_repo", "/root/.axon_site/_ro/trn_rl_repo"):
    if os.path.isdir(_p) and _p not in sys.path:
        sys.path.insert(0, _p)

from contextlib import ExitStack

import ml_dtypes
import numpy as np

import concourse.bass as bass
import concourse.mybir as mybir
import concourse.tile as tile
from concourse import bacc
from concourse.bass_utils import run_bass_kernel_spmd
from concourse.masks import make_identity

B, TOPK, H, I, E = 128, 8, 2048, 1408, 16
NCORES = 8
EPC = E // NCORES  # experts per core
KH = H // 128  # 16 k-tiles for GEMM1 contraction
KI = I // 128  # 11 k-tiles for GEMM2 contraction
I2 = 2 * I
F32 = mybir.dt.float32
BF16 = mybir.dt.bfloat16
MAGIC = float(3 * 2**22)  # 1.5*2^23: fp32 round-to-int magic (covers negatives)

# chunking of a 1408-wide GEMM1 half across PSUM (bank = 512 fp32)
N1_CHUNKS = [(0, 512), (512, 512), (1024, 384)]
N2_CHUNKS = [(0, 512), (512, 512), (1024, 512), (1536, 512)]

# weight storage in HBM: "int8" ships 1 B/weight and casts to bf16 during the
# SWDGE DMA; "bf16" ships 2 B/weight over HWDGE with no cast.
VARIANT = os.environ.get("MOE_VARIANT", "int8")
KG1 = 4  # w1 k-tiles per consolidated DMA (16 = 4 groups of 4)
W2_GROUPS = [(0, 2), (2, 2), (4, 2), (6, 2), (8, 2), (10, 1)]  # w2 DMA groups

_cache: dict = {}


def _build_program(debug_taps=False):
    nc = bacc.Bacc(
        "TRN2",
        target_bir_lowering=False,
        debug=False,
        num_devices=NCORES,
    )
    mult = mybir.AluOpType.mult

    WDT = mybir.dt.int8 if VARIANT == "int8" else BF16

    # --- per-core DRAM I/O ---
    xqT_d = nc.dram_tensor("xqT", [128, H], BF16, kind="ExternalInput").ap()
    sx_d = nc.dram_tensor("sx", [128, 1], F32, kind="ExternalInput").ap()
    comb_d = nc.dram_tensor("combs", [128, EPC], F32, kind="ExternalInput").ap()
    # w1 tiled [expert, half(gate/up), k, p, f]
    w1_d = nc.dram_tensor(
        "w1t", [EPC, 2, KH, 128, I], WDT, kind="ExternalInput"
    ).ap()
    # w2 tiled [expert, k, p, f]
    w2_d = nc.dram_tensor("w2t", [EPC, KI, 128, H], WDT, kind="ExternalInput").ap()
    sc1_d = nc.dram_tensor("scale1", [EPC, I2], F32, kind="ExternalInput").ap()
    sc2_d = nc.dram_tensor("scale2", [EPC, H], F32, kind="ExternalInput").ap()
    y_d = nc.dram_tensor("y", [128, H], F32, kind="ExternalOutput").ap()
    taps = {}
    if debug_taps:
        for e in range(EPC):
            taps[f"dbg_deq0_{e}"] = nc.dram_tensor(f"dbg_deq0_{e}", [128, I], F32, kind="ExternalOutput").ap()
            taps[f"dbg_deq1_{e}"] = nc.dram_tensor(f"dbg_deq1_{e}", [128, I], F32, kind="ExternalOutput").ap()
            taps[f"dbg_act_{e}"] = nc.dram_tensor(f"dbg_act_{e}", [128, I], F32, kind="ExternalOutput").ap()
            taps[f"dbg_mc_{e}"] = nc.dram_tensor(f"dbg_mc_{e}", [128, 1], F32, kind="ExternalOutput").ap()
            taps[f"dbg_aq_{e}"] = nc.dram_tensor(f"dbg_aq_{e}", [128, I], F32, kind="ExternalOutput").ap()
            taps[f"dbg_aqT_{e}"] = nc.dram_tensor(f"dbg_aqT_{e}", [128, KI * 128], F32, kind="ExternalOutput").ap()

    with tile.TileContext(nc) as tc, ExitStack() as ctx:
        consts = ctx.enter_context(tc.tile_pool(name="consts", bufs=1))
        rows = ctx.enter_context(tc.tile_pool(name="rows", bufs=1))
        bcast = ctx.enter_context(tc.tile_pool(name="bcast", bufs=1))
        w1_pool = ctx.enter_context(tc.tile_pool(name="w1p", bufs=4))
        w2_pool = ctx.enter_context(tc.tile_pool(name="w2p", bufs=6))
        actp = ctx.enter_context(tc.tile_pool(name="actp", bufs=1))
        aqTp = ctx.enter_context(tc.tile_pool(name="aqTp", bufs=2))
        w2sp = ctx.enter_context(tc.tile_pool(name="w2sp", bufs=2))
        stats = ctx.enter_context(tc.tile_pool(name="stats", bufs=2))
        yp = ctx.enter_context(tc.tile_pool(name="yp", bufs=1))
        ychunkp = ctx.enter_context(tc.tile_pool(name="ychunkp", bufs=4))
        ps1_pool = ctx.enter_context(tc.tile_pool(name="ps1", bufs=1, space="PSUM"))
        ps2_pool = ctx.enter_context(tc.tile_pool(name="ps2", bufs=1, space="PSUM"))
        psT_pool = ctx.enter_context(tc.tile_pool(name="psT", bufs=1, space="PSUM"))

        # --- prologue: constants ---
        xqT_s = consts.tile([128, H], BF16, name="xqT_s")
        nc.sync.dma_start(out=xqT_s[:], in_=xqT_d)
        ident = consts.tile([128, 128], BF16, name="ident")
        make_identity(nc, ident[:])
        sx_s = consts.tile([128, 1], F32, name="sx_s")
        nc.sync.dma_start(out=sx_s[:], in_=sx_d)
        comb_s = consts.tile([128, EPC], F32, name="comb_s")
        nc.sync.dma_start(out=comb_s[:], in_=comb_d)
        ones_s = consts.tile([1, 128], F32, name="ones_s")
        nc.vector.memset(ones_s[:], 1.0)

        def bcast_row(row_ap, dst, width, ename):
            # dst[p, f] = row_ap[0, f] via PE outer product ones^T @ row
            for off in range(0, width, 512):
                sz = min(512, width - off)
                psc = psT_pool.tile([128, 512], F32, tag="psT", name=f"psb_{ename}_{off}")
                nc.tensor.matmul(
                    psc[:, 0:sz],
                    lhsT=ones_s[:],
                    rhs=row_ap[0:1, off : off + sz],
                    start=True,
                    stop=True,
                )
                nc.vector.tensor_copy(out=dst[:, off : off + sz], in_=psc[:, 0:sz])

        # Phase A per expert: GEMM1 + SwiGLU + requant + transpose -> aqT.
        # Phase B per expert: GEMM2 + dequant + y. DMA stream order is
        # w1(e0), w1(e1), w2(e0), w2(e1): the last expert's epilogue chain
        # finishes while w2 still streams, so the kernel tail is only the
        # final chunk's dequant.
        contribs = []
        aqTs, s2cs, W2Ss = [], [], []
        for e in range(EPC):
            # --- per-channel dequant scales, broadcast across partitions ---
            row1 = rows.tile([1, I2], F32, tag="row1", name=f"row1_{e}")
            nc.sync.dma_start(out=row1[:], in_=sc1_d[e : e + 1, :])
            S1 = bcast.tile([128, I2], F32, tag="S1", name=f"S1_{e}")
            bcast_row(row1, S1, I2, f"s1_{e}")
            row2 = rows.tile([1, H], F32, tag="row2", name=f"row2_{e}")
            nc.sync.dma_start(out=row2[:], in_=sc2_d[e : e + 1, :])
            W2S = w2sp.tile([128, H], F32, tag="W2S", name=f"W2S_{e}")
            bcast_row(row2, W2S, H, f"w2s_{e}")
            W2Ss.append(W2S)

            # --- GEMM1: h = xq @ w1[e]  (token-stationary; k-outer) ---
            deqs = []
            for half in range(2):
                ps1 = ps1_pool.tile([128, I], F32, tag="ps1", name=f"ps1_{e}_{half}")
                for g in range(KH // KG1):
                    w1s = w1_pool.tile(
                        [128, KG1, I], BF16, tag="w1s", name=f"w1s_{e}_{half}_{g}"
                    )
                    src = w1_d[e, half, g * KG1 : (g + 1) * KG1].rearrange(
                        "j p f -> p j f"
                    )
                    if VARIANT == "int8":
                        nc.gpsimd.dma_start(out=w1s[:], in_=src)
                    else:
                        nc.sync.dma_start(out=w1s[:], in_=src)
                    for j in range(KG1):
                        k = g * KG1 + j
                        for off, sz in N1_CHUNKS:
                            nc.tensor.matmul(
                                ps1[:, off : off + sz],
                                lhsT=xqT_s[:, k * 128 : (k + 1) * 128],
                                rhs=w1s[:, j, off : off + sz],
                                start=(k == 0),
                                stop=(k == KH - 1),
                            )
                # dequant: (psum * sx[b]) * S1[chan]
                deq = actp.tile([128, I], F32, tag=f"deq{half}", name=f"deq_{e}_{half}")
                nc.vector.scalar_tensor_tensor(
                    out=deq[:],
                    in0=ps1[:],
                    scalar=sx_s[:, 0:1],
                    in1=S1[:, half * I : (half + 1) * I],
                    op0=mult,
                    op1=mult,
                )
                deqs.append(deq)
            gate_deq, up_deq = deqs

            # --- SwiGLU: act = gate * sigmoid(gate) * up (smooth folded into up scale) ---
            sig = actp.tile([128, I], F32, tag="sig", name=f"sig_{e}")
            nc.scalar.activation(
                out=sig[:], in_=gate_deq[:], func=mybir.ActivationFunctionType.Sigmoid
            )
            gsig = actp.tile([128, I], F32, tag="gsig", name=f"gsig_{e}")
            nc.vector.tensor_tensor(out=gsig[:], in0=gate_deq[:], in1=sig[:], op=mult)
            act = actp.tile([128, I], F32, tag="sig", name=f"act_{e}")
            nc.vector.tensor_tensor(out=act[:], in0=gsig[:], in1=up_deq[:], op=mult)

            # --- dynamic requant: aq = round(act * 127 / max|act|) ---
            m = stats.tile([128, 1], F32, tag="m", name=f"m_{e}")
            nc.vector.reduce_max(
                out=m[:], in_=act[:], axis=mybir.AxisListType.X,
                apply_absolute_value=True,
            )
            mc = stats.tile([128, 1], F32, tag="mc", name=f"mc_{e}")
            nc.vector.tensor_scalar_max(out=mc[:], in0=m[:], scalar1=1e-12)
            r = stats.tile([128, 1], F32, tag="r", name=f"r_{e}")
            nc.vector.reciprocal(out=r[:], in_=mc[:])
            r127 = stats.tile([128, 1], F32, tag="r127", name=f"r127_{e}")
            nc.vector.tensor_scalar_mul(out=r127[:], in0=r[:], scalar1=127.0)
            # s2c = (mc/127) * comb[:, e]
            s2c = stats.tile([128, 1], F32, tag="s2c", name=f"s2c_{e}")
            nc.vector.scalar_tensor_tensor(
                out=s2c[:],
                in0=mc[:],
                scalar=1.0 / 127.0,
                in1=comb_s[:, e : e + 1],
                op0=mult,
                op1=mult,
            )
            # magic-constant round-to-nearest-even, output exact ints in bf16
            t = actp.tile([128, I], F32, tag="deq0", name=f"t_{e}")
            nc.scalar.activation(
                out=t[:],
                in_=act[:],
                func=mybir.ActivationFunctionType.Copy,
                bias=MAGIC,
                scale=r127[:, 0:1],
            )
            aq = actp.tile([128, I], BF16, tag="aq", name=f"aq_{e}")
            nc.vector.tensor_scalar_add(out=aq[:], in0=t[:], scalar1=-MAGIC)

            # --- transpose aq -> aqT (I on partitions) via PE transpose ---
            aqT = aqTp.tile([128, KI * 128], BF16, tag="aqT", name=f"aqT_{e}")
            for k in range(KI):
                psT = psT_pool.tile([128, 128], BF16, tag="psT", name=f"psT_{e}_{k}")
                nc.tensor.transpose(
                    psT[:], aq[:, k * 128 : (k + 1) * 128], ident[:]
                )
                nc.vector.tensor_copy(
                    out=aqT[:, k * 128 : (k + 1) * 128], in_=psT[:]
                )
            aqTs.append(aqT)
            s2cs.append(s2c)
            if debug_taps:
                nc.gpsimd.dma_start(out=taps[f"dbg_deq0_{e}"], in_=gate_deq[:])
                nc.gpsimd.dma_start(out=taps[f"dbg_deq1_{e}"], in_=up_deq[:])
                nc.gpsimd.dma_start(out=taps[f"dbg_act_{e}"], in_=act[:])
                nc.gpsimd.dma_start(out=taps[f"dbg_mc_{e}"], in_=mc[:])
                nc.gpsimd.dma_start(out=taps[f"dbg_aq_{e}"], in_=aq[:])
                nc.gpsimd.dma_start(out=taps[f"dbg_aqT_{e}"], in_=aqT[:])

        for e in range(EPC):
            aqT, s2c, W2S = aqTs[e], s2cs[e], W2Ss[e]
            # --- GEMM2: o = aq @ w2[e] ---
            # k-outer so each streamed w2 tile is consumed on arrival; the
            # LAST k-group runs chunk-major so per-chunk dequant + y output
            # pipeline into the final matmuls (short kernel tail).
            if e == 0:
                contrib0 = yp.tile([128, H], F32, name="contrib0")
                contribs.append(contrib0)
            ps2c = {
                off: ps2_pool.tile(
                    [128, 512], F32, tag=f"ps2_{off}", name=f"ps2_{e}_{off}"
                )
                for off, _ in N2_CHUNKS
            }
            n_groups = len(W2_GROUPS)
            for g, (g0, gn) in enumerate(W2_GROUPS):
                w2s = w2_pool.tile([128, 2, H], BF16, tag="w2s", name=f"w2s_{e}_{g0}")
                src = w2_d[e, g0 : g0 + gn].rearrange("j p f -> p j f")
                if VARIANT == "int8":
                    nc.gpsimd.dma_start(out=w2s[:, 0:gn, :], in_=src)
                else:
                    nc.sync.dma_start(out=w2s[:, 0:gn, :], in_=src)
                last_group = g == n_groups - 1

                def mm2(j, off, sz):
                    k = g0 + j
                    nc.tensor.matmul(
                        ps2c[off][:, 0:sz],
                        lhsT=aqT[:, k * 128 : (k + 1) * 128],
                        rhs=w2s[:, j, off : off + sz],
                        start=(k == 0),
                        stop=(k == KI - 1),
                    )

                if not last_group:
                    for j in range(gn):
                        for off, sz in N2_CHUNKS:
                            mm2(j, off, sz)
                else:
                    # all but the final k-tile in normal (k-minor) order
                    for j in range(gn - 1):
                        for off, sz in N2_CHUNKS:
                            mm2(j, off, sz)
                    # final k-tile chunk-major: each chunk completes in turn
                    # (single LDWEIGHTS: lhsT is fixed at k = KI-1)
                    for off, sz in N2_CHUNKS:
                        mm2(gn - 1, off, sz)
                        # chunk complete: dequant + combine weight
                        if e == 0:
                            nc.vector.scalar_tensor_tensor(
                                out=contrib0[:, off : off + sz],
                                in0=ps2c[off][:, 0:sz],
                                scalar=s2c[:, 0:1],
                                in1=W2S[:, off : off + sz],
                                op0=mult,
                                op1=mult,
                            )
                        else:
                            tmpc = ychunkp.tile(
                                [128, 512], F32, tag="tmpc", name=f"tmpc_{off}"
                            )
                            nc.vector.scalar_tensor_tensor(
                                out=tmpc[:, 0:sz],
                                in0=ps2c[off][:, 0:sz],
                                scalar=s2c[:, 0:1],
                                in1=W2S[:, off : off + sz],
                                op0=mult,
                                op1=mult,
                            )
                            youtc = ychunkp.tile(
                                [128, 512], F32, tag="youtc", name=f"youtc_{off}"
                            )
                            nc.vector.tensor_tensor(
                                out=youtc[:, 0:sz],
                                in0=contribs[0][:, off : off + sz],
                                in1=tmpc[:, 0:sz],
                                op=mybir.AluOpType.add,
                            )
                            nc.sync.dma_start(
                                out=y_d[:, off : off + sz], in_=youtc[:, 0:sz]
                            )

    nc.compile()
    return nc


def get_program(debug_taps=False):
    key = ("nc", debug_taps)
    if key not in _cache:
        _cache[key] = _build_program(debug_taps=debug_taps)
    return _cache[key]


def _prep_inputs(x, expert_ids, smooth_scales, expert_scales, w1, w1_scale, w2, w2_scale):
    """Host-side dispatch: quantize x, build combine matrix, shard experts."""
    x = np.asarray(x, np.float32)
    expert_ids = np.asarray(expert_ids)
    smooth_scales = np.asarray(smooth_scales, np.float32)
    expert_scales = np.asarray(expert_scales, np.float32)
    w1_scale = np.asarray(w1_scale, np.float32)
    w2_scale = np.asarray(w2_scale, np.float32)

    # dynamic per-token int8 quantization (exact mirror of reference ops)
    sx = np.maximum(np.max(np.abs(x), axis=-1, keepdims=True), 1e-12) / 127.0
    xq = np.round(np.clip(x / sx, -128.0, 127.0)).astype(np.float32)  # ints

    # xqT tiled [p, k*128 + b] = xq[b, k*128 + p]
    xqT = np.ascontiguousarray(xq.T)  # [H, B]
    xqT_t = np.ascontiguousarray(
        xqT.reshape(KH, 128, B).transpose(1, 0, 2).reshape(128, KH * B)
    ).astype(ml_dtypes.bfloat16)

    # combine matrix [B, E]: scatter-add expert_scales at expert_ids
    comb = np.zeros((B, E), np.float32)
    np.add.at(comb, (np.arange(B)[:, None], expert_ids), expert_scales)

    w1v = w1.astype(np.int8)  # int8-valued
    w2v = w2.astype(np.int8)
    wdt = np.int8 if VARIANT == "int8" else ml_dtypes.bfloat16

    in_maps = []
    for c in range(NCORES):
        es = list(range(c * EPC, (c + 1) * EPC))
        # w1 [e, H, 2I] -> [e, half, k, p, f] bf16
        w1c = w1v[es].reshape(EPC, KH, 128, I2)
        w1gu = np.stack([w1c[..., :I], w1c[..., I:]], axis=1)  # [e,2,k,p,I]
        w1_bf = np.ascontiguousarray(w1gu).astype(wdt)
        w2_bf = np.ascontiguousarray(
            w2v[es].reshape(EPC, KI, 128, H)
        ).astype(wdt)
        # dequant scale rows; smooth folded into the up half
        sc1 = np.concatenate(
            [w1_scale[es][:, :I], w1_scale[es][:, I:] * smooth_scales[es]], axis=1
        ).astype(np.float32)
        sc2 = w2_scale[es].astype(np.float32)
        in_maps.append(
            {
                "xqT": xqT_t,
                "sx": sx.astype(np.float32),
                "combs": np.ascontiguousarray(comb[:, es]).astype(np.float32),
                "w1t": w1_bf,
                "w2t": w2_bf,
                "scale1": sc1,
                "scale2": sc2,
            }
        )
    return in_maps


def kernel(
    x,
    expert_ids,
    smooth_scales,
    expert_scales,
    x_active_mask,
    w1,
    w1_scale,
    w2,
    w2_scale,
    _trace=False,
    _trace_kwargs=None,
):
    in_maps = _prep_inputs(
        x, expert_ids, smooth_scales, expert_scales, w1, w1_scale, w2, w2_scale
    )
    nc = get_program()
    res = run_bass_kernel_spmd(
        nc,
        in_maps,
        core_ids=list(range(NCORES)),
        trace=_trace,
        **(_trace_kwargs or {}),
    )
    y = np.zeros((B, H), np.float32)
    for r in res.results:
        y += r["y"]
    y *= np.asarray(x_active_mask).astype(np.float32)[:, None]
    if _trace:
        kernel.last_results = res
    return y



# revision 17
# speedup vs baseline: 1.0880x; 1.0880x over previous
"""Trainium2 Bass kernel for nn_DecodeMoeOps (MoE decode: dispatch-quant,
grouped int8 GEMM1, SwiGLU, requant, grouped int8 GEMM2, weighted combine).

Expert-parallel across 8 NeuronCores: core c owns experts {2c, 2c+1}.

Key design (v2):
- Weights ship to SBUF as RAW INT8 (1 B/weight over HWDGE) and are upcast
  to bf16 on-chip, split across the DVE / ACT / GPSIMD engines. This halves
  the DMA-device byte volume vs casting during the DMA (which is charged at
  bf16 output bytes).
- Both GEMMs run WEIGHT-STATIONARY (weights are the PE's lhsT), so PE time
  scales with the number of routed tokens, not the weight volume. Each
  expert gets a fixed 64-position block of gathered tokens (host routing);
  tokens routed to both of a core's experts appear in both blocks and the
  host scatter-adds per-position outputs back to token rows.
- GEMM1 output lands channel-major [ch, tok]; SwiGLU/requant run in that
  layout (cross-partition absmax via gpsimd.partition_all_reduce), which
  makes the requantized activations directly usable as GEMM2's moving
  operand with no transposes. Final [h, pos] -> [pos, h] via PE transpose.
"""

import os
import sys

for _p in ("/opt/trn_rl_repo", "/root/.axon_site/_ro/trn_rl_repo"):
    if os.path.isdir(_p) and _p not in sys.path:
        sys.path.insert(0, _p)

from contextlib import ExitStack

import ml_dtypes
import numpy as np

import concourse.bass as bass
import concourse.bass_isa as bass_isa
import concourse.mybir as mybir
import concourse.tile as tile
from concourse import bacc
from concourse.bass_utils import run_bass_kernel_spmd
from concourse.masks import make_identity

B, TOPK, H, I, E = 128, 8, 2048, 1408, 16
NCORES = 8
EPC = E // NCORES  # experts per core
KH = H // 128  # 16 contraction tiles for GEMM1
KI = I // 128  # 11 contraction tiles for GEMM2
I2 = 2 * I
CT1 = I2 // 128  # 22 GEMM1 output-channel tiles (gate 0..10, up 11..21)
CT2 = H // 128  # 16 GEMM2 output-channel tiles
NT = 64  # token positions per expert block
NPOS = EPC * NT  # 128 positions per core
F32 = mybir.dt.float32
BF16 = mybir.dt.bfloat16
I8 = mybir.dt.int8
MAGIC = float(3 * 2**22)  # fp32 round-to-nearest-int magic (covers negatives)

# int8 -> bf16 upcast split points (free-dim columns) per engine:
# [0:V) on DVE, [V:A) on ACT, [A:end) on GPSIMD.  Tunables.
W1V, W1A = 1280, 2304  # of I2 = 2816
W2V, W2A = 224, 400  # of 512-wide w2 column chunks

_cache: dict = {}


def _build_program():
    nc = bacc.Bacc(
        "TRN2",
        target_bir_lowering=False,
        debug=False,
        num_devices=NCORES,
    )
    mult = mybir.AluOpType.mult
    opmax = mybir.AluOpType.max

    # --- per-core DRAM I/O ---
    xqT_d = nc.dram_tensor("xqT", [128, KH, NPOS], BF16, kind="ExternalInput").ap()
    sxr_d = nc.dram_tensor("sxrow", [1, NPOS], F32, kind="ExternalInput").ap()
    scr_d = nc.dram_tensor("scrow", [1, NPOS], F32, kind="ExternalInput").ap()
    w1_d = nc.dram_tensor("w1t", [EPC, 128, KH, I2], I8, kind="ExternalInput").ap()
    w2_d = nc.dram_tensor("w2t", [EPC, 128, KI, H], I8, kind="ExternalInput").ap()
    w1sc_d = nc.dram_tensor("w1sc", [EPC, 128, CT1], F32, kind="ExternalInput").ap()
    w2sc_d = nc.dram_tensor("w2sc", [EPC, 128, CT2], F32, kind="ExternalInput").ap()
    y_d = nc.dram_tensor("y", [NPOS, H], F32, kind="ExternalOutput").ap()

    with tile.TileContext(nc) as tc, ExitStack() as ctx:
        consts = ctx.enter_context(tc.tile_pool(name="consts", bufs=1))
        w1i8p = ctx.enter_context(tc.tile_pool(name="w1i8", bufs=4))
        w1bfp = ctx.enter_context(tc.tile_pool(name="w1bf", bufs=2))
        w2i8p = ctx.enter_context(tc.tile_pool(name="w2i8", bufs=4))
        w2bfp = ctx.enter_context(tc.tile_pool(name="w2bf", bufs=2))
        ep = ctx.enter_context(tc.tile_pool(name="ep", bufs=2))
        stats = ctx.enter_context(tc.tile_pool(name="stats", bufs=2))
        aqp = ctx.enter_context(tc.tile_pool(name="aqp", bufs=2))
        yp = ctx.enter_context(tc.tile_pool(name="yp", bufs=1))
        ps1p = ctx.enter_context(tc.tile_pool(name="ps1", bufs=2, space="PSUM"))
        ps2p = ctx.enter_context(tc.tile_pool(name="ps2", bufs=1, space="PSUM"))

        # --- prologue ---
        xqT_s = consts.tile([128, KH, NPOS], BF16, name="xqT_s")
        nc.scalar.dma_start(out=xqT_s[:], in_=xqT_d)
        ident = consts.tile([128, 128], F32, name="ident")
        make_identity(nc, ident[:])
        ones1 = consts.tile([1, 128], F32, name="ones1")
        nc.vector.memset(ones1[:], 1.0)
        sxr_s = consts.tile([1, NPOS], F32, name="sxr_s")
        nc.scalar.dma_start(out=sxr_s[:], in_=sxr_d)
        scr_s = consts.tile([1, NPOS], F32, name="scr_s")
        nc.scalar.dma_start(out=scr_s[:], in_=scr_d)
        w1sc_s, w2sc_s = [], []
        for e in range(EPC):
            t1 = consts.tile([128, CT1], F32, name=f"w1sc_{e}")
            nc.sync.dma_start(out=t1[:], in_=w1sc_d[e])
            w1sc_s.append(t1)
            t2 = consts.tile([128, CT2], F32, name=f"w2sc_{e}")
            nc.sync.dma_start(out=t2[:], in_=w2sc_d[e])
            w2sc_s.append(t2)

        # broadcast the per-position rows across partitions: out[p,t] = row[t]
        # (borrows a ps1-pool buffer; released before GEMM1 writes it)
        psb0 = ps1p.tile([128, 24, NT], F32, tag="ps1", name="psb0")
        sxb = consts.tile([128, NPOS], F32, name="sxb")
        bc0 = psb0[:, 0:2, :].rearrange("p a t -> p (a t)")
        nc.tensor.matmul(bc0, lhsT=ones1[:], rhs=sxr_s[:], start=True, stop=True)
        nc.vector.tensor_copy(out=sxb[:], in_=bc0)
        scb = consts.tile([128, NPOS], F32, name="scb")
        bc1 = psb0[:, 2:4, :].rearrange("p a t -> p (a t)")
        nc.tensor.matmul(bc1, lhsT=ones1[:], rhs=scr_s[:], start=True, stop=True)
        nc.vector.tensor_copy(out=scb[:], in_=bc1)

        # Emission order == per-engine execution order, so the phases below
        # software-pipeline the kernel: each expert's epilogue is emitted in
        # slices spliced between the NEXT phase's chunk pipelines, keeping
        # every engine queue free of long head-of-line dependency waits.
        st = {}  # per-expert tiles carried across phases
        aqs, s2cs = [], []

        def g1_chunk(e, kg):
            if kg == 0:
                # 3 exact PSUM banks; one accumulation group per bank (8
                # chunks share a bank: start on the bank's first chunk, stop
                # on its last -- HW zeroing is lazy per-write in the region).
                st[f"ps1_{e}"] = ps1p.tile([128, 24, NT], F32, tag="ps1", name=f"ps1_{e}")
            ps1 = st[f"ps1_{e}"]
            w1i = w1i8p.tile([128, 4, I2], I8, tag="w1i", name=f"w1i_{e}_{kg}")
            nc.sync.dma_start(out=w1i[:], in_=w1_d[e, :, kg * 4 : (kg + 1) * 4, :])
            w1b = w1bfp.tile([128, 4, I2], BF16, tag="w1b", name=f"w1b_{e}_{kg}")
            for j in range(4):
                nc.vector.tensor_copy(out=w1b[:, j, 0:W1V], in_=w1i[:, j, 0:W1V])
                nc.scalar.copy(out=w1b[:, j, W1V:W1A], in_=w1i[:, j, W1V:W1A])
                nc.gpsimd.tensor_copy(out=w1b[:, j, W1A:I2], in_=w1i[:, j, W1A:I2])
            for j in range(4):
                k = kg * 4 + j
                for c in range(CT1):
                    nc.tensor.matmul(
                        ps1[:, c, :],
                        lhsT=w1b[:, j, c * 128 : (c + 1) * 128],
                        rhs=xqT_s[:, k, e * NT : (e + 1) * NT],
                        start=(k == 0 and c % 8 == 0),
                        stop=(k == KH - 1 and (c % 8 == 7 or c == CT1 - 1)),
                    )

        def epi(e, s):
            """Epilogue slice s (1..4) for expert e."""
            if s == 1:
                # dequant: releases ps1
                ps1 = st[f"ps1_{e}"]
                sxb64 = sxb[:, e * NT : (e + 1) * NT]
                gate = st[f"gate_{e}"] = ep.tile(
                    [128, KI, NT], F32, tag="gate", name=f"gate_{e}"
                )
                up = st[f"up_{e}"] = ep.tile(
                    [128, KI, NT], F32, tag="up", name=f"up_{e}"
                )
                for c in range(KI):
                    # gate = psum * w1sc[ch] * sx[tok]
                    nc.vector.scalar_tensor_tensor(
                        out=gate[:, c, :],
                        in0=ps1[:, c, :],
                        scalar=w1sc_s[e][:, c : c + 1],
                        in1=sxb64,
                        op0=mult,
                        op1=mult,
                    )
                for c in range(KI):
                    # up = psum * (w1sc*smooth)[ch]  (sx deferred into s2c:
                    # aq is invariant to per-token scaling of act)
                    nc.scalar.activation(
                        out=up[:, c, :],
                        in_=ps1[:, KI + c, :],
                        func=mybir.ActivationFunctionType.Copy,
                        scale=w1sc_s[e][:, KI + c : KI + c + 1],
                    )
            elif s == 2:
                gate, up = st[f"gate_{e}"], st[f"up_{e}"]
                sig = ep.tile([128, KI, NT], F32, tag="sig", name=f"sig_{e}")
                nc.scalar.activation(
                    out=sig[:], in_=gate[:], func=mybir.ActivationFunctionType.Sigmoid
                )
                gsig = ep.tile([128, KI, NT], F32, tag="gsig", name=f"gsig_{e}")
                nc.vector.tensor_tensor(out=gsig[:], in0=gate[:], in1=sig[:], op=mult)
                act = st[f"act_{e}"] = ep.tile(
                    [128, KI, NT], F32, tag="act", name=f"act_{e}"
                )
                nc.vector.tensor_tensor(out=act[:], in0=gsig[:], in1=up[:], op=mult)
            elif s == 3:
                act = st[f"act_{e}"]
                # per-token absmax over all I channels (partitions x 11 tiles)
                mall = st[f"mall_{e}"] = ep.tile(
                    [128, KI, NT], F32, tag="mall", name=f"mall_{e}"
                )
                nc.gpsimd.partition_all_reduce(
                    mall[:].rearrange("p j t -> p (j t)"),
                    act[:].rearrange("p j t -> p (j t)"),
                    128,
                    bass_isa.ReduceOp.absmax,
                )
                mfin = stats.tile([128, NT], F32, tag="mfin", name=f"mfin_{e}")
                nc.vector.reduce_max(
                    out=mfin[:].unsqueeze(2),
                    in_=mall[:].rearrange("p j t -> p t j"),
                    axis=mybir.AxisListType.X,
                )
                mc = stats.tile([128, NT], F32, tag="mc", name=f"mc_{e}")
                nc.vector.tensor_scalar_max(out=mc[:], in0=mfin[:], scalar1=1e-12)
                rr = stats.tile([128, NT], F32, tag="rr", name=f"rr_{e}")
                nc.vector.reciprocal(out=rr[:], in_=mc[:])
                r127 = st[f"r127_{e}"] = stats.tile(
                    [128, NT], F32, tag="r127", name=f"r127_{e}"
                )
                nc.vector.tensor_scalar_mul(out=r127[:], in0=rr[:], scalar1=127.0)
                # s2c = (mc/127) * sx[tok] * comb[tok]  (sx folded in scrow)
                s2c = stats.tile([128, NT], F32, tag="s2c", name=f"s2c_{e}")
                nc.vector.scalar_tensor_tensor(
                    out=s2c[:],
                    in0=mc[:],
                    scalar=1.0 / 127.0,
                    in1=scb[:, e * NT : (e + 1) * NT],
                    op0=mult,
                    op1=mult,
                )
                s2cs.append(s2c)
            else:
                act, r127 = st[f"act_{e}"], st[f"r127_{e}"]
                tq = ep.tile([128, KI, NT], F32, tag="gate", name=f"tq_{e}")
                nc.vector.tensor_tensor(
                    out=tq[:],
                    in0=act[:],
                    in1=r127[:].unsqueeze(1).broadcast_to([128, KI, NT]),
                    op=mult,
                )
                trnd = ep.tile([128, KI, NT], F32, tag="mall", name=f"trnd_{e}")
                nc.vector.tensor_scalar_add(out=trnd[:], in0=tq[:], scalar1=MAGIC)
                aq = aqp.tile([128, KI, NT], BF16, tag="aq", name=f"aq_{e}")
                nc.vector.tensor_scalar_add(out=aq[:], in0=trnd[:], scalar1=-MAGIC)
                aqs.append(aq)

        def w2dma(e, g):
            w2i = st[f"w2i_{e}_{g}"] = w2i8p.tile(
                [128, KI, 512], I8, tag="w2i", name=f"w2i_{e}_{g}"
            )
            nc.sync.dma_start(out=w2i[:], in_=w2_d[e, :, :, g * 512 : (g + 1) * 512])

        def w2cast(e, g):
            w2i = st[f"w2i_{e}_{g}"]
            w2b = st[f"w2b_{e}_{g}"] = w2bfp.tile(
                [128, KI, 512], BF16, tag="w2b", name=f"w2b_{e}_{g}"
            )
            nc.vector.tensor_copy(out=w2b[:, :, 0:W2V], in_=w2i[:, :, 0:W2V])
            wam = (W2V + W2A) // 2
            nc.scalar.copy(out=w2b[:, :, W2V:wam], in_=w2i[:, :, W2V:wam])
            nc.scalar.copy(out=w2b[:, :, wam:W2A], in_=w2i[:, :, wam:W2A])
            wpm = (W2A + 512) // 2
            nc.gpsimd.tensor_copy(out=w2b[:, :, W2A:wpm], in_=w2i[:, :, W2A:wpm])
            nc.gpsimd.tensor_copy(out=w2b[:, :, wpm:512], in_=w2i[:, :, wpm:512])

        def g2mm(e, g):
            w2b = st[f"w2b_{e}_{g}"]
            # one full bank per tag; single accumulation group per bank
            ps2 = ps2p.tile([128, 8, NT], F32, tag=f"ps2{e}", name=f"ps2_{e}_{g}")
            for cc in range(4):
                for k in range(KI):
                    nc.tensor.matmul(
                        ps2[:, cc, :],
                        lhsT=w2b[:, k, cc * 128 : (cc + 1) * 128],
                        rhs=aqs[e][:, k, :],
                        start=(k == 0 and cc == 0),
                        stop=(k == KI - 1 and cc == 3),
                    )
            for cc in range(4):
                c = g * 4 + cc
                # deq2: o * w2sc[h] * (s2 * comb)[tok] -> yT columns
                nc.vector.scalar_tensor_tensor(
                    out=yT[:, c, e * NT : (e + 1) * NT],
                    in0=ps2[:, cc, :],
                    scalar=w2sc_s[e][:, c : c + 1],
                    in1=s2cs[e][:],
                    op0=mult,
                    op1=mult,
                )

        def ytail(g):
            # both experts done for these channel tiles: emit y rows
            if "pst" not in st:
                st["pst"] = ps1p.tile([128, 24, NT], F32, tag="ps1", name="pst")
            pst = st["pst"]
            views = [
                pst[:, 2 * cc : 2 * cc + 2, :].rearrange("p a t -> p (a t)")
                for cc in range(4)
            ]
            for cc in range(4):
                c = g * 4 + cc
                nc.tensor.transpose(views[cc], yT[:, c, :], ident[:])
            for cc in range(4):
                c = g * 4 + cc
                if cc < 2:
                    nc.vector.tensor_copy(
                        out=ysb[:, c * 128 : (c + 1) * 128], in_=views[cc]
                    )
                else:
                    nc.scalar.copy(
                        out=ysb[:, c * 128 : (c + 1) * 128], in_=views[cc]
                    )
            nc.sync.dma_start(
                out=y_d[:, g * 512 : (g + 1) * 512],
                in_=ysb[:, g * 512 : (g + 1) * 512],
            )

        yT = yp.tile([128, CT2, NPOS], F32, name="yT")
        ysb = yp.tile([128, H], F32, name="ysb")

        for kg in range(4):
            g1_chunk(0, kg)
            g1_chunk(1, kg)
        for s in range(1, 5):
            epi(0, s)
        for e in range(EPC):
            for g in range(4):
                w2dma(e, g)
        for s in range(1, 5):
            epi(1, s)
        for g in range(4):
            w2cast(0, g)
            g2mm(0, g)
        for g in range(4):
            w2cast(1, g)
            g2mm(1, g)
            ytail(g)

    nc.compile()
    return nc


def get_program():
    if "nc" not in _cache:
        _cache["nc"] = _build_program()
    return _cache["nc"]


def _prep_inputs(x, expert_ids, smooth_scales, expert_scales, w1, w1_scale, w2, w2_scale):
    """Host-side dispatch: quantize x, route tokens, shard experts."""
    x = np.asarray(x, np.float32)
    expert_ids = np.asarray(expert_ids)
    smooth_scales = np.asarray(smooth_scales, np.float32)
    expert_scales = np.asarray(expert_scales, np.float32)
    w1_scale = np.asarray(w1_scale, np.float32)
    w2_scale = np.asarray(w2_scale, np.float32)

    # dynamic per-token int8 quantization (exact mirror of reference ops)
    sx = np.maximum(np.max(np.abs(x), axis=-1, keepdims=True), 1e-12) / 127.0
    xq = np.round(np.clip(x / sx, -128.0, 127.0)).astype(np.float32)

    # combine matrix [B, E]: scatter-add expert_scales at expert_ids
    comb = np.zeros((B, E), np.float32)
    np.add.at(comb, (np.arange(B)[:, None], expert_ids), expert_scales)

    w1v = w1.astype(np.int8)
    w2v = w2.astype(np.int8)

    in_maps = []
    pos2tok = np.zeros((NCORES, NPOS), np.int64)
    for c in range(NCORES):
        valid = np.zeros(NPOS, bool)
        ge_of_pos = np.zeros(NPOS, np.int64)
        for le in range(EPC):
            ge = EPC * c + le
            toks = np.nonzero((expert_ids == ge).any(axis=1))[0]
            assert len(toks) <= NT, f"expert {ge}: {len(toks)} tokens > capacity {NT}"
            pos2tok[c, le * NT : le * NT + len(toks)] = toks
            valid[le * NT : le * NT + len(toks)] = True
            ge_of_pos[le * NT : (le + 1) * NT] = ge

        toks_c = pos2tok[c]
        xg = xq[toks_c]  # [NPOS, H]
        xqT = np.ascontiguousarray(
            xg.T.reshape(KH, 128, NPOS).transpose(1, 0, 2)
        ).astype(ml_dtypes.bfloat16)
        sxrow = sx[toks_c, 0][None, :].astype(np.float32)
        combp = comb[toks_c, ge_of_pos] * valid  # zero at padding positions
        scrow = (sx[toks_c, 0] * combp)[None, :].astype(np.float32)

        es = list(range(EPC * c, EPC * (c + 1)))
        w1t = np.ascontiguousarray(
            w1v[es].reshape(EPC, KH, 128, I2).transpose(0, 2, 1, 3)
        )
        w2t = np.ascontiguousarray(
            w2v[es].reshape(EPC, KI, 128, H).transpose(0, 2, 1, 3)
        )
        w1sc = np.empty((EPC, 128, CT1), np.float32)
        w2sc = np.empty((EPC, 128, CT2), np.float32)
        for le, ge in enumerate(es):
            gatesc = w1_scale[ge, :I].reshape(KI, 128).T
            upsc = (w1_scale[ge, I:] * smooth_scales[ge]).reshape(KI, 128).T
            w1sc[le] = np.concatenate([gatesc, upsc], axis=1)
            w2sc[le] = w2_scale[ge].reshape(CT2, 128).T

        in_maps.append(
            {
                "xqT": xqT,
                "sxrow": sxrow,
                "scrow": scrow,
                "w1t": w1t,
                "w2t": w2t,
                "w1sc": w1sc,
                "w2sc": w2sc,
            }
        )
    return in_maps, pos2tok


def kernel(
    x,
    expert_ids,
    smooth_scales,
    expert_scales,
    x_active_mask,
    w1,
    w1_scale,
    w2,
    w2_scale,
    _trace=False,
    _trace_kwargs=None,
):
    in_maps, pos2tok = _prep_inputs(
        x, expert_ids, smooth_scales, expert_scales, w1, w1_scale, w2, w2_scale
    )
    nc = get_program()
    res = run_bass_kernel_spmd(
        nc,
        in_maps,
        core_ids=list(range(NCORES)),
        trace=_trace,
        **(_trace_kwargs or {}),
    )
    y = np.zeros((B, H), np.float32)
    for c, r in enumerate(res.results):
        np.add.at(y, pos2tok[c], r["y"].astype(np.float32))
    y *= np.asarray(x_active_mask).astype(np.float32)[:, None]
    if _trace:
        kernel.last_results = res
    return y


# revision 18
# speedup vs baseline: 1.2733x; 1.1703x over previous
"""Trainium2 Bass kernel for nn_DecodeMoeOps (MoE decode: dispatch-quant,
grouped int8 GEMM1, SwiGLU, requant, grouped int8 GEMM2, weighted combine).

Expert-parallel across 8 NeuronCores: core c owns experts {2c, 2c+1}.

Key design (v2):
- Weights ship to SBUF as RAW INT8 (1 B/weight over HWDGE) and are upcast
  to bf16 on-chip, split across the DVE / ACT / GPSIMD engines. This halves
  the DMA-device byte volume vs casting during the DMA (which is charged at
  bf16 output bytes).
- Both GEMMs run WEIGHT-STATIONARY (weights are the PE's lhsT), so PE time
  scales with the number of routed tokens, not the weight volume. Each
  expert gets a fixed 64-position block of gathered tokens (host routing);
  tokens routed to both of a core's experts appear in both blocks and the
  host scatter-adds per-position outputs back to token rows.
- GEMM1 output lands channel-major [ch, tok]; SwiGLU/requant run in that
  layout (cross-partition absmax via gpsimd.partition_all_reduce), which
  makes the requantized activations directly usable as GEMM2's moving
  operand with no transposes. Final [h, pos] -> [pos, h] via PE transpose.
"""

import os
import sys

for _p in ("/opt/trn_rl_repo", "/root/.axon_site/_ro/trn_rl_repo"):
    if os.path.isdir(_p) and _p not in sys.path:
        sys.path.insert(0, _p)

from contextlib import ExitStack

import ml_dtypes
import numpy as np

import concourse.bass as bass
import concourse.bass_isa as bass_isa
import concourse.mybir as mybir
import concourse.tile as tile
from concourse import bacc
from concourse.bass_utils import run_bass_kernel_spmd
from concourse.masks import make_identity

B, TOPK, H, I, E = 128, 8, 2048, 1408, 16
NCORES = 8
EPC = E // NCORES  # experts per core
KH = H // 128  # 16 contraction tiles for GEMM1
KI = I // 128  # 11 contraction tiles for GEMM2
I2 = 2 * I
CT1 = I2 // 128  # 22 GEMM1 output-channel tiles (gate 0..10, up 11..21)
CT2 = H // 128  # 16 GEMM2 output-channel tiles
NT = 64  # token positions per expert block
NPOS = EPC * NT  # 128 positions per core
F32 = mybir.dt.float32
BF16 = mybir.dt.bfloat16
I8 = mybir.dt.int8
MAGIC = float(3 * 2**22)  # fp32 round-to-nearest-int magic (covers negatives)

# int8 -> bf16 upcast split points (free-dim columns) per engine:
# [0:V) on DVE, [V:A) on ACT, [A:end) on GPSIMD.  Tunables.
W1V, W1A = 1280, 2304  # of I2 = 2816
W2V, W2A = 224, 400  # of 512-wide w2 column chunks

_cache: dict = {}


def _build_program():
    nc = bacc.Bacc(
        "TRN2",
        target_bir_lowering=False,
        debug=False,
        num_devices=NCORES,
    )
    mult = mybir.AluOpType.mult
    opmax = mybir.AluOpType.max

    # --- per-core DRAM I/O ---
    xqT_d = nc.dram_tensor("xqT", [128, KH, NPOS], BF16, kind="ExternalInput").ap()
    sxr_d = nc.dram_tensor("sxrow", [1, NPOS], F32, kind="ExternalInput").ap()
    scr_d = nc.dram_tensor("scrow", [1, NPOS], F32, kind="ExternalInput").ap()
    w1_d = nc.dram_tensor("w1t", [EPC, 128, KH, I2], I8, kind="ExternalInput").ap()
    w2_d = nc.dram_tensor("w2t", [EPC, 128, KI, H], I8, kind="ExternalInput").ap()
    w1sc_d = nc.dram_tensor("w1sc", [EPC, 128, CT1], F32, kind="ExternalInput").ap()
    w2sc_d = nc.dram_tensor("w2sc", [EPC, 128, CT2], F32, kind="ExternalInput").ap()
    y_d = nc.dram_tensor("y", [NPOS, H], F32, kind="ExternalOutput").ap()

    with tile.TileContext(nc) as tc, ExitStack() as ctx:
        consts = ctx.enter_context(tc.tile_pool(name="consts", bufs=1))
        w1i8p = ctx.enter_context(tc.tile_pool(name="w1i8", bufs=4))
        w1bfp = ctx.enter_context(tc.tile_pool(name="w1bf", bufs=2))
        w2i8p = ctx.enter_context(tc.tile_pool(name="w2i8", bufs=4))
        w2bfp = ctx.enter_context(tc.tile_pool(name="w2bf", bufs=2))
        ep = ctx.enter_context(tc.tile_pool(name="ep", bufs=2))
        stats = ctx.enter_context(tc.tile_pool(name="stats", bufs=2))
        aqp = ctx.enter_context(tc.tile_pool(name="aqp", bufs=2))
        yp = ctx.enter_context(tc.tile_pool(name="yp", bufs=1))
        ps1p = ctx.enter_context(tc.tile_pool(name="ps1", bufs=2, space="PSUM"))
        ps2p = ctx.enter_context(tc.tile_pool(name="ps2", bufs=1, space="PSUM"))

        # --- prologue ---
        xqT_s = consts.tile([128, KH, NPOS], BF16, name="xqT_s")
        nc.scalar.dma_start(out=xqT_s[:], in_=xqT_d)
        ident = consts.tile([128, 128], F32, name="ident")
        make_identity(nc, ident[:])
        ones1 = consts.tile([1, 128], F32, name="ones1")
        nc.vector.memset(ones1[:], 1.0)
        sxr_s = consts.tile([1, NPOS], F32, name="sxr_s")
        nc.scalar.dma_start(out=sxr_s[:], in_=sxr_d)
        scr_s = consts.tile([1, NPOS], F32, name="scr_s")
        nc.scalar.dma_start(out=scr_s[:], in_=scr_d)
        w1sc_s, w2sc_s = [], []
        for e in range(EPC):
            t1 = consts.tile([128, CT1], F32, name=f"w1sc_{e}")
            nc.sync.dma_start(out=t1[:], in_=w1sc_d[e])
            w1sc_s.append(t1)
            t2 = consts.tile([128, CT2], F32, name=f"w2sc_{e}")
            nc.sync.dma_start(out=t2[:], in_=w2sc_d[e])
            w2sc_s.append(t2)

        # broadcast the per-position rows across partitions: out[p,t] = row[t]
        # (borrows a ps1-pool buffer; released before GEMM1 writes it)
        psb0 = ps1p.tile([128, 24, NT], F32, tag="ps1", name="psb0")
        sxb = consts.tile([128, NPOS], F32, name="sxb")
        bc0 = psb0[:, 0:2, :].rearrange("p a t -> p (a t)")
        nc.tensor.matmul(bc0, lhsT=ones1[:], rhs=sxr_s[:], start=True, stop=True)
        nc.vector.tensor_copy(out=sxb[:], in_=bc0)
        scb = consts.tile([128, NPOS], F32, name="scb")
        bc1 = psb0[:, 2:4, :].rearrange("p a t -> p (a t)")
        nc.tensor.matmul(bc1, lhsT=ones1[:], rhs=scr_s[:], start=True, stop=True)
        nc.vector.tensor_copy(out=scb[:], in_=bc1)

        # Emission order == per-engine execution order, so the phases below
        # software-pipeline the kernel: each expert's epilogue is emitted in
        # slices spliced between the NEXT phase's chunk pipelines, keeping
        # every engine queue free of long head-of-line dependency waits.
        st = {}  # per-expert tiles carried across phases
        aqs, s2cs = [], []

        def g1_chunk(e, kg):
            if kg == 0:
                # 3 exact PSUM banks; one accumulation group per bank (8
                # chunks share a bank: start on the bank's first chunk, stop
                # on its last -- HW zeroing is lazy per-write in the region).
                st[f"ps1_{e}"] = ps1p.tile([128, 24, NT], F32, tag="ps1", name=f"ps1_{e}")
            ps1 = st[f"ps1_{e}"]
            w1i = w1i8p.tile([128, 4, I2], I8, tag="w1i", name=f"w1i_{e}_{kg}")
            nc.sync.dma_start(out=w1i[:], in_=w1_d[e, :, kg * 4 : (kg + 1) * 4, :])
            w1b = w1bfp.tile([128, 4, I2], BF16, tag="w1b", name=f"w1b_{e}_{kg}")
            for j in range(4):
                nc.vector.tensor_copy(out=w1b[:, j, 0:W1V], in_=w1i[:, j, 0:W1V])
                nc.scalar.copy(out=w1b[:, j, W1V:W1A], in_=w1i[:, j, W1V:W1A])
                nc.gpsimd.tensor_copy(out=w1b[:, j, W1A:I2], in_=w1i[:, j, W1A:I2])
            for j in range(4):
                k = kg * 4 + j
                for c in range(CT1):
                    nc.tensor.matmul(
                        ps1[:, c, :],
                        lhsT=w1b[:, j, c * 128 : (c + 1) * 128],
                        rhs=xqT_s[:, k, e * NT : (e + 1) * NT],
                        start=(k == 0 and c % 8 == 0),
                        stop=(k == KH - 1 and (c % 8 == 7 or c == CT1 - 1)),
                    )

        def epi(e, s):
            """Epilogue slice s (1..4) for expert e."""
            if s == 1:
                # dequant: releases ps1
                ps1 = st[f"ps1_{e}"]
                sxb64 = sxb[:, e * NT : (e + 1) * NT]
                gate = st[f"gate_{e}"] = ep.tile(
                    [128, KI, NT], F32, tag="gate", name=f"gate_{e}"
                )
                up = st[f"up_{e}"] = ep.tile(
                    [128, KI, NT], F32, tag="up", name=f"up_{e}"
                )
                # gate = psum * w1sc[ch] * sx[tok]; up = psum * w1sc_up[ch]
                # (sx deferred into s2c: aq is invariant to per-token scale)
                w1g = w1sc_s[e][:, 0:KI].unsqueeze(2).broadcast_to([128, KI, NT])
                w1u = w1sc_s[e][:, KI:CT1].unsqueeze(2).broadcast_to([128, KI, NT])
                sx3 = sxb64.unsqueeze(1).broadcast_to([128, KI, NT])
                nc.vector.tensor_tensor(out=gate[:], in0=ps1[:, 0:KI, :], in1=w1g, op=mult)
                nc.vector.tensor_tensor(out=gate[:], in0=gate[:], in1=sx3, op=mult)
                nc.vector.tensor_tensor(out=up[:], in0=ps1[:, KI : 2 * KI, :], in1=w1u, op=mult)
            elif s == 2:
                gate, up = st[f"gate_{e}"], st[f"up_{e}"]
                sig = ep.tile([128, KI, NT], F32, tag="sig", name=f"sig_{e}")
                nc.scalar.activation(
                    out=sig[:], in_=gate[:], func=mybir.ActivationFunctionType.Sigmoid
                )
                gsig = ep.tile([128, KI, NT], F32, tag="gsig", name=f"gsig_{e}")
                nc.vector.tensor_tensor(out=gsig[:], in0=gate[:], in1=sig[:], op=mult)
                act = st[f"act_{e}"] = ep.tile(
                    [128, KI, NT], F32, tag="act", name=f"act_{e}"
                )
                nc.vector.tensor_tensor(out=act[:], in0=gsig[:], in1=up[:], op=mult)
            elif s == 3:
                act = st[f"act_{e}"]
                # per-token absmax over all I channels (partitions x 11 tiles)
                mall = st[f"mall_{e}"] = ep.tile(
                    [128, KI, NT], F32, tag="mall", name=f"mall_{e}"
                )
                nc.gpsimd.partition_all_reduce(
                    mall[:].rearrange("p j t -> p (j t)"),
                    act[:].rearrange("p j t -> p (j t)"),
                    128,
                    bass_isa.ReduceOp.absmax,
                )
                mfin = stats.tile([128, NT], F32, tag="mfin", name=f"mfin_{e}")
                nc.vector.reduce_max(
                    out=mfin[:].unsqueeze(2),
                    in_=mall[:].rearrange("p j t -> p t j"),
                    axis=mybir.AxisListType.X,
                )
                mc = stats.tile([128, NT], F32, tag="mc", name=f"mc_{e}")
                nc.vector.tensor_scalar_max(out=mc[:], in0=mfin[:], scalar1=1e-12)
                rr = stats.tile([128, NT], F32, tag="rr", name=f"rr_{e}")
                nc.vector.reciprocal(out=rr[:], in_=mc[:])
                r127 = st[f"r127_{e}"] = stats.tile(
                    [128, NT], F32, tag="r127", name=f"r127_{e}"
                )
                nc.vector.tensor_scalar_mul(out=r127[:], in0=rr[:], scalar1=127.0)
                # s2c = (mc/127) * sx[tok] * comb[tok]  (sx folded in scrow)
                s2c = stats.tile([128, NT], F32, tag="s2c", name=f"s2c_{e}")
                nc.vector.scalar_tensor_tensor(
                    out=s2c[:],
                    in0=mc[:],
                    scalar=1.0 / 127.0,
                    in1=scb[:, e * NT : (e + 1) * NT],
                    op0=mult,
                    op1=mult,
                )
                s2cs.append(s2c)
            else:
                act, r127 = st[f"act_{e}"], st[f"r127_{e}"]
                tq = ep.tile([128, KI, NT], F32, tag="gate", name=f"tq_{e}")
                nc.vector.tensor_tensor(
                    out=tq[:],
                    in0=act[:],
                    in1=r127[:].unsqueeze(1).broadcast_to([128, KI, NT]),
                    op=mult,
                )
                trnd = ep.tile([128, KI, NT], F32, tag="mall", name=f"trnd_{e}")
                nc.vector.tensor_scalar_add(out=trnd[:], in0=tq[:], scalar1=MAGIC)
                aq = aqp.tile([128, KI, NT], BF16, tag="aq", name=f"aq_{e}")
                nc.vector.tensor_scalar_add(out=aq[:], in0=trnd[:], scalar1=-MAGIC)
                aqs.append(aq)

        def w2dma(e, g):
            w2i = st[f"w2i_{e}_{g}"] = w2i8p.tile(
                [128, KI, 512], I8, tag="w2i", name=f"w2i_{e}_{g}"
            )
            nc.sync.dma_start(out=w2i[:], in_=w2_d[e, :, :, g * 512 : (g + 1) * 512])

        def w2cast(e, g):
            w2i = st[f"w2i_{e}_{g}"]
            w2b = st[f"w2b_{e}_{g}"] = w2bfp.tile(
                [128, KI, 512], BF16, tag="w2b", name=f"w2b_{e}_{g}"
            )
            nc.vector.tensor_copy(out=w2b[:, :, 0:W2V], in_=w2i[:, :, 0:W2V])
            wam = (W2V + W2A) // 2
            nc.scalar.copy(out=w2b[:, :, W2V:wam], in_=w2i[:, :, W2V:wam])
            nc.scalar.copy(out=w2b[:, :, wam:W2A], in_=w2i[:, :, wam:W2A])
            wpm = (W2A + 512) // 2
            nc.gpsimd.tensor_copy(out=w2b[:, :, W2A:wpm], in_=w2i[:, :, W2A:wpm])
            nc.gpsimd.tensor_copy(out=w2b[:, :, wpm:512], in_=w2i[:, :, wpm:512])

        def g2mm(e, g):
            w2b = st[f"w2b_{e}_{g}"]
            # one full bank per tag; single accumulation group per bank
            ps2 = ps2p.tile([128, 8, NT], F32, tag=f"ps2{e}", name=f"ps2_{e}_{g}")
            for cc in range(4):
                for k in range(KI):
                    nc.tensor.matmul(
                        ps2[:, cc, :],
                        lhsT=w2b[:, k, cc * 128 : (cc + 1) * 128],
                        rhs=aqs[e][:, k, :],
                        start=(k == 0 and cc == 0),
                        stop=(k == KI - 1 and cc == 3),
                    )
            # deq2: o * w2sc[h] * (s2 * comb)[tok] -> yT columns
            w2s3 = (
                w2sc_s[e][:, g * 4 : (g + 1) * 4]
                .unsqueeze(2)
                .broadcast_to([128, 4, NT])
            )
            s2c3 = s2cs[e][:].unsqueeze(1).broadcast_to([128, 4, NT])
            dtmp = ep.tile([128, 4, NT], F32, tag="dtmp", name=f"dtmp_{e}_{g}")
            nc.vector.tensor_tensor(out=dtmp[:], in0=ps2[:, 0:4, :], in1=w2s3, op=mult)
            nc.vector.tensor_tensor(
                out=yT[:, g * 4 : (g + 1) * 4, e * NT : (e + 1) * NT],
                in0=dtmp[:],
                in1=s2c3,
                op=mult,
            )

        def ytail(g):
            # both experts done for these channel tiles: emit y rows
            if "pst" not in st:
                st["pst"] = ps1p.tile([128, 24, NT], F32, tag="ps1", name="pst")
            pst = st["pst"]
            views = [
                pst[:, 2 * cc : 2 * cc + 2, :].rearrange("p a t -> p (a t)")
                for cc in range(4)
            ]
            for cc in range(4):
                c = g * 4 + cc
                nc.tensor.transpose(views[cc], yT[:, c, :], ident[:])
            for cc in range(4):
                c = g * 4 + cc
                if cc < 2:
                    nc.vector.tensor_copy(
                        out=ysb[:, c * 128 : (c + 1) * 128], in_=views[cc]
                    )
                else:
                    nc.scalar.copy(
                        out=ysb[:, c * 128 : (c + 1) * 128], in_=views[cc]
                    )
            nc.sync.dma_start(
                out=y_d[:, g * 512 : (g + 1) * 512],
                in_=ysb[:, g * 512 : (g + 1) * 512],
            )

        yT = yp.tile([128, CT2, NPOS], F32, name="yT")
        ysb = yp.tile([128, H], F32, name="ysb")

        for kg in range(4):
            g1_chunk(0, kg)
            g1_chunk(1, kg)
        for s in range(1, 5):
            epi(0, s)
        for e in range(EPC):
            for g in range(4):
                w2dma(e, g)
        for s in range(1, 5):
            epi(1, s)
        for g in range(4):
            w2cast(0, g)
            g2mm(0, g)
        for g in range(4):
            w2cast(1, g)
            g2mm(1, g)
            ytail(g)

    nc.compile()
    return nc


def get_program():
    if "nc" not in _cache:
        _cache["nc"] = _build_program()
    return _cache["nc"]


def _prep_inputs(x, expert_ids, smooth_scales, expert_scales, w1, w1_scale, w2, w2_scale):
    """Host-side dispatch: quantize x, route tokens, shard experts."""
    x = np.asarray(x, np.float32)
    expert_ids = np.asarray(expert_ids)
    smooth_scales = np.asarray(smooth_scales, np.float32)
    expert_scales = np.asarray(expert_scales, np.float32)
    w1_scale = np.asarray(w1_scale, np.float32)
    w2_scale = np.asarray(w2_scale, np.float32)

    # dynamic per-token int8 quantization (exact mirror of reference ops)
    sx = np.maximum(np.max(np.abs(x), axis=-1, keepdims=True), 1e-12) / 127.0
    xq = np.round(np.clip(x / sx, -128.0, 127.0)).astype(np.float32)

    # combine matrix [B, E]: scatter-add expert_scales at expert_ids
    comb = np.zeros((B, E), np.float32)
    np.add.at(comb, (np.arange(B)[:, None], expert_ids), expert_scales)

    w1v = w1.astype(np.int8)
    w2v = w2.astype(np.int8)

    in_maps = []
    pos2tok = np.zeros((NCORES, NPOS), np.int64)
    for c in range(NCORES):
        valid = np.zeros(NPOS, bool)
        ge_of_pos = np.zeros(NPOS, np.int64)
        for le in range(EPC):
            ge = EPC * c + le
            toks = np.nonzero((expert_ids == ge).any(axis=1))[0]
            assert len(toks) <= NT, f"expert {ge}: {len(toks)} tokens > capacity {NT}"
            pos2tok[c, le * NT : le * NT + len(toks)] = toks
            valid[le * NT : le * NT + len(toks)] = True
            ge_of_pos[le * NT : (le + 1) * NT] = ge

        toks_c = pos2tok[c]
        xg = xq[toks_c]  # [NPOS, H]
        xqT = np.ascontiguousarray(
            xg.T.reshape(KH, 128, NPOS).transpose(1, 0, 2)
        ).astype(ml_dtypes.bfloat16)
        sxrow = sx[toks_c, 0][None, :].astype(np.float32)
        combp = comb[toks_c, ge_of_pos] * valid  # zero at padding positions
        scrow = (sx[toks_c, 0] * combp)[None, :].astype(np.float32)

        es = list(range(EPC * c, EPC * (c + 1)))
        w1t = np.ascontiguousarray(
            w1v[es].reshape(EPC, KH, 128, I2).transpose(0, 2, 1, 3)
        )
        w2t = np.ascontiguousarray(
            w2v[es].reshape(EPC, KI, 128, H).transpose(0, 2, 1, 3)
        )
        w1sc = np.empty((EPC, 128, CT1), np.float32)
        w2sc = np.empty((EPC, 128, CT2), np.float32)
        for le, ge in enumerate(es):
            gatesc = w1_scale[ge, :I].reshape(KI, 128).T
            upsc = (w1_scale[ge, I:] * smooth_scales[ge]).reshape(KI, 128).T
            w1sc[le] = np.concatenate([gatesc, upsc], axis=1)
            w2sc[le] = w2_scale[ge].reshape(CT2, 128).T

        in_maps.append(
            {
                "xqT": xqT,
                "sxrow": sxrow,
                "scrow": scrow,
                "w1t": w1t,
                "w2t": w2t,
                "w1sc": w1sc,
                "w2sc": w2sc,
            }
        )
    return in_maps, pos2tok


def kernel(
    x,
    expert_ids,
    smooth_scales,
    expert_scales,
    x_active_mask,
    w1,
    w1_scale,
    w2,
    w2_scale,
    _trace=False,
    _trace_kwargs=None,
):
    in_maps, pos2tok = _prep_inputs(
        x, expert_ids, smooth_scales, expert_scales, w1, w1_scale, w2, w2_scale
    )
    nc = get_program()
    res = run_bass_kernel_spmd(
        nc,
        in_maps,
        core_ids=list(range(NCORES)),
        trace=_trace,
        **(_trace_kwargs or {}),
    )
    y = np.zeros((B, H), np.float32)
    for c, r in enumerate(res.results):
        np.add.at(y, pos2tok[c], r["y"].astype(np.float32))
    y *= np.asarray(x_active_mask).astype(np.float32)[:, None]
    if _trace:
        kernel.last_results = res
    return y


# revision 19
# speedup vs baseline: 1.3230x; 1.0390x over previous
"""Trainium2 Bass kernel for nn_DecodeMoeOps (MoE decode: dispatch-quant,
grouped int8 GEMM1, SwiGLU, requant, grouped int8 GEMM2, weighted combine).

Expert-parallel across 8 NeuronCores: core c owns experts {2c, 2c+1}.

Key design (v2):
- Weights ship to SBUF as RAW INT8 (1 B/weight over HWDGE) and are upcast
  to bf16 on-chip, split across the DVE / ACT / GPSIMD engines. This halves
  the DMA-device byte volume vs casting during the DMA (which is charged at
  bf16 output bytes).
- Both GEMMs run WEIGHT-STATIONARY (weights are the PE's lhsT), so PE time
  scales with the number of routed tokens, not the weight volume. Each
  expert gets a fixed 64-position block of gathered tokens (host routing);
  tokens routed to both of a core's experts appear in both blocks and the
  host scatter-adds per-position outputs back to token rows.
- GEMM1 output lands channel-major [ch, tok]; SwiGLU/requant run in that
  layout (cross-partition absmax via gpsimd.partition_all_reduce), which
  makes the requantized activations directly usable as GEMM2's moving
  operand with no transposes. Final [h, pos] -> [pos, h] via PE transpose.
"""

import os
import sys

for _p in ("/opt/trn_rl_repo", "/root/.axon_site/_ro/trn_rl_repo"):
    if os.path.isdir(_p) and _p not in sys.path:
        sys.path.insert(0, _p)

from contextlib import ExitStack

import ml_dtypes
import numpy as np

import concourse.bass as bass
import concourse.bass_isa as bass_isa
import concourse.mybir as mybir
import concourse.tile as tile
from concourse import bacc
from concourse.bass_utils import run_bass_kernel_spmd
from concourse.masks import make_identity

B, TOPK, H, I, E = 128, 8, 2048, 1408, 16
NCORES = 8
EPC = E // NCORES  # experts per core
KH = H // 128  # 16 contraction tiles for GEMM1
KI = I // 128  # 11 contraction tiles for GEMM2
I2 = 2 * I
CT1 = I2 // 128  # 22 GEMM1 output-channel tiles (gate 0..10, up 11..21)
CT2 = H // 128  # 16 GEMM2 output-channel tiles
NT = 64  # token positions per expert block
NPOS = EPC * NT  # 128 positions per core
F32 = mybir.dt.float32
BF16 = mybir.dt.bfloat16
I8 = mybir.dt.int8
MAGIC = float(3 * 2**22)  # fp32 round-to-nearest-int magic (covers negatives)

# int8 -> bf16 upcast split points (free-dim columns) per engine:
# [0:V) on DVE, [V:A) on ACT, [A:end) on GPSIMD.  Tunables.
W1V, W1A = 1280, 2304  # of I2 = 2816
W2V, W2A = 224, 400  # of 512-wide w2 column chunks

_cache: dict = {}


def _build_program():
    nc = bacc.Bacc(
        "TRN2",
        target_bir_lowering=False,
        debug=False,
        num_devices=NCORES,
    )
    mult = mybir.AluOpType.mult
    opmax = mybir.AluOpType.max

    # --- per-core DRAM I/O ---
    xqT_d = nc.dram_tensor("xqT", [128, KH, NPOS], BF16, kind="ExternalInput").ap()
    sxr_d = nc.dram_tensor("sxrow", [1, NPOS], F32, kind="ExternalInput").ap()
    scr_d = nc.dram_tensor("scrow", [1, NPOS], F32, kind="ExternalInput").ap()
    w1_d = nc.dram_tensor("w1t", [EPC, 128, KH, I2], I8, kind="ExternalInput").ap()
    w2_d = nc.dram_tensor("w2t", [EPC, 128, KI, H], I8, kind="ExternalInput").ap()
    w1sc_d = nc.dram_tensor("w1sc", [EPC, 128, CT1], F32, kind="ExternalInput").ap()
    w2sc_d = nc.dram_tensor("w2sc", [EPC, 128, CT2], F32, kind="ExternalInput").ap()
    y_d = nc.dram_tensor("y", [NPOS, H], F32, kind="ExternalOutput").ap()

    with tile.TileContext(nc) as tc, ExitStack() as ctx:
        consts = ctx.enter_context(tc.tile_pool(name="consts", bufs=1))
        w1i8p = ctx.enter_context(tc.tile_pool(name="w1i8", bufs=4))
        w1bfp = ctx.enter_context(tc.tile_pool(name="w1bf", bufs=2))
        w2i8p = ctx.enter_context(tc.tile_pool(name="w2i8", bufs=4))
        w2bfp = ctx.enter_context(tc.tile_pool(name="w2bf", bufs=2))
        ep = ctx.enter_context(tc.tile_pool(name="ep", bufs=2))
        stats = ctx.enter_context(tc.tile_pool(name="stats", bufs=2))
        aqp = ctx.enter_context(tc.tile_pool(name="aqp", bufs=2))
        yp = ctx.enter_context(tc.tile_pool(name="yp", bufs=1))
        ps1p = ctx.enter_context(tc.tile_pool(name="ps1", bufs=2, space="PSUM"))
        ps2p = ctx.enter_context(tc.tile_pool(name="ps2", bufs=1, space="PSUM"))

        # --- prologue ---
        xqT_s = consts.tile([128, KH, NPOS], BF16, name="xqT_s")
        nc.scalar.dma_start(out=xqT_s[:], in_=xqT_d)
        ident = consts.tile([128, 128], F32, name="ident")
        make_identity(nc, ident[:])
        ones1 = consts.tile([1, 128], F32, name="ones1")
        nc.vector.memset(ones1[:], 1.0)
        sxr_s = consts.tile([1, NPOS], F32, name="sxr_s")
        nc.scalar.dma_start(out=sxr_s[:], in_=sxr_d)
        scr_s = consts.tile([1, NPOS], F32, name="scr_s")
        nc.scalar.dma_start(out=scr_s[:], in_=scr_d)
        w1sc_s, w2sc_s = [], []
        for e in range(EPC):
            t1 = consts.tile([128, CT1], F32, name=f"w1sc_{e}")
            nc.sync.dma_start(out=t1[:], in_=w1sc_d[e])
            w1sc_s.append(t1)
            t2 = consts.tile([128, CT2], F32, name=f"w2sc_{e}")
            nc.sync.dma_start(out=t2[:], in_=w2sc_d[e])
            w2sc_s.append(t2)

        # broadcast the per-position rows across partitions: out[p,t] = row[t]
        # (borrows a ps1-pool buffer; released before GEMM1 writes it)
        psb0 = ps1p.tile([128, 24, NT], F32, tag="ps1", name="psb0")
        sxb = consts.tile([128, NPOS], F32, name="sxb")
        bc0 = psb0[:, 0:2, :].rearrange("p a t -> p (a t)")
        nc.tensor.matmul(bc0, lhsT=ones1[:], rhs=sxr_s[:], start=True, stop=True)
        nc.vector.tensor_copy(out=sxb[:], in_=bc0)
        scb = consts.tile([128, NPOS], F32, name="scb")
        bc1 = psb0[:, 2:4, :].rearrange("p a t -> p (a t)")
        nc.tensor.matmul(bc1, lhsT=ones1[:], rhs=scr_s[:], start=True, stop=True)
        nc.vector.tensor_copy(out=scb[:], in_=bc1)

        # Emission order == per-engine execution order, so the phases below
        # software-pipeline the kernel: each expert's epilogue is emitted in
        # slices spliced between the NEXT phase's chunk pipelines, keeping
        # every engine queue free of long head-of-line dependency waits.
        st = {}  # per-expert tiles carried across phases
        aqs, s2cs = [], []

        def g1_chunk(e, kg):
            if kg == 0:
                # 3 exact PSUM banks; one accumulation group per bank (8
                # chunks share a bank: start on the bank's first chunk, stop
                # on its last -- HW zeroing is lazy per-write in the region).
                st[f"ps1_{e}"] = ps1p.tile([128, 24, NT], F32, tag="ps1", name=f"ps1_{e}")
            ps1 = st[f"ps1_{e}"]
            w1i = w1i8p.tile([128, 4, I2], I8, tag="w1i", name=f"w1i_{e}_{kg}")
            nc.sync.dma_start(out=w1i[:], in_=w1_d[e, :, kg * 4 : (kg + 1) * 4, :])
            w1b = w1bfp.tile([128, 4, I2], BF16, tag="w1b", name=f"w1b_{e}_{kg}")
            for j in range(4):
                nc.vector.tensor_copy(out=w1b[:, j, 0:W1V], in_=w1i[:, j, 0:W1V])
                nc.scalar.copy(out=w1b[:, j, W1V:W1A], in_=w1i[:, j, W1V:W1A])
                nc.gpsimd.tensor_copy(out=w1b[:, j, W1A:I2], in_=w1i[:, j, W1A:I2])
            for j in range(4):
                k = kg * 4 + j
                for c in range(CT1):
                    nc.tensor.matmul(
                        ps1[:, c, :],
                        lhsT=w1b[:, j, c * 128 : (c + 1) * 128],
                        rhs=xqT_s[:, k, e * NT : (e + 1) * NT],
                        start=(k == 0 and c % 8 == 0),
                        stop=(k == KH - 1 and (c % 8 == 7 or c == CT1 - 1)),
                    )

        def epi(e, s):
            """Epilogue slice s (1..4) for expert e."""
            if s == 1:
                # dequant: releases ps1
                ps1 = st[f"ps1_{e}"]
                sxb64 = sxb[:, e * NT : (e + 1) * NT]
                gate = st[f"gate_{e}"] = ep.tile(
                    [128, KI, NT], F32, tag="gate", name=f"gate_{e}"
                )
                up = st[f"up_{e}"] = ep.tile(
                    [128, KI, NT], F32, tag="up", name=f"up_{e}"
                )
                # gate = psum * w1sc[ch] * sx[tok]; up = psum * w1sc_up[ch]
                # (sx deferred into s2c: aq is invariant to per-token scale)
                w1g = w1sc_s[e][:, 0:KI].unsqueeze(2).broadcast_to([128, KI, NT])
                w1u = w1sc_s[e][:, KI:CT1].unsqueeze(2).broadcast_to([128, KI, NT])
                sx3 = sxb64.unsqueeze(1).broadcast_to([128, KI, NT])
                nc.vector.tensor_tensor(out=gate[:], in0=ps1[:, 0:KI, :], in1=w1g, op=mult)
                nc.vector.tensor_tensor(out=gate[:], in0=gate[:], in1=sx3, op=mult)
                nc.vector.tensor_tensor(out=up[:], in0=ps1[:, KI : 2 * KI, :], in1=w1u, op=mult)
            elif s == 2:
                gate, up = st[f"gate_{e}"], st[f"up_{e}"]
                sig = ep.tile([128, KI, NT], F32, tag="sig", name=f"sig_{e}")
                nc.scalar.activation(
                    out=sig[:], in_=gate[:], func=mybir.ActivationFunctionType.Sigmoid
                )
                gsig = ep.tile([128, KI, NT], F32, tag="gsig", name=f"gsig_{e}")
                nc.vector.tensor_tensor(out=gsig[:], in0=gate[:], in1=sig[:], op=mult)
                act = st[f"act_{e}"] = ep.tile(
                    [128, KI, NT], F32, tag="act", name=f"act_{e}"
                )
                nc.vector.tensor_tensor(out=act[:], in0=gsig[:], in1=up[:], op=mult)
            elif s == 3:
                act = st[f"act_{e}"]
                # per-token absmax over all I channels (partitions x 11 tiles)
                mall = st[f"mall_{e}"] = ep.tile(
                    [128, KI, NT], F32, tag="mall", name=f"mall_{e}"
                )
                nc.gpsimd.partition_all_reduce(
                    mall[:].rearrange("p j t -> p (j t)"),
                    act[:].rearrange("p j t -> p (j t)"),
                    128,
                    bass_isa.ReduceOp.absmax,
                )
                mfin = stats.tile([128, NT], F32, tag="mfin", name=f"mfin_{e}")
                nc.vector.reduce_max(
                    out=mfin[:].unsqueeze(2),
                    in_=mall[:].rearrange("p j t -> p t j"),
                    axis=mybir.AxisListType.X,
                )
                mc = stats.tile([128, NT], F32, tag="mc", name=f"mc_{e}")
                nc.vector.tensor_scalar_max(out=mc[:], in0=mfin[:], scalar1=1e-12)
                rr = stats.tile([128, NT], F32, tag="rr", name=f"rr_{e}")
                nc.vector.reciprocal(out=rr[:], in_=mc[:])
                r127 = st[f"r127_{e}"] = stats.tile(
                    [128, NT], F32, tag="r127", name=f"r127_{e}"
                )
                nc.vector.tensor_scalar_mul(out=r127[:], in0=rr[:], scalar1=127.0)
                # s2c = (mc/127) * sx[tok] * comb[tok]  (sx folded in scrow)
                s2c = stats.tile([128, NT], F32, tag="s2c", name=f"s2c_{e}")
                nc.vector.scalar_tensor_tensor(
                    out=s2c[:],
                    in0=mc[:],
                    scalar=1.0 / 127.0,
                    in1=scb[:, e * NT : (e + 1) * NT],
                    op0=mult,
                    op1=mult,
                )
                s2cs.append(s2c)
            else:
                act, r127 = st[f"act_{e}"], st[f"r127_{e}"]
                tq = ep.tile([128, KI, NT], F32, tag="gate", name=f"tq_{e}")
                nc.vector.tensor_tensor(
                    out=tq[:],
                    in0=act[:],
                    in1=r127[:].unsqueeze(1).broadcast_to([128, KI, NT]),
                    op=mult,
                )
                trnd = ep.tile([128, KI, NT], F32, tag="mall", name=f"trnd_{e}")
                nc.vector.tensor_scalar_add(out=trnd[:], in0=tq[:], scalar1=MAGIC)
                aq = aqp.tile([128, KI, NT], BF16, tag="aq", name=f"aq_{e}")
                nc.vector.tensor_scalar_add(out=aq[:], in0=trnd[:], scalar1=-MAGIC)
                aqs.append(aq)

        def w2dma(e, g):
            w2i = st[f"w2i_{e}_{g}"] = w2i8p.tile(
                [128, KI, 512], I8, tag="w2i", name=f"w2i_{e}_{g}"
            )
            nc.sync.dma_start(out=w2i[:], in_=w2_d[e, :, :, g * 512 : (g + 1) * 512])

        def w2cast(e, g):
            w2i = st[f"w2i_{e}_{g}"]
            w2b = st[f"w2b_{e}_{g}"] = w2bfp.tile(
                [128, KI, 512], BF16, tag="w2b", name=f"w2b_{e}_{g}"
            )
            nc.vector.tensor_copy(out=w2b[:, :, 0:W2V], in_=w2i[:, :, 0:W2V])
            wam = (W2V + W2A) // 2
            nc.scalar.copy(out=w2b[:, :, W2V:wam], in_=w2i[:, :, W2V:wam])
            nc.scalar.copy(out=w2b[:, :, wam:W2A], in_=w2i[:, :, wam:W2A])
            wpm = (W2A + 512) // 2
            nc.gpsimd.tensor_copy(out=w2b[:, :, W2A:wpm], in_=w2i[:, :, W2A:wpm])
            nc.gpsimd.tensor_copy(out=w2b[:, :, wpm:512], in_=w2i[:, :, wpm:512])

        def g2mm(e, g):
            w2b = st[f"w2b_{e}_{g}"]
            # one full bank per tag; single accumulation group per bank
            ps2 = ps2p.tile([128, 8, NT], F32, tag=f"ps2{e}", name=f"ps2_{e}_{g}")
            for cc in range(4):
                for k in range(KI):
                    nc.tensor.matmul(
                        ps2[:, cc, :],
                        lhsT=w2b[:, k, cc * 128 : (cc + 1) * 128],
                        rhs=aqs[e][:, k, :],
                        start=(k == 0 and cc == 0),
                        stop=(k == KI - 1 and cc == 3),
                    )
            # deq2: o * w2sc[h] * (s2 * comb)[tok] -> yT columns
            w2s3 = (
                w2sc_s[e][:, g * 4 : (g + 1) * 4]
                .unsqueeze(2)
                .broadcast_to([128, 4, NT])
            )
            s2c3 = s2cs[e][:].unsqueeze(1).broadcast_to([128, 4, NT])
            dtmp = ep.tile([128, 4, NT], F32, tag="dtmp", name=f"dtmp_{e}_{g}")
            nc.vector.tensor_tensor(out=dtmp[:], in0=ps2[:, 0:4, :], in1=w2s3, op=mult)
            nc.vector.tensor_tensor(
                out=yT[:, g * 4 : (g + 1) * 4, e * NT : (e + 1) * NT],
                in0=dtmp[:],
                in1=s2c3,
                op=mult,
            )

        def ytail(g):
            # both experts done for these channel tiles: emit y rows
            if "pst" not in st:
                st["pst"] = ps1p.tile([128, 24, NT], F32, tag="ps1", name="pst")
            pst = st["pst"]
            views = [
                pst[:, 2 * cc : 2 * cc + 2, :].rearrange("p a t -> p (a t)")
                for cc in range(4)
            ]
            for cc in range(4):
                c = g * 4 + cc
                nc.tensor.transpose(views[cc], yT[:, c, :], ident[:])
            nc.vector.tensor_copy(
                out=ysb[:, g * 512 : (g + 1) * 512],
                in_=pst[:, 0:8, :].rearrange("p a t -> p (a t)"),
            )
            nc.sync.dma_start(
                out=y_d[:, g * 512 : (g + 1) * 512],
                in_=ysb[:, g * 512 : (g + 1) * 512],
            )

        yT = yp.tile([128, CT2, NPOS], F32, name="yT")
        ysb = yp.tile([128, H], F32, name="ysb")

        for kg in range(4):
            g1_chunk(0, kg)
            g1_chunk(1, kg)
        for s in range(1, 5):
            epi(0, s)
        for e in range(EPC):
            for g in range(4):
                w2dma(e, g)
        for s in range(1, 5):
            epi(1, s)
        for g in range(4):
            w2cast(0, g)
            g2mm(0, g)
        for g in range(4):
            w2cast(1, g)
            g2mm(1, g)
            ytail(g)

    nc.compile()
    return nc


def get_program():
    if "nc" not in _cache:
        _cache["nc"] = _build_program()
    return _cache["nc"]


def _prep_inputs(x, expert_ids, smooth_scales, expert_scales, w1, w1_scale, w2, w2_scale):
    """Host-side dispatch: quantize x, route tokens, shard experts."""
    x = np.asarray(x, np.float32)
    expert_ids = np.asarray(expert_ids)
    smooth_scales = np.asarray(smooth_scales, np.float32)
    expert_scales = np.asarray(expert_scales, np.float32)
    w1_scale = np.asarray(w1_scale, np.float32)
    w2_scale = np.asarray(w2_scale, np.float32)

    # dynamic per-token int8 quantization (exact mirror of reference ops)
    sx = np.maximum(np.max(np.abs(x), axis=-1, keepdims=True), 1e-12) / 127.0
    xq = np.round(np.clip(x / sx, -128.0, 127.0)).astype(np.float32)

    # combine matrix [B, E]: scatter-add expert_scales at expert_ids
    comb = np.zeros((B, E), np.float32)
    np.add.at(comb, (np.arange(B)[:, None], expert_ids), expert_scales)

    w1v = w1.astype(np.int8)
    w2v = w2.astype(np.int8)

    in_maps = []
    pos2tok = np.zeros((NCORES, NPOS), np.int64)
    for c in range(NCORES):
        valid = np.zeros(NPOS, bool)
        ge_of_pos = np.zeros(NPOS, np.int64)
        for le in range(EPC):
            ge = EPC * c + le
            toks = np.nonzero((expert_ids == ge).any(axis=1))[0]
            assert len(toks) <= NT, f"expert {ge}: {len(toks)} tokens > capacity {NT}"
            pos2tok[c, le * NT : le * NT + len(toks)] = toks
            valid[le * NT : le * NT + len(toks)] = True
            ge_of_pos[le * NT : (le + 1) * NT] = ge

        toks_c = pos2tok[c]
        xg = xq[toks_c]  # [NPOS, H]
        xqT = np.ascontiguousarray(
            xg.T.reshape(KH, 128, NPOS).transpose(1, 0, 2)
        ).astype(ml_dtypes.bfloat16)
        sxrow = sx[toks_c, 0][None, :].astype(np.float32)
        combp = comb[toks_c, ge_of_pos] * valid  # zero at padding positions
        scrow = (sx[toks_c, 0] * combp)[None, :].astype(np.float32)

        es = list(range(EPC * c, EPC * (c + 1)))
        w1t = np.ascontiguousarray(
            w1v[es].reshape(EPC, KH, 128, I2).transpose(0, 2, 1, 3)
        )
        w2t = np.ascontiguousarray(
            w2v[es].reshape(EPC, KI, 128, H).transpose(0, 2, 1, 3)
        )
        w1sc = np.empty((EPC, 128, CT1), np.float32)
        w2sc = np.empty((EPC, 128, CT2), np.float32)
        for le, ge in enumerate(es):
            gatesc = w1_scale[ge, :I].reshape(KI, 128).T
            upsc = (w1_scale[ge, I:] * smooth_scales[ge]).reshape(KI, 128).T
            w1sc[le] = np.concatenate([gatesc, upsc], axis=1)
            w2sc[le] = w2_scale[ge].reshape(CT2, 128).T

        in_maps.append(
            {
                "xqT": xqT,
                "sxrow": sxrow,
                "scrow": scrow,
                "w1t": w1t,
                "w2t": w2t,
                "w1sc": w1sc,
                "w2sc": w2sc,
            }
        )
    return in_maps, pos2tok


def kernel(
    x,
    expert_ids,
    smooth_scales,
    expert_scales,
    x_active_mask,
    w1,
    w1_scale,
    w2,
    w2_scale,
    _trace=False,
    _trace_kwargs=None,
):
    in_maps, pos2tok = _prep_inputs(
        x, expert_ids, smooth_scales, expert_scales, w1, w1_scale, w2, w2_scale
    )
    nc = get_program()
    res = run_bass_kernel_spmd(
        nc,
        in_maps,
        core_ids=list(range(NCORES)),
        trace=_trace,
        **(_trace_kwargs or {}),
    )
    y = np.zeros((B, H), np.float32)
    for c, r in enumerate(res.results):
        np.add.at(y, pos2tok[c], r["y"].astype(np.float32))
    y *= np.asarray(x_active_mask).astype(np.float32)[:, None]
    if _trace:
        kernel.last_results = res
    return y


# revision 26
# speedup vs baseline: 1.3523x; 1.0221x over previous
"""Trainium2 Bass kernel for nn_DecodeMoeOps (MoE decode: dispatch-quant,
grouped int8 GEMM1, SwiGLU, requant, grouped int8 GEMM2, weighted combine).

Expert-parallel across 8 NeuronCores: core c owns experts {2c, 2c+1}.

Key design (v2):
- Weights ship to SBUF as RAW INT8 (1 B/weight over HWDGE) and are upcast
  to bf16 on-chip, split across the DVE / ACT / GPSIMD engines. This halves
  the DMA-device byte volume vs casting during the DMA (which is charged at
  bf16 output bytes).
- Both GEMMs run WEIGHT-STATIONARY (weights are the PE's lhsT), so PE time
  scales with the number of routed tokens, not the weight volume. Each
  expert gets a fixed 64-position block of gathered tokens (host routing);
  tokens routed to both of a core's experts appear in both blocks and the
  host scatter-adds per-position outputs back to token rows.
- GEMM1 output lands channel-major [ch, tok]; SwiGLU/requant run in that
  layout (cross-partition absmax via gpsimd.partition_all_reduce), which
  makes the requantized activations directly usable as GEMM2's moving
  operand with no transposes. Final [h, pos] -> [pos, h] via PE transpose.
"""

import os
import sys

for _p in ("/opt/trn_rl_repo", "/root/.axon_site/_ro/trn_rl_repo"):
    if os.path.isdir(_p) and _p not in sys.path:
        sys.path.insert(0, _p)

from contextlib import ExitStack

import ml_dtypes
import numpy as np

import concourse.bass as bass
import concourse.bass_isa as bass_isa
import concourse.mybir as mybir
import concourse.tile as tile
from concourse import bacc
from concourse.bass_utils import run_bass_kernel_spmd
from concourse.masks import make_identity

B, TOPK, H, I, E = 128, 8, 2048, 1408, 16
NCORES = 8
EPC = E // NCORES  # experts per core
KH = H // 128  # 16 contraction tiles for GEMM1
KI = I // 128  # 11 contraction tiles for GEMM2
I2 = 2 * I
CT1 = I2 // 128  # 22 GEMM1 output-channel tiles (gate 0..10, up 11..21)
CT2 = H // 128  # 16 GEMM2 output-channel tiles
NT = 64  # token positions per expert block
NPOS = EPC * NT  # 128 positions per core
F32 = mybir.dt.float32
BF16 = mybir.dt.bfloat16
I8 = mybir.dt.int8
MAGIC = float(3 * 2**22)  # fp32 round-to-nearest-int magic (covers negatives)

# int8 -> bf16 upcast split points (free-dim columns) per engine:
# [0:V) on DVE, [V:A) on ACT, [A:end) on GPSIMD.  Tunables.
W1V, W1A = 1280, 2304  # of I2 = 2816
W2V, W2A = 224, 400  # of 512-wide w2 column chunks

_cache: dict = {}


def _build_program():
    nc = bacc.Bacc(
        "TRN2",
        target_bir_lowering=False,
        debug=False,
        num_devices=NCORES,
    )
    mult = mybir.AluOpType.mult
    opmax = mybir.AluOpType.max

    # --- per-core DRAM I/O ---
    xqT_d = nc.dram_tensor("xqT", [128, KH, NPOS], BF16, kind="ExternalInput").ap()
    sxr_d = nc.dram_tensor("sxrow", [1, NPOS], F32, kind="ExternalInput").ap()
    scr_d = nc.dram_tensor("scrow", [1, NPOS], F32, kind="ExternalInput").ap()
    w1_d = nc.dram_tensor("w1t", [EPC, 128, KH, I2], I8, kind="ExternalInput").ap()
    w2_d = nc.dram_tensor("w2t", [EPC, 128, KI, H], I8, kind="ExternalInput").ap()
    w1sc_d = nc.dram_tensor("w1sc", [EPC, 128, CT1], F32, kind="ExternalInput").ap()
    w2sc_d = nc.dram_tensor("w2sc", [EPC, 128, CT2], F32, kind="ExternalInput").ap()
    y_d = nc.dram_tensor("y", [NPOS, H], F32, kind="ExternalOutput").ap()

    with tile.TileContext(nc) as tc, ExitStack() as ctx:
        consts = ctx.enter_context(tc.tile_pool(name="consts", bufs=1))
        w1i8p = ctx.enter_context(tc.tile_pool(name="w1i8", bufs=4))
        w1bfp = ctx.enter_context(tc.tile_pool(name="w1bf", bufs=2))
        w2i8p = ctx.enter_context(tc.tile_pool(name="w2i8", bufs=4))
        w2bfp = ctx.enter_context(tc.tile_pool(name="w2bf", bufs=2))
        ep = ctx.enter_context(tc.tile_pool(name="ep", bufs=2))
        stats = ctx.enter_context(tc.tile_pool(name="stats", bufs=2))
        aqp = ctx.enter_context(tc.tile_pool(name="aqp", bufs=2))
        yp = ctx.enter_context(tc.tile_pool(name="yp", bufs=1))
        ps1p = ctx.enter_context(tc.tile_pool(name="ps1", bufs=2, space="PSUM"))
        ps2p = ctx.enter_context(tc.tile_pool(name="ps2", bufs=1, space="PSUM"))

        # --- prologue ---
        xqT_s = consts.tile([128, KH, NPOS], BF16, name="xqT_s")
        nc.scalar.dma_start(out=xqT_s[:], in_=xqT_d)
        ident = consts.tile([128, 128], F32, name="ident")
        make_identity(nc, ident[:])
        ones1 = consts.tile([1, 128], F32, name="ones1")
        nc.vector.memset(ones1[:], 1.0)
        sxr_s = consts.tile([1, NPOS], F32, name="sxr_s")
        nc.scalar.dma_start(out=sxr_s[:], in_=sxr_d)
        scr_s = consts.tile([1, NPOS], F32, name="scr_s")
        nc.scalar.dma_start(out=scr_s[:], in_=scr_d)
        w1sc_s, w2sc_s = [], []
        for e in range(EPC):
            w1sc_s.append(consts.tile([128, CT1], F32, name=f"w1sc_{e}"))
            w2sc_s.append(consts.tile([128, CT2], F32, name=f"w2sc_{e}"))

        def scale_dmas():
            for e in range(EPC):
                nc.sync.dma_start(out=w1sc_s[e][:], in_=w1sc_d[e])
                nc.sync.dma_start(out=w2sc_s[e][:], in_=w2sc_d[e])

        # broadcast the per-position rows across partitions: out[p,t] = row[t]
        # (borrows a ps1-pool buffer; released before GEMM1 writes it)
        psb0 = ps1p.tile([128, 24, NT], F32, tag="ps1", name="psb0")
        sxb = consts.tile([128, NPOS], F32, name="sxb")
        bc0 = psb0[:, 0:2, :].rearrange("p a t -> p (a t)")
        nc.tensor.matmul(bc0, lhsT=ones1[:], rhs=sxr_s[:], start=True, stop=True)
        nc.vector.tensor_copy(out=sxb[:], in_=bc0)
        scb = consts.tile([128, NPOS], F32, name="scb")
        bc1 = psb0[:, 2:4, :].rearrange("p a t -> p (a t)")
        nc.tensor.matmul(bc1, lhsT=ones1[:], rhs=scr_s[:], start=True, stop=True)
        nc.vector.tensor_copy(out=scb[:], in_=bc1)

        # Emission order == per-engine execution order, so the phases below
        # software-pipeline the kernel: each expert's epilogue is emitted in
        # slices spliced between the NEXT phase's chunk pipelines, keeping
        # every engine queue free of long head-of-line dependency waits.
        st = {}  # per-expert tiles carried across phases
        aqs, s2cs = [], []

        def g1_chunk(e, kg):
            if kg == 0:
                # 3 exact PSUM banks; one accumulation group per bank (8
                # chunks share a bank: start on the bank's first chunk, stop
                # on its last -- HW zeroing is lazy per-write in the region).
                st[f"ps1_{e}"] = ps1p.tile([128, 24, NT], F32, tag="ps1", name=f"ps1_{e}")
            ps1 = st[f"ps1_{e}"]
            w1i = w1i8p.tile([128, 4, I2], I8, tag="w1i", name=f"w1i_{e}_{kg}")
            nc.sync.dma_start(out=w1i[:], in_=w1_d[e, :, kg * 4 : (kg + 1) * 4, :])
            w1b = w1bfp.tile([128, 4, I2], BF16, tag="w1b", name=f"w1b_{e}_{kg}")
            for j in range(4):
                nc.vector.tensor_copy(out=w1b[:, j, 0:W1V], in_=w1i[:, j, 0:W1V])
                nc.scalar.copy(out=w1b[:, j, W1V:W1A], in_=w1i[:, j, W1V:W1A])
                nc.gpsimd.tensor_copy(out=w1b[:, j, W1A:I2], in_=w1i[:, j, W1A:I2])
            for j in range(4):
                k = kg * 4 + j
                for c in range(CT1):
                    nc.tensor.matmul(
                        ps1[:, c, :],
                        lhsT=w1b[:, j, c * 128 : (c + 1) * 128],
                        rhs=xqT_s[:, k, e * NT : (e + 1) * NT],
                        start=(k == 0 and c % 8 == 0),
                        stop=(k == KH - 1 and (c % 8 == 7 or c == CT1 - 1)),
                    )

        def epi(e, s):
            """Epilogue slice s (1..4) for expert e."""
            if s == 1:
                # dequant: releases ps1
                ps1 = st[f"ps1_{e}"]
                sxb64 = sxb[:, e * NT : (e + 1) * NT]
                gate = st[f"gate_{e}"] = ep.tile(
                    [128, KI, NT], F32, tag="gate", name=f"gate_{e}"
                )
                up = st[f"up_{e}"] = ep.tile(
                    [128, KI, NT], F32, tag="up", name=f"up_{e}"
                )
                # gate = psum * w1sc[ch] * sx[tok]; up = psum * w1sc_up[ch]
                # (sx deferred into s2c: aq is invariant to per-token scale)
                w1g = w1sc_s[e][:, 0:KI].unsqueeze(2).broadcast_to([128, KI, NT])
                w1u = w1sc_s[e][:, KI:CT1].unsqueeze(2).broadcast_to([128, KI, NT])
                sx3 = sxb64.unsqueeze(1).broadcast_to([128, KI, NT])
                nc.vector.tensor_tensor(out=gate[:], in0=ps1[:, 0:KI, :], in1=w1g, op=mult)
                nc.vector.tensor_tensor(out=gate[:], in0=gate[:], in1=sx3, op=mult)
                nc.vector.tensor_tensor(out=up[:], in0=ps1[:, KI : 2 * KI, :], in1=w1u, op=mult)
            elif s == 2:
                gate, up = st[f"gate_{e}"], st[f"up_{e}"]
                sig = ep.tile([128, KI, NT], F32, tag="sig", name=f"sig_{e}")
                nc.scalar.activation(
                    out=sig[:], in_=gate[:], func=mybir.ActivationFunctionType.Sigmoid
                )
                gsig = ep.tile([128, KI, NT], F32, tag="gsig", name=f"gsig_{e}")
                nc.vector.tensor_tensor(out=gsig[:], in0=gate[:], in1=sig[:], op=mult)
                act = st[f"act_{e}"] = ep.tile(
                    [128, KI, NT], F32, tag="act", name=f"act_{e}"
                )
                nc.vector.tensor_tensor(out=act[:], in0=gsig[:], in1=up[:], op=mult)
            elif s == 3:
                act = st[f"act_{e}"]
                # per-token absmax over all I channels (partitions x 11 tiles)
                mall = st[f"mall_{e}"] = ep.tile(
                    [128, KI, NT], F32, tag="mall", name=f"mall_{e}"
                )
                nc.gpsimd.partition_all_reduce(
                    mall[:].rearrange("p j t -> p (j t)"),
                    act[:].rearrange("p j t -> p (j t)"),
                    128,
                    bass_isa.ReduceOp.absmax,
                )
                mfin = stats.tile([128, NT], F32, tag="mfin", name=f"mfin_{e}")
                nc.vector.reduce_max(
                    out=mfin[:].unsqueeze(2),
                    in_=mall[:].rearrange("p j t -> p t j"),
                    axis=mybir.AxisListType.X,
                )
                mc = stats.tile([128, NT], F32, tag="mc", name=f"mc_{e}")
                nc.vector.tensor_scalar_max(out=mc[:], in0=mfin[:], scalar1=1e-12)
                rr = stats.tile([128, NT], F32, tag="rr", name=f"rr_{e}")
                nc.vector.reciprocal(out=rr[:], in_=mc[:])
                r127 = st[f"r127_{e}"] = stats.tile(
                    [128, NT], F32, tag="r127", name=f"r127_{e}"
                )
                nc.vector.tensor_scalar_mul(out=r127[:], in0=rr[:], scalar1=127.0)
                # s2c = (mc/127) * sx[tok] * comb[tok]  (sx folded in scrow)
                s2c = stats.tile([128, NT], F32, tag="s2c", name=f"s2c_{e}")
                nc.vector.scalar_tensor_tensor(
                    out=s2c[:],
                    in0=mc[:],
                    scalar=1.0 / 127.0,
                    in1=scb[:, e * NT : (e + 1) * NT],
                    op0=mult,
                    op1=mult,
                )
                s2cs.append(s2c)
            else:
                act, r127 = st[f"act_{e}"], st[f"r127_{e}"]
                tq = ep.tile([128, KI, NT], F32, tag="gate", name=f"tq_{e}")
                nc.vector.tensor_tensor(
                    out=tq[:],
                    in0=act[:],
                    in1=r127[:].unsqueeze(1).broadcast_to([128, KI, NT]),
                    op=mult,
                )
                trnd = ep.tile([128, KI, NT], F32, tag="mall", name=f"trnd_{e}")
                nc.vector.tensor_scalar_add(out=trnd[:], in0=tq[:], scalar1=MAGIC)
                aq = aqp.tile([128, KI, NT], BF16, tag="aq", name=f"aq_{e}")
                nc.vector.tensor_scalar_add(out=aq[:], in0=trnd[:], scalar1=-MAGIC)
                aqs.append(aq)

        def w2dma(e, g):
            w2i = st[f"w2i_{e}_{g}"] = w2i8p.tile(
                [128, KI, 512], I8, tag="w2i", name=f"w2i_{e}_{g}"
            )
            nc.sync.dma_start(out=w2i[:], in_=w2_d[e, :, :, g * 512 : (g + 1) * 512])

        def w2cast(e, g):
            w2i = st[f"w2i_{e}_{g}"]
            w2b = st[f"w2b_{e}_{g}"] = w2bfp.tile(
                [128, KI, 512], BF16, tag="w2b", name=f"w2b_{e}_{g}"
            )
            nc.vector.tensor_copy(out=w2b[:, :, 0:W2V], in_=w2i[:, :, 0:W2V])
            wam = (W2V + W2A) // 2
            nc.scalar.copy(out=w2b[:, :, W2V:wam], in_=w2i[:, :, W2V:wam])
            nc.scalar.copy(out=w2b[:, :, wam:W2A], in_=w2i[:, :, wam:W2A])
            wpm = (W2A + 512) // 2
            nc.gpsimd.tensor_copy(out=w2b[:, :, W2A:wpm], in_=w2i[:, :, W2A:wpm])
            nc.gpsimd.tensor_copy(out=w2b[:, :, wpm:512], in_=w2i[:, :, wpm:512])

        def g2mm(e, g):
            w2b = st[f"w2b_{e}_{g}"]
            # one full bank per tag; single accumulation group per bank
            ps2 = ps2p.tile([128, 8, NT], F32, tag=f"ps2{e}", name=f"ps2_{e}_{g}")
            for cc in range(4):
                for k in range(KI):
                    nc.tensor.matmul(
                        ps2[:, cc, :],
                        lhsT=w2b[:, k, cc * 128 : (cc + 1) * 128],
                        rhs=aqs[e][:, k, :],
                        start=(k == 0 and cc == 0),
                        stop=(k == KI - 1 and cc == 3),
                    )
            # deq2: o * w2sc[h] * (s2 * comb)[tok] -> yT columns
            w2s3 = (
                w2sc_s[e][:, g * 4 : (g + 1) * 4]
                .unsqueeze(2)
                .broadcast_to([128, 4, NT])
            )
            s2c3 = s2cs[e][:].unsqueeze(1).broadcast_to([128, 4, NT])
            dtmp = ep.tile([128, 4, NT], F32, tag="dtmp", name=f"dtmp_{e}_{g}")
            nc.vector.tensor_tensor(out=dtmp[:], in0=ps2[:, 0:4, :], in1=w2s3, op=mult)
            nc.vector.tensor_tensor(
                out=yT[:, g * 4 : (g + 1) * 4, e * NT : (e + 1) * NT],
                in0=dtmp[:],
                in1=s2c3,
                op=mult,
            )

        def ytail(g):
            # both experts done for these channel tiles: emit y rows
            if "pst" not in st:
                st["pst"] = ps1p.tile([128, 24, NT], F32, tag="ps1", name="pst")
            pst = st["pst"]
            views = [
                pst[:, 2 * cc : 2 * cc + 2, :].rearrange("p a t -> p (a t)")
                for cc in range(4)
            ]
            for cc in range(4):
                c = g * 4 + cc
                nc.tensor.transpose(views[cc], yT[:, c, :], ident[:])
            nc.scalar.copy(
                out=ysb[:, g * 512 : (g + 1) * 512],
                in_=pst[:, 0:8, :].rearrange("p a t -> p (a t)"),
            )
            nc.sync.dma_start(
                out=y_d[:, g * 512 : (g + 1) * 512],
                in_=ysb[:, g * 512 : (g + 1) * 512],
            )

        yT = yp.tile([128, CT2, NPOS], F32, name="yT")
        ysb = yp.tile([128, H], F32, name="ysb")

        g1_chunk(0, 0)
        scale_dmas()
        g1_chunk(1, 0)
        for kg in range(1, 4):
            g1_chunk(0, kg)
            g1_chunk(1, kg)
        for s in range(1, 5):
            epi(0, s)
        for e in range(EPC):
            for g in range(4):
                w2dma(e, g)
        for s in range(1, 5):
            epi(1, s)
        for g in range(4):
            w2cast(0, g)
            g2mm(0, g)
        for g in range(4):
            w2cast(1, g)
            g2mm(1, g)
            ytail(g)

    nc.compile()
    return nc


def get_program():
    if "nc" not in _cache:
        _cache["nc"] = _build_program()
    return _cache["nc"]


def _prep_inputs(x, expert_ids, smooth_scales, expert_scales, w1, w1_scale, w2, w2_scale):
    """Host-side dispatch: quantize x, route tokens, shard experts."""
    x = np.asarray(x, np.float32)
    expert_ids = np.asarray(expert_ids)
    smooth_scales = np.asarray(smooth_scales, np.float32)
    expert_scales = np.asarray(expert_scales, np.float32)
    w1_scale = np.asarray(w1_scale, np.float32)
    w2_scale = np.asarray(w2_scale, np.float32)

    # dynamic per-token int8 quantization (exact mirror of reference ops)
    sx = np.maximum(np.max(np.abs(x), axis=-1, keepdims=True), 1e-12) / 127.0
    xq = np.round(np.clip(x / sx, -128.0, 127.0)).astype(np.float32)

    # combine matrix [B, E]: scatter-add expert_scales at expert_ids
    comb = np.zeros((B, E), np.float32)
    np.add.at(comb, (np.arange(B)[:, None], expert_ids), expert_scales)

    w1v = w1.astype(np.int8)
    w2v = w2.astype(np.int8)

    in_maps = []
    pos2tok = np.zeros((NCORES, NPOS), np.int64)
    for c in range(NCORES):
        valid = np.zeros(NPOS, bool)
        ge_of_pos = np.zeros(NPOS, np.int64)
        for le in range(EPC):
            ge = EPC * c + le
            toks = np.nonzero((expert_ids == ge).any(axis=1))[0]
            assert len(toks) <= NT, f"expert {ge}: {len(toks)} tokens > capacity {NT}"
            pos2tok[c, le * NT : le * NT + len(toks)] = toks
            valid[le * NT : le * NT + len(toks)] = True
            ge_of_pos[le * NT : (le + 1) * NT] = ge

        toks_c = pos2tok[c]
        xg = xq[toks_c]  # [NPOS, H]
        xqT = np.ascontiguousarray(
            xg.T.reshape(KH, 128, NPOS).transpose(1, 0, 2)
        ).astype(ml_dtypes.bfloat16)
        sxrow = sx[toks_c, 0][None, :].astype(np.float32)
        combp = comb[toks_c, ge_of_pos] * valid  # zero at padding positions
        scrow = (sx[toks_c, 0] * combp)[None, :].astype(np.float32)

        es = list(range(EPC * c, EPC * (c + 1)))
        w1t = np.ascontiguousarray(
            w1v[es].reshape(EPC, KH, 128, I2).transpose(0, 2, 1, 3)
        )
        w2t = np.ascontiguousarray(
            w2v[es].reshape(EPC, KI, 128, H).transpose(0, 2, 1, 3)
        )
        w1sc = np.empty((EPC, 128, CT1), np.float32)
        w2sc = np.empty((EPC, 128, CT2), np.float32)
        for le, ge in enumerate(es):
            gatesc = w1_scale[ge, :I].reshape(KI, 128).T
            upsc = (w1_scale[ge, I:] * smooth_scales[ge]).reshape(KI, 128).T
            w1sc[le] = np.concatenate([gatesc, upsc], axis=1)
            w2sc[le] = w2_scale[ge].reshape(CT2, 128).T

        in_maps.append(
            {
                "xqT": xqT,
                "sxrow": sxrow,
                "scrow": scrow,
                "w1t": w1t,
                "w2t": w2t,
                "w1sc": w1sc,
                "w2sc": w2sc,
            }
        )
    return in_maps, pos2tok


def kernel(
    x,
    expert_ids,
    smooth_scales,
    expert_scales,
    x_active_mask,
    w1,
    w1_scale,
    w2,
    w2_scale,
    _trace=False,
    _trace_kwargs=None,
):
    in_maps, pos2tok = _prep_inputs(
        x, expert_ids, smooth_scales, expert_scales, w1, w1_scale, w2, w2_scale
    )
    nc = get_program()
    res = run_bass_kernel_spmd(
        nc,
        in_maps,
        core_ids=list(range(NCORES)),
        trace=_trace,
        **(_trace_kwargs or {}),
    )
    y = np.zeros((B, H), np.float32)
    for c, r in enumerate(res.results):
        np.add.at(y, pos2tok[c], r["y"].astype(np.float32))
    y *= np.asarray(x_active_mask).astype(np.float32)[:, None]
    if _trace:
        kernel.last_results = res
    return y


# revision 33
# speedup vs baseline: 1.4085x; 1.0416x over previous
"""Trainium2 Bass kernel for nn_DecodeMoeOps (MoE decode: dispatch-quant,
grouped int8 GEMM1, SwiGLU, requant, grouped int8 GEMM2, weighted combine).

Expert-parallel across 8 NeuronCores: core c owns experts {2c, 2c+1}.

Key design (v2):
- Weights ship to SBUF as RAW INT8 (1 B/weight over HWDGE) and are upcast
  to bf16 on-chip, split across the DVE / ACT / GPSIMD engines. This halves
  the DMA-device byte volume vs casting during the DMA (which is charged at
  bf16 output bytes).
- Both GEMMs run WEIGHT-STATIONARY (weights are the PE's lhsT), so PE time
  scales with the number of routed tokens, not the weight volume. Each
  expert gets a fixed 64-position block of gathered tokens (host routing);
  tokens routed to both of a core's experts appear in both blocks and the
  host scatter-adds per-position outputs back to token rows.
- GEMM1 output lands channel-major [ch, tok]; SwiGLU/requant run in that
  layout (cross-partition absmax via gpsimd.partition_all_reduce), which
  makes the requantized activations directly usable as GEMM2's moving
  operand with no transposes. Final [h, pos] -> [pos, h] via PE transpose.
"""

import os
import sys

for _p in ("/opt/trn_rl_repo", "/root/.axon_site/_ro/trn_rl_repo"):
    if os.path.isdir(_p) and _p not in sys.path:
        sys.path.insert(0, _p)

from contextlib import ExitStack

import ml_dtypes
import numpy as np

import concourse.bass as bass
import concourse.bass_isa as bass_isa
import concourse.mybir as mybir
import concourse.tile as tile
from concourse import bacc
from concourse.bass_utils import run_bass_kernel_spmd
from concourse.masks import make_identity

B, TOPK, H, I, E = 128, 8, 2048, 1408, 16
NCORES = 8
EPC = E // NCORES  # experts per core
KH = H // 128  # 16 contraction tiles for GEMM1
KI = I // 128  # 11 contraction tiles for GEMM2
I2 = 2 * I
CT1 = I2 // 128  # 22 GEMM1 output-channel tiles (gate 0..10, up 11..21)
CT2 = H // 128  # 16 GEMM2 output-channel tiles
NT = 64  # token positions per expert block
NPOS = EPC * NT  # 128 positions per core
F32 = mybir.dt.float32
BF16 = mybir.dt.bfloat16
I8 = mybir.dt.int8
MAGIC = float(3 * 2**22)  # fp32 round-to-nearest-int magic (covers negatives)

# int8 -> bf16 upcast split points (free-dim columns) per engine:
# [0:V) on DVE, [V:A) on ACT, [A:end) on GPSIMD.  Tunables.
W1V, W1A = 1280, 2304  # of I2 = 2816
W2V, W2A = 224, 400  # of 512-wide w2 column chunks

_cache: dict = {}


def _build_program():
    nc = bacc.Bacc(
        "TRN2",
        target_bir_lowering=False,
        debug=False,
        num_devices=NCORES,
    )
    mult = mybir.AluOpType.mult
    opmax = mybir.AluOpType.max

    # --- per-core DRAM I/O ---
    xqT_d = nc.dram_tensor("xqT", [128, KH, NPOS], BF16, kind="ExternalInput").ap()
    sxr_d = nc.dram_tensor("sxrow", [1, NPOS], F32, kind="ExternalInput").ap()
    scr_d = nc.dram_tensor("scrow", [1, NPOS], F32, kind="ExternalInput").ap()
    w1_d = nc.dram_tensor("w1t", [EPC, 128, KH, I2], I8, kind="ExternalInput").ap()
    w2_d = nc.dram_tensor("w2t", [EPC, 128, KI, H], I8, kind="ExternalInput").ap()
    w1sc_d = nc.dram_tensor("w1sc", [EPC, 128, CT1], F32, kind="ExternalInput").ap()
    w2sc_d = nc.dram_tensor("w2sc", [EPC, 128, CT2], F32, kind="ExternalInput").ap()
    y_d = nc.dram_tensor("y", [NPOS, H], F32, kind="ExternalOutput").ap()

    with tile.TileContext(nc) as tc, ExitStack() as ctx:
        consts = ctx.enter_context(tc.tile_pool(name="consts", bufs=1))
        w1i8p = ctx.enter_context(tc.tile_pool(name="w1i8", bufs=4))
        w1bfp = ctx.enter_context(tc.tile_pool(name="w1bf", bufs=2))
        w2i8p = ctx.enter_context(tc.tile_pool(name="w2i8", bufs=4))
        w2bfp = ctx.enter_context(tc.tile_pool(name="w2bf", bufs=2))
        ep = ctx.enter_context(tc.tile_pool(name="ep", bufs=2))
        stats = ctx.enter_context(tc.tile_pool(name="stats", bufs=2))
        aqp = ctx.enter_context(tc.tile_pool(name="aqp", bufs=2))
        yp = ctx.enter_context(tc.tile_pool(name="yp", bufs=1))
        ps1p = ctx.enter_context(tc.tile_pool(name="ps1", bufs=2, space="PSUM"))
        ps2p = ctx.enter_context(tc.tile_pool(name="ps2", bufs=1, space="PSUM"))

        # --- prologue ---
        xqT_s = consts.tile([128, KH, NPOS], BF16, name="xqT_s")
        nc.scalar.dma_start(out=xqT_s[:], in_=xqT_d)
        ident = consts.tile([128, 128], F32, name="ident")
        make_identity(nc, ident[:])
        ones1 = consts.tile([1, 128], F32, name="ones1")
        nc.vector.memset(ones1[:], 1.0)
        sxr_s = consts.tile([1, NPOS], F32, name="sxr_s")
        scr_s = consts.tile([1, NPOS], F32, name="scr_s")
        w1sc_s, w2sc_s = [], []
        for e in range(EPC):
            w1sc_s.append(consts.tile([128, CT1], F32, name=f"w1sc_{e}"))
            w2sc_s.append(consts.tile([128, CT2], F32, name=f"w2sc_{e}"))

        def scale_dmas():
            for e in range(EPC):
                nc.sync.dma_start(out=w1sc_s[e][:], in_=w1sc_d[e])
                nc.sync.dma_start(out=w2sc_s[e][:], in_=w2sc_d[e])

        # broadcast of per-position rows happens in late_prologue (emitted
        # after the first weight chunks so small DMAs don't hog HWDGE early);
        # the psum buffer is reserved here to keep the ps1 tag rotation.
        psb0 = ps1p.tile([128, 24, NT], F32, tag="ps1", name="psb0")
        sxb = consts.tile([128, NPOS], F32, name="sxb")
        scb = consts.tile([128, NPOS], F32, name="scb")

        def late_prologue():
            nc.scalar.dma_start(out=sxr_s[:], in_=sxr_d)
            nc.scalar.dma_start(out=scr_s[:], in_=scr_d)
            bc0 = psb0[:, 0:2, :].rearrange("p a t -> p (a t)")
            nc.tensor.matmul(bc0, lhsT=ones1[:], rhs=sxr_s[:], start=True, stop=True)
            nc.vector.tensor_copy(out=sxb[:], in_=bc0)
            bc1 = psb0[:, 2:4, :].rearrange("p a t -> p (a t)")
            nc.tensor.matmul(bc1, lhsT=ones1[:], rhs=scr_s[:], start=True, stop=True)
            nc.vector.tensor_copy(out=scb[:], in_=bc1)

        # Emission order == per-engine execution order, so the phases below
        # software-pipeline the kernel: each expert's epilogue is emitted in
        # slices spliced between the NEXT phase's chunk pipelines, keeping
        # every engine queue free of long head-of-line dependency waits.
        st = {}  # per-expert tiles carried across phases
        aqs, s2cs = [], []

        def g1_chunk(e, kg):
            if kg == 0:
                # 3 exact PSUM banks; one accumulation group per bank (8
                # chunks share a bank: start on the bank's first chunk, stop
                # on its last -- HW zeroing is lazy per-write in the region).
                st[f"ps1_{e}"] = ps1p.tile([128, 24, NT], F32, tag="ps1", name=f"ps1_{e}")
            ps1 = st[f"ps1_{e}"]
            w1i = w1i8p.tile([128, 4, I2], I8, tag="w1i", name=f"w1i_{e}_{kg}")
            nc.sync.dma_start(out=w1i[:], in_=w1_d[e, :, kg * 4 : (kg + 1) * 4, :])
            w1b = w1bfp.tile([128, 4, I2], BF16, tag="w1b", name=f"w1b_{e}_{kg}")
            for j in range(4):
                nc.vector.tensor_copy(out=w1b[:, j, 0:W1V], in_=w1i[:, j, 0:W1V])
                nc.scalar.copy(out=w1b[:, j, W1V:W1A], in_=w1i[:, j, W1V:W1A])
                nc.gpsimd.tensor_copy(out=w1b[:, j, W1A:I2], in_=w1i[:, j, W1A:I2])
            for j in range(4):
                k = kg * 4 + j
                for c in range(CT1):
                    nc.tensor.matmul(
                        ps1[:, c, :],
                        lhsT=w1b[:, j, c * 128 : (c + 1) * 128],
                        rhs=xqT_s[:, k, e * NT : (e + 1) * NT],
                        start=(k == 0 and c % 8 == 0),
                        stop=(k == KH - 1 and (c % 8 == 7 or c == CT1 - 1)),
                    )

        def epi(e, s):
            """Epilogue slice s (1..4) for expert e."""
            if s == 1:
                # dequant: releases ps1
                ps1 = st[f"ps1_{e}"]
                sxb64 = sxb[:, e * NT : (e + 1) * NT]
                gate = st[f"gate_{e}"] = ep.tile(
                    [128, KI, NT], F32, tag="gate", name=f"gate_{e}"
                )
                up = st[f"up_{e}"] = ep.tile(
                    [128, KI, NT], F32, tag="up", name=f"up_{e}"
                )
                # gate = psum * w1sc[ch] * sx[tok]; up = psum * w1sc_up[ch]
                # (sx deferred into s2c: aq is invariant to per-token scale)
                w1g = w1sc_s[e][:, 0:KI].unsqueeze(2).broadcast_to([128, KI, NT])
                w1u = w1sc_s[e][:, KI:CT1].unsqueeze(2).broadcast_to([128, KI, NT])
                sx3 = sxb64.unsqueeze(1).broadcast_to([128, KI, NT])
                nc.vector.tensor_tensor(out=gate[:], in0=ps1[:, 0:KI, :], in1=w1g, op=mult)
                nc.vector.tensor_tensor(out=gate[:], in0=gate[:], in1=sx3, op=mult)
                nc.vector.tensor_tensor(out=up[:], in0=ps1[:, KI : 2 * KI, :], in1=w1u, op=mult)
            elif s == 2:
                gate, up = st[f"gate_{e}"], st[f"up_{e}"]
                sig = ep.tile([128, KI, NT], F32, tag="sig", name=f"sig_{e}")
                nc.scalar.activation(
                    out=sig[:], in_=gate[:], func=mybir.ActivationFunctionType.Sigmoid
                )
                gsig = ep.tile([128, KI, NT], F32, tag="gsig", name=f"gsig_{e}")
                nc.vector.tensor_tensor(out=gsig[:], in0=gate[:], in1=sig[:], op=mult)
                act = st[f"act_{e}"] = ep.tile(
                    [128, KI, NT], F32, tag="act", name=f"act_{e}"
                )
                nc.vector.tensor_tensor(out=act[:], in0=gsig[:], in1=up[:], op=mult)
            elif s == 3:
                act = st[f"act_{e}"]
                # per-token absmax over all I channels (partitions x 11 tiles)
                mall = st[f"mall_{e}"] = ep.tile(
                    [128, KI, NT], F32, tag="mall", name=f"mall_{e}"
                )
                nc.gpsimd.partition_all_reduce(
                    mall[:].rearrange("p j t -> p (j t)"),
                    act[:].rearrange("p j t -> p (j t)"),
                    128,
                    bass_isa.ReduceOp.absmax,
                )
                mfin = stats.tile([128, NT], F32, tag="mfin", name=f"mfin_{e}")
                nc.vector.reduce_max(
                    out=mfin[:].unsqueeze(2),
                    in_=mall[:].rearrange("p j t -> p t j"),
                    axis=mybir.AxisListType.X,
                )
                mc = stats.tile([128, NT], F32, tag="mc", name=f"mc_{e}")
                nc.vector.tensor_scalar_max(out=mc[:], in0=mfin[:], scalar1=1e-12)
                rr = stats.tile([128, NT], F32, tag="rr", name=f"rr_{e}")
                nc.vector.reciprocal(out=rr[:], in_=mc[:])
                r127 = st[f"r127_{e}"] = stats.tile(
                    [128, NT], F32, tag="r127", name=f"r127_{e}"
                )
                nc.vector.tensor_scalar_mul(out=r127[:], in0=rr[:], scalar1=127.0)
                # s2c = (mc/127) * sx[tok] * comb[tok]  (sx folded in scrow)
                s2c = stats.tile([128, NT], F32, tag="s2c", name=f"s2c_{e}")
                nc.vector.scalar_tensor_tensor(
                    out=s2c[:],
                    in0=mc[:],
                    scalar=1.0 / 127.0,
                    in1=scb[:, e * NT : (e + 1) * NT],
                    op0=mult,
                    op1=mult,
                )
                s2cs.append(s2c)
            else:
                act, r127 = st[f"act_{e}"], st[f"r127_{e}"]
                tq = ep.tile([128, KI, NT], F32, tag="gate", name=f"tq_{e}")
                nc.vector.tensor_tensor(
                    out=tq[:],
                    in0=act[:],
                    in1=r127[:].unsqueeze(1).broadcast_to([128, KI, NT]),
                    op=mult,
                )
                aq = aqp.tile([128, KI, NT], BF16, tag="aq", name=f"aq_{e}")
                nc.vector.tensor_scalar(
                    out=aq[:],
                    in0=tq[:],
                    scalar1=MAGIC,
                    scalar2=MAGIC,
                    op0=mybir.AluOpType.add,
                    op1=mybir.AluOpType.subtract,
                )
                aqs.append(aq)

        def w2dma(e, g):
            w2i = st[f"w2i_{e}_{g}"] = w2i8p.tile(
                [128, KI, 512], I8, tag="w2i", name=f"w2i_{e}_{g}"
            )
            nc.sync.dma_start(out=w2i[:], in_=w2_d[e, :, :, g * 512 : (g + 1) * 512])

        def w2cast(e, g):
            w2i = st[f"w2i_{e}_{g}"]
            w2b = st[f"w2b_{e}_{g}"] = w2bfp.tile(
                [128, KI, 512], BF16, tag="w2b", name=f"w2b_{e}_{g}"
            )
            nc.vector.tensor_copy(out=w2b[:, :, 0:W2V], in_=w2i[:, :, 0:W2V])
            wam = (W2V + W2A) // 2
            nc.scalar.copy(out=w2b[:, :, W2V:wam], in_=w2i[:, :, W2V:wam])
            nc.scalar.copy(out=w2b[:, :, wam:W2A], in_=w2i[:, :, wam:W2A])
            wpm = (W2A + 512) // 2
            nc.gpsimd.tensor_copy(out=w2b[:, :, W2A:wpm], in_=w2i[:, :, W2A:wpm])
            nc.gpsimd.tensor_copy(out=w2b[:, :, wpm:512], in_=w2i[:, :, wpm:512])

        def g2mm(e, g):
            w2b = st[f"w2b_{e}_{g}"]
            # one full bank per tag; single accumulation group per bank
            ps2 = ps2p.tile([128, 8, NT], F32, tag=f"ps2{e}", name=f"ps2_{e}_{g}")
            for cc in range(4):
                for k in range(KI):
                    nc.tensor.matmul(
                        ps2[:, cc, :],
                        lhsT=w2b[:, k, cc * 128 : (cc + 1) * 128],
                        rhs=aqs[e][:, k, :],
                        start=(k == 0 and cc == 0),
                        stop=(k == KI - 1 and cc == 3),
                    )
            # deq2: o * w2sc[h] * (s2 * comb)[tok] -> yT columns
            w2s3 = (
                w2sc_s[e][:, g * 4 : (g + 1) * 4]
                .unsqueeze(2)
                .broadcast_to([128, 4, NT])
            )
            s2c3 = s2cs[e][:].unsqueeze(1).broadcast_to([128, 4, NT])
            dtmp = ep.tile([128, 4, NT], F32, tag="dtmp", name=f"dtmp_{e}_{g}")
            nc.vector.tensor_tensor(out=dtmp[:], in0=ps2[:, 0:4, :], in1=w2s3, op=mult)
            nc.vector.tensor_tensor(
                out=yT[:, g * 4 : (g + 1) * 4, e * NT : (e + 1) * NT],
                in0=dtmp[:],
                in1=s2c3,
                op=mult,
            )

        def ytail(g):
            # both experts done for these channel tiles: emit y rows
            if "pst" not in st:
                st["pst"] = ps1p.tile([128, 24, NT], F32, tag="ps1", name="pst")
            pst = st["pst"]
            views = [
                pst[:, 2 * cc : 2 * cc + 2, :].rearrange("p a t -> p (a t)")
                for cc in range(4)
            ]
            for cc in range(4):
                c = g * 4 + cc
                nc.tensor.transpose(views[cc], yT[:, c, :], ident[:])
            nc.scalar.copy(
                out=ysb[:, g * 512 : (g + 1) * 512],
                in_=pst[:, 0:8, :].rearrange("p a t -> p (a t)"),
            )
            nc.sync.dma_start(
                out=y_d[:, g * 512 : (g + 1) * 512],
                in_=ysb[:, g * 512 : (g + 1) * 512],
            )

        yT = yp.tile([128, CT2, NPOS], F32, name="yT")
        ysb = yp.tile([128, H], F32, name="ysb")

        g1_chunk(0, 0)
        g1_chunk(1, 0)
        scale_dmas()
        late_prologue()
        for kg in range(1, 4):
            g1_chunk(0, kg)
            g1_chunk(1, kg)
        for s in range(1, 5):
            epi(0, s)
            epi(1, s)
        for e in range(EPC):
            for g in range(4):
                w2dma(e, g)
        for g in range(4):
            w2cast(0, g)
            g2mm(0, g)
            w2cast(1, g)
            g2mm(1, g)
            ytail(g)

    nc.compile()
    return nc


def get_program():
    if "nc" not in _cache:
        _cache["nc"] = _build_program()
    return _cache["nc"]


def _prep_inputs(x, expert_ids, smooth_scales, expert_scales, w1, w1_scale, w2, w2_scale):
    """Host-side dispatch: quantize x, route tokens, shard experts."""
    x = np.asarray(x, np.float32)
    expert_ids = np.asarray(expert_ids)
    smooth_scales = np.asarray(smooth_scales, np.float32)
    expert_scales = np.asarray(expert_scales, np.float32)
    w1_scale = np.asarray(w1_scale, np.float32)
    w2_scale = np.asarray(w2_scale, np.float32)

    # dynamic per-token int8 quantization (exact mirror of reference ops)
    sx = np.maximum(np.max(np.abs(x), axis=-1, keepdims=True), 1e-12) / 127.0
    xq = np.round(np.clip(x / sx, -128.0, 127.0)).astype(np.float32)

    # combine matrix [B, E]: scatter-add expert_scales at expert_ids
    comb = np.zeros((B, E), np.float32)
    np.add.at(comb, (np.arange(B)[:, None], expert_ids), expert_scales)

    w1v = w1.astype(np.int8)
    w2v = w2.astype(np.int8)

    in_maps = []
    pos2tok = np.zeros((NCORES, NPOS), np.int64)
    for c in range(NCORES):
        valid = np.zeros(NPOS, bool)
        ge_of_pos = np.zeros(NPOS, np.int64)
        for le in range(EPC):
            ge = EPC * c + le
            toks = np.nonzero((expert_ids == ge).any(axis=1))[0]
            assert len(toks) <= NT, f"expert {ge}: {len(toks)} tokens > capacity {NT}"
            pos2tok[c, le * NT : le * NT + len(toks)] = toks
            valid[le * NT : le * NT + len(toks)] = True
            ge_of_pos[le * NT : (le + 1) * NT] = ge

        toks_c = pos2tok[c]
        xg = xq[toks_c]  # [NPOS, H]
        xqT = np.ascontiguousarray(
            xg.T.reshape(KH, 128, NPOS).transpose(1, 0, 2)
        ).astype(ml_dtypes.bfloat16)
        sxrow = sx[toks_c, 0][None, :].astype(np.float32)
        combp = comb[toks_c, ge_of_pos] * valid  # zero at padding positions
        scrow = (sx[toks_c, 0] * combp)[None, :].astype(np.float32)

        es = list(range(EPC * c, EPC * (c + 1)))
        w1t = np.ascontiguousarray(
            w1v[es].reshape(EPC, KH, 128, I2).transpose(0, 2, 1, 3)
        )
        w2t = np.ascontiguousarray(
            w2v[es].reshape(EPC, KI, 128, H).transpose(0, 2, 1, 3)
        )
        w1sc = np.empty((EPC, 128, CT1), np.float32)
        w2sc = np.empty((EPC, 128, CT2), np.float32)
        for le, ge in enumerate(es):
            gatesc = w1_scale[ge, :I].reshape(KI, 128).T
            upsc = (w1_scale[ge, I:] * smooth_scales[ge]).reshape(KI, 128).T
            w1sc[le] = np.concatenate([gatesc, upsc], axis=1)
            w2sc[le] = w2_scale[ge].reshape(CT2, 128).T

        in_maps.append(
            {
                "xqT": xqT,
                "sxrow": sxrow,
                "scrow": scrow,
                "w1t": w1t,
                "w2t": w2t,
                "w1sc": w1sc,
                "w2sc": w2sc,
            }
        )
    return in_maps, pos2tok


def kernel(
    x,
    expert_ids,
    smooth_scales,
    expert_scales,
    x_active_mask,
    w1,
    w1_scale,
    w2,
    w2_scale,
    _trace=False,
    _trace_kwargs=None,
):
    in_maps, pos2tok = _prep_inputs(
        x, expert_ids, smooth_scales, expert_scales, w1, w1_scale, w2, w2_scale
    )
    nc = get_program()
    res = run_bass_kernel_spmd(
        nc,
        in_maps,
        core_ids=list(range(NCORES)),
        trace=_trace,
        **(_trace_kwargs or {}),
    )
    y = np.zeros((B, H), np.float32)
    for c, r in enumerate(res.results):
        np.add.at(y, pos2tok[c], r["y"].astype(np.float32))
    y *= np.asarray(x_active_mask).astype(np.float32)[:, None]
    if _trace:
        kernel.last_results = res
    return y


# revision 38
# speedup vs baseline: 1.4250x; 1.0117x over previous
"""Trainium2 Bass kernel for nn_DecodeMoeOps (MoE decode: dispatch-quant,
grouped int8 GEMM1, SwiGLU, requant, grouped int8 GEMM2, weighted combine).

Expert-parallel across 8 NeuronCores: core c owns experts {2c, 2c+1}.

Key design (v2):
- Weights ship to SBUF as RAW INT8 (1 B/weight over HWDGE) and are upcast
  to bf16 on-chip, split across the DVE / ACT / GPSIMD engines. This halves
  the DMA-device byte volume vs casting during the DMA (which is charged at
  bf16 output bytes).
- Both GEMMs run WEIGHT-STATIONARY (weights are the PE's lhsT), so PE time
  scales with the number of routed tokens, not the weight volume. Each
  expert gets a fixed 64-position block of gathered tokens (host routing);
  tokens routed to both of a core's experts appear in both blocks and the
  host scatter-adds per-position outputs back to token rows.
- GEMM1 output lands channel-major [ch, tok]; SwiGLU/requant run in that
  layout (cross-partition absmax via gpsimd.partition_all_reduce), which
  makes the requantized activations directly usable as GEMM2's moving
  operand with no transposes. Final [h, pos] -> [pos, h] via PE transpose.
"""

import os
import sys

for _p in ("/opt/trn_rl_repo", "/root/.axon_site/_ro/trn_rl_repo"):
    if os.path.isdir(_p) and _p not in sys.path:
        sys.path.insert(0, _p)

from contextlib import ExitStack

import ml_dtypes
import numpy as np

import concourse.bass as bass
import concourse.bass_isa as bass_isa
import concourse.mybir as mybir
import concourse.tile as tile
from concourse import bacc
from concourse.bass_utils import run_bass_kernel_spmd
from concourse.masks import make_identity

B, TOPK, H, I, E = 128, 8, 2048, 1408, 16
NCORES = 8
EPC = E // NCORES  # experts per core
KH = H // 128  # 16 contraction tiles for GEMM1
KI = I // 128  # 11 contraction tiles for GEMM2
I2 = 2 * I
CT1 = I2 // 128  # 22 GEMM1 output-channel tiles (gate 0..10, up 11..21)
CT2 = H // 128  # 16 GEMM2 output-channel tiles
NT = 64  # token positions per expert block
NPOS = EPC * NT  # 128 positions per core
F32 = mybir.dt.float32
BF16 = mybir.dt.bfloat16
I8 = mybir.dt.int8
MAGIC = float(3 * 2**22)  # fp32 round-to-nearest-int magic (covers negatives)

# int8 -> bf16 upcast split points (free-dim columns) per engine:
# [0:V) on DVE, [V:A) on ACT, [A:end) on GPSIMD.  Tunables.
W1V, W1A = 1280, 2304  # of I2 = 2816
W2V, W2A = 224, 400  # of 512-wide w2 column chunks

_cache: dict = {}


def _build_program():
    nc = bacc.Bacc(
        "TRN2",
        target_bir_lowering=False,
        debug=False,
        num_devices=NCORES,
    )
    mult = mybir.AluOpType.mult
    opmax = mybir.AluOpType.max

    # --- per-core DRAM I/O ---
    xqT_d = nc.dram_tensor("xqT", [128, KH, NPOS], BF16, kind="ExternalInput").ap()
    sxr_d = nc.dram_tensor("sxrow", [1, NPOS], F32, kind="ExternalInput").ap()
    scr_d = nc.dram_tensor("scrow", [1, NPOS], F32, kind="ExternalInput").ap()
    w1_d = nc.dram_tensor("w1t", [EPC, 128, KH, I2], I8, kind="ExternalInput").ap()
    w2_d = nc.dram_tensor("w2t", [EPC, 128, KI, H], I8, kind="ExternalInput").ap()
    w1sc_d = nc.dram_tensor("w1sc", [EPC, 128, CT1], F32, kind="ExternalInput").ap()
    w2sc_d = nc.dram_tensor("w2sc", [EPC, 128, CT2], F32, kind="ExternalInput").ap()
    y_d = nc.dram_tensor("y", [NPOS, H], F32, kind="ExternalOutput").ap()

    with tile.TileContext(nc) as tc, ExitStack() as ctx:
        consts = ctx.enter_context(tc.tile_pool(name="consts", bufs=1))
        w1i8p = ctx.enter_context(tc.tile_pool(name="w1i8", bufs=4))
        w1bfp = ctx.enter_context(tc.tile_pool(name="w1bf", bufs=2))
        w2i8p = ctx.enter_context(tc.tile_pool(name="w2i8", bufs=4))
        w2bfp = ctx.enter_context(tc.tile_pool(name="w2bf", bufs=2))
        ep = ctx.enter_context(tc.tile_pool(name="ep", bufs=2))
        stats = ctx.enter_context(tc.tile_pool(name="stats", bufs=2))
        aqp = ctx.enter_context(tc.tile_pool(name="aqp", bufs=2))
        yp = ctx.enter_context(tc.tile_pool(name="yp", bufs=1))
        ps1p = ctx.enter_context(tc.tile_pool(name="ps1", bufs=2, space="PSUM"))
        ps2p = ctx.enter_context(tc.tile_pool(name="ps2", bufs=1, space="PSUM"))

        # --- prologue ---
        xqT_s = consts.tile([128, KH, NPOS], BF16, name="xqT_s")
        nc.scalar.dma_start(out=xqT_s[:], in_=xqT_d)
        ident = consts.tile([128, 128], F32, name="ident")
        make_identity(nc, ident[:])
        ones1 = consts.tile([1, 128], F32, name="ones1")
        nc.vector.memset(ones1[:], 1.0)
        sxr_s = consts.tile([1, NPOS], F32, name="sxr_s")
        scr_s = consts.tile([1, NPOS], F32, name="scr_s")
        w1sc_s, w2sc_s = [], []
        for e in range(EPC):
            w1sc_s.append(consts.tile([128, CT1], F32, name=f"w1sc_{e}"))
            w2sc_s.append(consts.tile([128, CT2], F32, name=f"w2sc_{e}"))

        def scale_dmas():
            for e in range(EPC):
                nc.sync.dma_start(out=w1sc_s[e][:], in_=w1sc_d[e])
                nc.sync.dma_start(out=w2sc_s[e][:], in_=w2sc_d[e])

        # broadcast of per-position rows happens in late_prologue (emitted
        # after the first weight chunks so small DMAs don't hog HWDGE early);
        # the psum buffer is reserved here to keep the ps1 tag rotation.
        psb0 = ps1p.tile([128, 24, NT], F32, tag="ps1", name="psb0")
        sxb = consts.tile([128, NPOS], F32, name="sxb")
        scb = consts.tile([128, NPOS], F32, name="scb")

        def late_prologue():
            nc.scalar.dma_start(out=sxr_s[:], in_=sxr_d)
            nc.scalar.dma_start(out=scr_s[:], in_=scr_d)
            bc0 = psb0[:, 0:2, :].rearrange("p a t -> p (a t)")
            nc.tensor.matmul(bc0, lhsT=ones1[:], rhs=sxr_s[:], start=True, stop=True)
            nc.vector.tensor_copy(out=sxb[:], in_=bc0)
            bc1 = psb0[:, 2:4, :].rearrange("p a t -> p (a t)")
            nc.tensor.matmul(bc1, lhsT=ones1[:], rhs=scr_s[:], start=True, stop=True)
            nc.vector.tensor_copy(out=scb[:], in_=bc1)

        # Emission order == per-engine execution order, so the phases below
        # software-pipeline the kernel: each expert's epilogue is emitted in
        # slices spliced between the NEXT phase's chunk pipelines, keeping
        # every engine queue free of long head-of-line dependency waits.
        st = {}  # per-expert tiles carried across phases
        aqs, s2cs = [], []

        def g1_chunk(e, kg):
            if kg == 0:
                # 3 exact PSUM banks; one accumulation group per bank (8
                # chunks share a bank: start on the bank's first chunk, stop
                # on its last -- HW zeroing is lazy per-write in the region).
                st[f"ps1_{e}"] = ps1p.tile([128, 24, NT], F32, tag="ps1", name=f"ps1_{e}")
            ps1 = st[f"ps1_{e}"]
            w1i = w1i8p.tile([128, 4, I2], I8, tag="w1i", name=f"w1i_{e}_{kg}")
            nc.sync.dma_start(out=w1i[:], in_=w1_d[e, :, kg * 4 : (kg + 1) * 4, :])
            w1b = w1bfp.tile([128, 4, I2], BF16, tag="w1b", name=f"w1b_{e}_{kg}")
            for j in (0, 2):
                nc.vector.tensor_copy(
                    out=w1b[:, j : j + 2, 0:W1V], in_=w1i[:, j : j + 2, 0:W1V]
                )
                nc.scalar.copy(
                    out=w1b[:, j : j + 2, W1V:W1A], in_=w1i[:, j : j + 2, W1V:W1A]
                )
                nc.gpsimd.tensor_copy(
                    out=w1b[:, j : j + 2, W1A:I2], in_=w1i[:, j : j + 2, W1A:I2]
                )
            for j in range(4):
                k = kg * 4 + j
                for c in range(CT1):
                    nc.tensor.matmul(
                        ps1[:, c, :],
                        lhsT=w1b[:, j, c * 128 : (c + 1) * 128],
                        rhs=xqT_s[:, k, e * NT : (e + 1) * NT],
                        start=(k == 0 and c % 8 == 0),
                        stop=(k == KH - 1 and (c % 8 == 7 or c == CT1 - 1)),
                    )

        def epi(e, s):
            """Epilogue slice s (1..4) for expert e."""
            if s == 1:
                # dequant: releases ps1
                ps1 = st[f"ps1_{e}"]
                sxb64 = sxb[:, e * NT : (e + 1) * NT]
                gate = st[f"gate_{e}"] = ep.tile(
                    [128, KI, NT], F32, tag="gate", name=f"gate_{e}"
                )
                up = st[f"up_{e}"] = ep.tile(
                    [128, KI, NT], F32, tag="up", name=f"up_{e}"
                )
                # gate = psum * w1sc[ch] * sx[tok]; up = psum * w1sc_up[ch]
                # (sx deferred into s2c: aq is invariant to per-token scale)
                w1g = w1sc_s[e][:, 0:KI].unsqueeze(2).broadcast_to([128, KI, NT])
                w1u = w1sc_s[e][:, KI:CT1].unsqueeze(2).broadcast_to([128, KI, NT])
                sx3 = sxb64.unsqueeze(1).broadcast_to([128, KI, NT])
                nc.vector.tensor_tensor(out=gate[:], in0=ps1[:, 0:KI, :], in1=w1g, op=mult)
                nc.vector.tensor_tensor(out=gate[:], in0=gate[:], in1=sx3, op=mult)
                nc.vector.tensor_tensor(out=up[:], in0=ps1[:, KI : 2 * KI, :], in1=w1u, op=mult)
            elif s == 2:
                gate, up = st[f"gate_{e}"], st[f"up_{e}"]
                sig = ep.tile([128, KI, NT], F32, tag="sig", name=f"sig_{e}")
                nc.scalar.activation(
                    out=sig[:], in_=gate[:], func=mybir.ActivationFunctionType.Sigmoid
                )
                gsig = ep.tile([128, KI, NT], F32, tag="gsig", name=f"gsig_{e}")
                nc.vector.tensor_tensor(out=gsig[:], in0=gate[:], in1=sig[:], op=mult)
                act = st[f"act_{e}"] = ep.tile(
                    [128, KI, NT], F32, tag="act", name=f"act_{e}"
                )
                nc.vector.tensor_tensor(out=act[:], in0=gsig[:], in1=up[:], op=mult)
            elif s == 3:
                act = st[f"act_{e}"]
                # per-token absmax over all I channels (partitions x 11 tiles)
                mall = st[f"mall_{e}"] = ep.tile(
                    [128, KI, NT], F32, tag="mall", name=f"mall_{e}"
                )
                nc.gpsimd.partition_all_reduce(
                    mall[:].rearrange("p j t -> p (j t)"),
                    act[:].rearrange("p j t -> p (j t)"),
                    128,
                    bass_isa.ReduceOp.absmax,
                )
                mfin = stats.tile([128, NT], F32, tag="mfin", name=f"mfin_{e}")
                nc.vector.reduce_max(
                    out=mfin[:].unsqueeze(2),
                    in_=mall[:].rearrange("p j t -> p t j"),
                    axis=mybir.AxisListType.X,
                )
                mc = stats.tile([128, NT], F32, tag="mc", name=f"mc_{e}")
                nc.vector.tensor_scalar_max(out=mc[:], in0=mfin[:], scalar1=1e-12)
                rr = stats.tile([128, NT], F32, tag="rr", name=f"rr_{e}")
                nc.vector.reciprocal(out=rr[:], in_=mc[:])
                r127 = st[f"r127_{e}"] = stats.tile(
                    [128, NT], F32, tag="r127", name=f"r127_{e}"
                )
                nc.vector.tensor_scalar_mul(out=r127[:], in0=rr[:], scalar1=127.0)
                # s2c = (mc/127) * sx[tok] * comb[tok]  (sx folded in scrow)
                s2c = stats.tile([128, NT], F32, tag="s2c", name=f"s2c_{e}")
                nc.vector.scalar_tensor_tensor(
                    out=s2c[:],
                    in0=mc[:],
                    scalar=1.0 / 127.0,
                    in1=scb[:, e * NT : (e + 1) * NT],
                    op0=mult,
                    op1=mult,
                )
                s2cs.append(s2c)
            else:
                act, r127 = st[f"act_{e}"], st[f"r127_{e}"]
                tq = ep.tile([128, KI, NT], F32, tag="gate", name=f"tq_{e}")
                nc.vector.tensor_tensor(
                    out=tq[:],
                    in0=act[:],
                    in1=r127[:].unsqueeze(1).broadcast_to([128, KI, NT]),
                    op=mult,
                )
                aq = aqp.tile([128, KI, NT], BF16, tag="aq", name=f"aq_{e}")
                nc.vector.tensor_scalar(
                    out=aq[:],
                    in0=tq[:],
                    scalar1=MAGIC,
                    scalar2=MAGIC,
                    op0=mybir.AluOpType.add,
                    op1=mybir.AluOpType.subtract,
                )
                aqs.append(aq)

        def w2dma(e, g):
            w2i = st[f"w2i_{e}_{g}"] = w2i8p.tile(
                [128, KI, 512], I8, tag="w2i", name=f"w2i_{e}_{g}"
            )
            nc.sync.dma_start(out=w2i[:], in_=w2_d[e, :, :, g * 512 : (g + 1) * 512])

        def w2cast(e, g):
            w2i = st[f"w2i_{e}_{g}"]
            w2b = st[f"w2b_{e}_{g}"] = w2bfp.tile(
                [128, KI, 512], BF16, tag="w2b", name=f"w2b_{e}_{g}"
            )
            nc.vector.tensor_copy(out=w2b[:, :, 0:W2V], in_=w2i[:, :, 0:W2V])
            nc.scalar.copy(out=w2b[:, :, W2V:W2A], in_=w2i[:, :, W2V:W2A])
            nc.gpsimd.tensor_copy(out=w2b[:, :, W2A:512], in_=w2i[:, :, W2A:512])

        def g2mm(e, g):
            w2b = st[f"w2b_{e}_{g}"]
            # one full bank per tag; single accumulation group per bank
            ps2 = ps2p.tile([128, 8, NT], F32, tag=f"ps2{e}", name=f"ps2_{e}_{g}")
            for cc in range(4):
                for k in range(KI):
                    nc.tensor.matmul(
                        ps2[:, cc, :],
                        lhsT=w2b[:, k, cc * 128 : (cc + 1) * 128],
                        rhs=aqs[e][:, k, :],
                        start=(k == 0 and cc == 0),
                        stop=(k == KI - 1 and cc == 3),
                    )
            # deq2: o * w2sc[h] * (s2 * comb)[tok] -> yT columns
            w2s3 = (
                w2sc_s[e][:, g * 4 : (g + 1) * 4]
                .unsqueeze(2)
                .broadcast_to([128, 4, NT])
            )
            s2c3 = s2cs[e][:].unsqueeze(1).broadcast_to([128, 4, NT])
            dtmp = ep.tile([128, 4, NT], F32, tag="dtmp", name=f"dtmp_{e}_{g}")
            nc.vector.tensor_tensor(out=dtmp[:], in0=ps2[:, 0:4, :], in1=w2s3, op=mult)
            nc.vector.tensor_tensor(
                out=yT[:, g * 4 : (g + 1) * 4, e * NT : (e + 1) * NT],
                in0=dtmp[:],
                in1=s2c3,
                op=mult,
            )

        def ytail(g):
            # both experts done for these channel tiles: emit y rows
            if "pst" not in st:
                st["pst"] = ps1p.tile([128, 24, NT], F32, tag="ps1", name="pst")
            pst = st["pst"]
            views = [
                pst[:, 2 * cc : 2 * cc + 2, :].rearrange("p a t -> p (a t)")
                for cc in range(4)
            ]
            for cc in range(4):
                c = g * 4 + cc
                nc.tensor.transpose(views[cc], yT[:, c, :], ident[:])
            nc.scalar.copy(
                out=ysb[:, g * 512 : (g + 1) * 512],
                in_=pst[:, 0:8, :].rearrange("p a t -> p (a t)"),
            )
            nc.sync.dma_start(
                out=y_d[:, g * 512 : (g + 1) * 512],
                in_=ysb[:, g * 512 : (g + 1) * 512],
            )

        yT = yp.tile([128, CT2, NPOS], F32, name="yT")
        ysb = yp.tile([128, H], F32, name="ysb")

        g1_chunk(0, 0)
        g1_chunk(1, 0)
        scale_dmas()
        late_prologue()
        for kg in range(1, 4):
            g1_chunk(0, kg)
            g1_chunk(1, kg)
        for s in range(1, 5):
            epi(0, s)
            epi(1, s)
        for e in range(EPC):
            for g in range(4):
                w2dma(e, g)
        for g in range(4):
            w2cast(0, g)
            g2mm(0, g)
            w2cast(1, g)
            g2mm(1, g)
            ytail(g)

    nc.compile()
    return nc


def get_program():
    if "nc" not in _cache:
        _cache["nc"] = _build_program()
    return _cache["nc"]


def _prep_inputs(x, expert_ids, smooth_scales, expert_scales, w1, w1_scale, w2, w2_scale):
    """Host-side dispatch: quantize x, route tokens, shard experts."""
    x = np.asarray(x, np.float32)
    expert_ids = np.asarray(expert_ids)
    smooth_scales = np.asarray(smooth_scales, np.float32)
    expert_scales = np.asarray(expert_scales, np.float32)
    w1_scale = np.asarray(w1_scale, np.float32)
    w2_scale = np.asarray(w2_scale, np.float32)

    # dynamic per-token int8 quantization (exact mirror of reference ops)
    sx = np.maximum(np.max(np.abs(x), axis=-1, keepdims=True), 1e-12) / 127.0
    xq = np.round(np.clip(x / sx, -128.0, 127.0)).astype(np.float32)

    # combine matrix [B, E]: scatter-add expert_scales at expert_ids
    comb = np.zeros((B, E), np.float32)
    np.add.at(comb, (np.arange(B)[:, None], expert_ids), expert_scales)

    w1v = w1.astype(np.int8)
    w2v = w2.astype(np.int8)

    in_maps = []
    pos2tok = np.zeros((NCORES, NPOS), np.int64)
    for c in range(NCORES):
        valid = np.zeros(NPOS, bool)
        ge_of_pos = np.zeros(NPOS, np.int64)
        for le in range(EPC):
            ge = EPC * c + le
            toks = np.nonzero((expert_ids == ge).any(axis=1))[0]
            assert len(toks) <= NT, f"expert {ge}: {len(toks)} tokens > capacity {NT}"
            pos2tok[c, le * NT : le * NT + len(toks)] = toks
            valid[le * NT : le * NT + len(toks)] = True
            ge_of_pos[le * NT : (le + 1) * NT] = ge

        toks_c = pos2tok[c]
        xg = xq[toks_c]  # [NPOS, H]
        xqT = np.ascontiguousarray(
            xg.T.reshape(KH, 128, NPOS).transpose(1, 0, 2)
        ).astype(ml_dtypes.bfloat16)
        sxrow = sx[toks_c, 0][None, :].astype(np.float32)
        combp = comb[toks_c, ge_of_pos] * valid  # zero at padding positions
        scrow = (sx[toks_c, 0] * combp)[None, :].astype(np.float32)

        es = list(range(EPC * c, EPC * (c + 1)))
        w1t = np.ascontiguousarray(
            w1v[es].reshape(EPC, KH, 128, I2).transpose(0, 2, 1, 3)
        )
        w2t = np.ascontiguousarray(
            w2v[es].reshape(EPC, KI, 128, H).transpose(0, 2, 1, 3)
        )
        w1sc = np.empty((EPC, 128, CT1), np.float32)
        w2sc = np.empty((EPC, 128, CT2), np.float32)
        for le, ge in enumerate(es):
            gatesc = w1_scale[ge, :I].reshape(KI, 128).T
            upsc = (w1_scale[ge, I:] * smooth_scales[ge]).reshape(KI, 128).T
            w1sc[le] = np.concatenate([gatesc, upsc], axis=1)
            w2sc[le] = w2_scale[ge].reshape(CT2, 128).T

        in_maps.append(
            {
                "xqT": xqT,
                "sxrow": sxrow,
                "scrow": scrow,
                "w1t": w1t,
                "w2t": w2t,
                "w1sc": w1sc,
                "w2sc": w2sc,
            }
        )
    return in_maps, pos2tok


def kernel(
    x,
    expert_ids,
    smooth_scales,
    expert_scales,
    x_active_mask,
    w1,
    w1_scale,
    w2,
    w2_scale,
    _trace=False,
    _trace_kwargs=None,
):
    in_maps, pos2tok = _prep_inputs(
        x, expert_ids, smooth_scales, expert_scales, w1, w1_scale, w2, w2_scale
    )
    nc = get_program()
    res = run_bass_kernel_spmd(
        nc,
        in_maps,
        core_ids=list(range(NCORES)),
        trace=_trace,
        **(_trace_kwargs or {}),
    )
    y = np.zeros((B, H), np.float32)
    for c, r in enumerate(res.results):
        np.add.at(y, pos2tok[c], r["y"].astype(np.float32))
    y *= np.asarray(x_active_mask).astype(np.float32)[:, None]
    if _trace:
        kernel.last_results = res
    return y


# revision 43
# speedup vs baseline: 1.4340x; 1.0063x over previous
"""Trainium2 Bass kernel for nn_DecodeMoeOps (MoE decode: dispatch-quant,
grouped int8 GEMM1, SwiGLU, requant, grouped int8 GEMM2, weighted combine).

Expert-parallel across 8 NeuronCores: core c owns experts {2c, 2c+1}.

Key design (v2):
- Weights ship to SBUF as RAW INT8 (1 B/weight over HWDGE) and are upcast
  to bf16 on-chip, split across the DVE / ACT / GPSIMD engines. This halves
  the DMA-device byte volume vs casting during the DMA (which is charged at
  bf16 output bytes).
- Both GEMMs run WEIGHT-STATIONARY (weights are the PE's lhsT), so PE time
  scales with the number of routed tokens, not the weight volume. Each
  expert gets a fixed 64-position block of gathered tokens (host routing);
  tokens routed to both of a core's experts appear in both blocks and the
  host scatter-adds per-position outputs back to token rows.
- GEMM1 output lands channel-major [ch, tok]; SwiGLU/requant run in that
  layout (cross-partition absmax via gpsimd.partition_all_reduce), which
  makes the requantized activations directly usable as GEMM2's moving
  operand with no transposes. Final [h, pos] -> [pos, h] via PE transpose.
"""

import os
import sys

for _p in ("/opt/trn_rl_repo", "/root/.axon_site/_ro/trn_rl_repo"):
    if os.path.isdir(_p) and _p not in sys.path:
        sys.path.insert(0, _p)

from contextlib import ExitStack

import ml_dtypes
import numpy as np

import concourse.bass as bass
import concourse.bass_isa as bass_isa
import concourse.mybir as mybir
import concourse.tile as tile
from concourse import bacc
from concourse.bass_utils import run_bass_kernel_spmd
from concourse.masks import make_identity

B, TOPK, H, I, E = 128, 8, 2048, 1408, 16
NCORES = 8
EPC = E // NCORES  # experts per core
KH = H // 128  # 16 contraction tiles for GEMM1
KI = I // 128  # 11 contraction tiles for GEMM2
I2 = 2 * I
CT1 = I2 // 128  # 22 GEMM1 output-channel tiles (gate 0..10, up 11..21)
CT2 = H // 128  # 16 GEMM2 output-channel tiles
NT = 64  # token positions per expert block
NPOS = EPC * NT  # 128 positions per core
F32 = mybir.dt.float32
BF16 = mybir.dt.bfloat16
I8 = mybir.dt.int8
MAGIC = float(3 * 2**22)  # fp32 round-to-nearest-int magic (covers negatives)

# int8 -> bf16 upcast split points (free-dim columns) per engine:
# [0:V) on DVE, [V:A) on ACT, [A:end) on GPSIMD.  Tunables.
W1V, W1A = 1280, 2304  # of I2 = 2816
W2V, W2A = 224, 400  # of 512-wide w2 column chunks

_cache: dict = {}


def _build_program():
    nc = bacc.Bacc(
        "TRN2",
        target_bir_lowering=False,
        debug=False,
        num_devices=NCORES,
    )
    mult = mybir.AluOpType.mult
    opmax = mybir.AluOpType.max

    # --- per-core DRAM I/O ---
    xqT_d = nc.dram_tensor("xqT", [128, KH, NPOS], BF16, kind="ExternalInput").ap()
    sxr_d = nc.dram_tensor("sxrow", [1, NPOS], F32, kind="ExternalInput").ap()
    scr_d = nc.dram_tensor("scrow", [1, NPOS], F32, kind="ExternalInput").ap()
    w1_d = nc.dram_tensor("w1t", [EPC, 128, KH, I2], I8, kind="ExternalInput").ap()
    w2_d = nc.dram_tensor("w2t", [EPC, 128, KI, H], I8, kind="ExternalInput").ap()
    w2b1_d = nc.dram_tensor("w2bf1", [4, 128, KI, 512], BF16, kind="ExternalInput").ap()
    w1sc_d = nc.dram_tensor("w1sc", [EPC, 128, CT1], F32, kind="ExternalInput").ap()
    w2sc_d = nc.dram_tensor("w2sc", [EPC, 128, CT2], F32, kind="ExternalInput").ap()
    y_d = nc.dram_tensor("y", [NPOS, H], F32, kind="ExternalOutput").ap()

    with tile.TileContext(nc) as tc, ExitStack() as ctx:
        consts = ctx.enter_context(tc.tile_pool(name="consts", bufs=1))
        w1i8p = ctx.enter_context(tc.tile_pool(name="w1i8", bufs=3))
        w1bfp = ctx.enter_context(tc.tile_pool(name="w1bf", bufs=2))
        w2i8p = ctx.enter_context(tc.tile_pool(name="w2i8", bufs=2))
        w2bfp = ctx.enter_context(tc.tile_pool(name="w2bf", bufs=2))
        ep = ctx.enter_context(tc.tile_pool(name="ep", bufs=2))
        stats = ctx.enter_context(tc.tile_pool(name="stats", bufs=2))
        aqp = ctx.enter_context(tc.tile_pool(name="aqp", bufs=2))
        yp = ctx.enter_context(tc.tile_pool(name="yp", bufs=1))
        ps1p = ctx.enter_context(tc.tile_pool(name="ps1", bufs=2, space="PSUM"))
        ps2p = ctx.enter_context(tc.tile_pool(name="ps2", bufs=1, space="PSUM"))

        # --- prologue ---
        xqT_s = consts.tile([128, KH, NPOS], BF16, name="xqT_s")
        nc.scalar.dma_start(out=xqT_s[:], in_=xqT_d)
        ident = consts.tile([128, 128], F32, name="ident")
        make_identity(nc, ident[:])
        ones1 = consts.tile([1, 128], F32, name="ones1")
        nc.vector.memset(ones1[:], 1.0)
        sxr_s = consts.tile([1, NPOS], F32, name="sxr_s")
        scr_s = consts.tile([1, NPOS], F32, name="scr_s")
        w1sc_s, w2sc_s = [], []
        for e in range(EPC):
            w1sc_s.append(consts.tile([128, CT1], F32, name=f"w1sc_{e}"))
            w2sc_s.append(consts.tile([128, CT2], F32, name=f"w2sc_{e}"))

        def scale_dmas():
            for e in range(EPC):
                nc.sync.dma_start(out=w1sc_s[e][:], in_=w1sc_d[e])
                nc.sync.dma_start(out=w2sc_s[e][:], in_=w2sc_d[e])

        # broadcast of per-position rows happens in late_prologue (emitted
        # after the first weight chunks so small DMAs don't hog HWDGE early);
        # the psum buffer is reserved here to keep the ps1 tag rotation.
        psb0 = ps1p.tile([128, 24, NT], F32, tag="ps1", name="psb0")
        sxb = consts.tile([128, NPOS], F32, name="sxb")
        scb = consts.tile([128, NPOS], F32, name="scb")

        def late_prologue():
            nc.scalar.dma_start(out=sxr_s[:], in_=sxr_d)
            nc.scalar.dma_start(out=scr_s[:], in_=scr_d)
            bc0 = psb0[:, 0:2, :].rearrange("p a t -> p (a t)")
            nc.tensor.matmul(bc0, lhsT=ones1[:], rhs=sxr_s[:], start=True, stop=True)
            nc.vector.tensor_copy(out=sxb[:], in_=bc0)
            bc1 = psb0[:, 2:4, :].rearrange("p a t -> p (a t)")
            nc.tensor.matmul(bc1, lhsT=ones1[:], rhs=scr_s[:], start=True, stop=True)
            nc.vector.tensor_copy(out=scb[:], in_=bc1)

        # Emission order == per-engine execution order, so the phases below
        # software-pipeline the kernel: each expert's epilogue is emitted in
        # slices spliced between the NEXT phase's chunk pipelines, keeping
        # every engine queue free of long head-of-line dependency waits.
        st = {}  # per-expert tiles carried across phases
        aqs, s2cs = [], []

        def g1_chunk(e, kg):
            if kg == 0:
                # 3 exact PSUM banks; one accumulation group per bank (8
                # chunks share a bank: start on the bank's first chunk, stop
                # on its last -- HW zeroing is lazy per-write in the region).
                st[f"ps1_{e}"] = ps1p.tile([128, 24, NT], F32, tag="ps1", name=f"ps1_{e}")
            ps1 = st[f"ps1_{e}"]
            w1i = w1i8p.tile([128, 4, I2], I8, tag="w1i", name=f"w1i_{e}_{kg}")
            nc.sync.dma_start(out=w1i[:], in_=w1_d[e, :, kg * 4 : (kg + 1) * 4, :])
            w1b = w1bfp.tile([128, 4, I2], BF16, tag="w1b", name=f"w1b_{e}_{kg}")
            for j in (0, 2):
                nc.vector.tensor_copy(
                    out=w1b[:, j : j + 2, 0:W1V], in_=w1i[:, j : j + 2, 0:W1V]
                )
                nc.scalar.copy(
                    out=w1b[:, j : j + 2, W1V:W1A], in_=w1i[:, j : j + 2, W1V:W1A]
                )
                nc.gpsimd.tensor_copy(
                    out=w1b[:, j : j + 2, W1A:I2], in_=w1i[:, j : j + 2, W1A:I2]
                )
            for j in range(4):
                k = kg * 4 + j
                for c in range(CT1):
                    nc.tensor.matmul(
                        ps1[:, c, :],
                        lhsT=w1b[:, j, c * 128 : (c + 1) * 128],
                        rhs=xqT_s[:, k, e * NT : (e + 1) * NT],
                        start=(k == 0 and c % 8 == 0),
                        stop=(k == KH - 1 and (c % 8 == 7 or c == CT1 - 1)),
                    )

        def epi(e, s):
            """Epilogue slice s (1..4) for expert e."""
            if s == 1:
                # dequant: releases ps1
                ps1 = st[f"ps1_{e}"]
                sxb64 = sxb[:, e * NT : (e + 1) * NT]
                gate = st[f"gate_{e}"] = ep.tile(
                    [128, KI, NT], F32, tag="gate", name=f"gate_{e}"
                )
                up = st[f"up_{e}"] = ep.tile(
                    [128, KI, NT], F32, tag="up", name=f"up_{e}"
                )
                # gate = psum * w1sc[ch] * sx[tok]; up = psum * w1sc_up[ch]
                # (sx deferred into s2c: aq is invariant to per-token scale)
                w1g = w1sc_s[e][:, 0:KI].unsqueeze(2).broadcast_to([128, KI, NT])
                w1u = w1sc_s[e][:, KI:CT1].unsqueeze(2).broadcast_to([128, KI, NT])
                sx3 = sxb64.unsqueeze(1).broadcast_to([128, KI, NT])
                nc.vector.tensor_tensor(out=gate[:], in0=ps1[:, 0:KI, :], in1=w1g, op=mult)
                nc.vector.tensor_tensor(out=gate[:], in0=gate[:], in1=sx3, op=mult)
                nc.vector.tensor_tensor(out=up[:], in0=ps1[:, KI : 2 * KI, :], in1=w1u, op=mult)
            elif s == 2:
                gate, up = st[f"gate_{e}"], st[f"up_{e}"]
                sig = ep.tile([128, KI, NT], F32, tag="sig", name=f"sig_{e}")
                nc.scalar.activation(
                    out=sig[:], in_=gate[:], func=mybir.ActivationFunctionType.Sigmoid
                )
                gsig = ep.tile([128, KI, NT], F32, tag="gsig", name=f"gsig_{e}")
                nc.vector.tensor_tensor(out=gsig[:], in0=gate[:], in1=sig[:], op=mult)
                act = st[f"act_{e}"] = ep.tile(
                    [128, KI, NT], F32, tag="act", name=f"act_{e}"
                )
                nc.vector.tensor_tensor(out=act[:], in0=gsig[:], in1=up[:], op=mult)
            elif s == 3:
                act = st[f"act_{e}"]
                # per-token absmax over all I channels (partitions x 11 tiles)
                mall = st[f"mall_{e}"] = ep.tile(
                    [128, KI, NT], F32, tag="mall", name=f"mall_{e}"
                )
                nc.gpsimd.partition_all_reduce(
                    mall[:].rearrange("p j t -> p (j t)"),
                    act[:].rearrange("p j t -> p (j t)"),
                    128,
                    bass_isa.ReduceOp.absmax,
                )
                mfin = stats.tile([128, NT], F32, tag="mfin", name=f"mfin_{e}")
                nc.vector.reduce_max(
                    out=mfin[:].unsqueeze(2),
                    in_=mall[:].rearrange("p j t -> p t j"),
                    axis=mybir.AxisListType.X,
                )
                mc = stats.tile([128, NT], F32, tag="mc", name=f"mc_{e}")
                nc.vector.tensor_scalar_max(out=mc[:], in0=mfin[:], scalar1=1e-12)
                rr = stats.tile([128, NT], F32, tag="rr", name=f"rr_{e}")
                nc.vector.reciprocal(out=rr[:], in_=mc[:])
                r127 = st[f"r127_{e}"] = stats.tile(
                    [128, NT], F32, tag="r127", name=f"r127_{e}"
                )
                nc.vector.tensor_scalar_mul(out=r127[:], in0=rr[:], scalar1=127.0)
                # s2c = (mc/127) * sx[tok] * comb[tok]  (sx folded in scrow)
                s2c = stats.tile([128, NT], F32, tag="s2c", name=f"s2c_{e}")
                nc.vector.scalar_tensor_tensor(
                    out=s2c[:],
                    in0=mc[:],
                    scalar=1.0 / 127.0,
                    in1=scb[:, e * NT : (e + 1) * NT],
                    op0=mult,
                    op1=mult,
                )
                s2cs.append(s2c)
            else:
                act, r127 = st[f"act_{e}"], st[f"r127_{e}"]
                tq = ep.tile([128, KI, NT], F32, tag="gate", name=f"tq_{e}")
                nc.vector.tensor_tensor(
                    out=tq[:],
                    in0=act[:],
                    in1=r127[:].unsqueeze(1).broadcast_to([128, KI, NT]),
                    op=mult,
                )
                aq = aqp.tile([128, KI, NT], BF16, tag="aq", name=f"aq_{e}")
                nc.vector.tensor_scalar(
                    out=aq[:],
                    in0=tq[:],
                    scalar1=MAGIC,
                    scalar2=MAGIC,
                    op0=mybir.AluOpType.add,
                    op1=mybir.AluOpType.subtract,
                )
                aqs.append(aq)

        def w2dma(e, g):
            if e == EPC - 1:
                # expert-1 w2 ships pre-cast as bf16 over plain HWDGE: costs
                # 2 B/weight on the DMA device (idle in this window) but needs
                # zero engine-cast work during the saturated tail
                w2b = st[f"w2b_{e}_{g}"] = w2bfp.tile(
                    [128, KI, 512], BF16, tag="w2be1", name=f"w2b_{e}_{g}"
                )
                nc.sync.dma_start(out=w2b[:], in_=w2b1_d[g])
                return
            w2i = st[f"w2i_{e}_{g}"] = w2i8p.tile(
                [128, KI, 512], I8, tag="w2i", name=f"w2i_{e}_{g}"
            )
            nc.sync.dma_start(out=w2i[:], in_=w2_d[e, :, :, g * 512 : (g + 1) * 512])

        def w2cast(e, g):
            w2i = st[f"w2i_{e}_{g}"]
            w2b = st[f"w2b_{e}_{g}"] = w2bfp.tile(
                [128, KI, 512], BF16, tag="w2b", name=f"w2b_{e}_{g}"
            )
            nc.vector.tensor_copy(out=w2b[:, :, 0:W2V], in_=w2i[:, :, 0:W2V])
            nc.scalar.copy(out=w2b[:, :, W2V:W2A], in_=w2i[:, :, W2V:W2A])
            nc.gpsimd.tensor_copy(out=w2b[:, :, W2A:512], in_=w2i[:, :, W2A:512])

        def g2mm(e, g):
            w2b = st[f"w2b_{e}_{g}"]
            # one full bank per tag; single accumulation group per bank
            ps2 = ps2p.tile([128, 8, NT], F32, tag=f"ps2{e}", name=f"ps2_{e}_{g}")
            for cc in range(4):
                for k in range(KI):
                    nc.tensor.matmul(
                        ps2[:, cc, :],
                        lhsT=w2b[:, k, cc * 128 : (cc + 1) * 128],
                        rhs=aqs[e][:, k, :],
                        start=(k == 0 and cc == 0),
                        stop=(k == KI - 1 and cc == 3),
                    )
            # deq2: o * w2sc[h] * (s2 * comb)[tok] -> yT columns
            w2s3 = (
                w2sc_s[e][:, g * 4 : (g + 1) * 4]
                .unsqueeze(2)
                .broadcast_to([128, 4, NT])
            )
            s2c3 = s2cs[e][:].unsqueeze(1).broadcast_to([128, 4, NT])
            dtmp = ep.tile([128, 4, NT], F32, tag="dtmp", name=f"dtmp_{e}_{g}")
            nc.vector.tensor_tensor(out=dtmp[:], in0=ps2[:, 0:4, :], in1=w2s3, op=mult)
            nc.vector.tensor_tensor(
                out=yT[:, g * 4 : (g + 1) * 4, e * NT : (e + 1) * NT],
                in0=dtmp[:],
                in1=s2c3,
                op=mult,
            )

        def ytail(g):
            # both experts done for these channel tiles: emit y rows
            if "pst" not in st:
                st["pst"] = ps1p.tile([128, 24, NT], F32, tag="ps1", name="pst")
            pst = st["pst"]
            views = [
                pst[:, 2 * cc : 2 * cc + 2, :].rearrange("p a t -> p (a t)")
                for cc in range(4)
            ]
            for cc in range(4):
                c = g * 4 + cc
                nc.tensor.transpose(views[cc], yT[:, c, :], ident[:])
            nc.scalar.copy(
                out=ysb[:, g * 512 : (g + 1) * 512],
                in_=pst[:, 0:8, :].rearrange("p a t -> p (a t)"),
            )
            nc.sync.dma_start(
                out=y_d[:, g * 512 : (g + 1) * 512],
                in_=ysb[:, g * 512 : (g + 1) * 512],
            )

        yT = yp.tile([128, CT2, NPOS], F32, name="yT")
        ysb = yp.tile([128, H], F32, name="ysb")

        g1_chunk(0, 0)
        g1_chunk(1, 0)
        scale_dmas()
        late_prologue()
        for kg in range(1, 4):
            g1_chunk(0, kg)
            g1_chunk(1, kg)
        for s in range(1, 5):
            epi(0, s)
            epi(1, s)
        for e in range(EPC):
            for g in range(4):
                w2dma(e, g)
        for g in range(4):
            w2cast(0, g)
            g2mm(0, g)
            g2mm(1, g)
            ytail(g)

    nc.compile()
    return nc


def get_program():
    if "nc" not in _cache:
        _cache["nc"] = _build_program()
    return _cache["nc"]


def _prep_inputs(x, expert_ids, smooth_scales, expert_scales, w1, w1_scale, w2, w2_scale):
    """Host-side dispatch: quantize x, route tokens, shard experts."""
    x = np.asarray(x, np.float32)
    expert_ids = np.asarray(expert_ids)
    smooth_scales = np.asarray(smooth_scales, np.float32)
    expert_scales = np.asarray(expert_scales, np.float32)
    w1_scale = np.asarray(w1_scale, np.float32)
    w2_scale = np.asarray(w2_scale, np.float32)

    # dynamic per-token int8 quantization (exact mirror of reference ops)
    sx = np.maximum(np.max(np.abs(x), axis=-1, keepdims=True), 1e-12) / 127.0
    xq = np.round(np.clip(x / sx, -128.0, 127.0)).astype(np.float32)

    # combine matrix [B, E]: scatter-add expert_scales at expert_ids
    comb = np.zeros((B, E), np.float32)
    np.add.at(comb, (np.arange(B)[:, None], expert_ids), expert_scales)

    w1v = w1.astype(np.int8)
    w2v = w2.astype(np.int8)

    in_maps = []
    pos2tok = np.zeros((NCORES, NPOS), np.int64)
    for c in range(NCORES):
        valid = np.zeros(NPOS, bool)
        ge_of_pos = np.zeros(NPOS, np.int64)
        for le in range(EPC):
            ge = EPC * c + le
            toks = np.nonzero((expert_ids == ge).any(axis=1))[0]
            assert len(toks) <= NT, f"expert {ge}: {len(toks)} tokens > capacity {NT}"
            pos2tok[c, le * NT : le * NT + len(toks)] = toks
            valid[le * NT : le * NT + len(toks)] = True
            ge_of_pos[le * NT : (le + 1) * NT] = ge

        toks_c = pos2tok[c]
        xg = xq[toks_c]  # [NPOS, H]
        xqT = np.ascontiguousarray(
            xg.T.reshape(KH, 128, NPOS).transpose(1, 0, 2)
        ).astype(ml_dtypes.bfloat16)
        sxrow = sx[toks_c, 0][None, :].astype(np.float32)
        combp = comb[toks_c, ge_of_pos] * valid  # zero at padding positions
        scrow = (sx[toks_c, 0] * combp)[None, :].astype(np.float32)

        es = list(range(EPC * c, EPC * (c + 1)))
        w1t = np.ascontiguousarray(
            w1v[es].reshape(EPC, KH, 128, I2).transpose(0, 2, 1, 3)
        )
        w2t = np.ascontiguousarray(
            w2v[es].reshape(EPC, KI, 128, H).transpose(0, 2, 1, 3)
        )
        w2bf1 = np.ascontiguousarray(
            w2t[EPC - 1].reshape(128, KI, 4, 512).transpose(2, 0, 1, 3)
        ).astype(ml_dtypes.bfloat16)
        w1sc = np.empty((EPC, 128, CT1), np.float32)
        w2sc = np.empty((EPC, 128, CT2), np.float32)
        for le, ge in enumerate(es):
            gatesc = w1_scale[ge, :I].reshape(KI, 128).T
            upsc = (w1_scale[ge, I:] * smooth_scales[ge]).reshape(KI, 128).T
            w1sc[le] = np.concatenate([gatesc, upsc], axis=1)
            w2sc[le] = w2_scale[ge].reshape(CT2, 128).T

        in_maps.append(
            {
                "xqT": xqT,
                "sxrow": sxrow,
                "scrow": scrow,
                "w1t": w1t,
                "w2t": w2t,
                "w2bf1": w2bf1,
                "w1sc": w1sc,
                "w2sc": w2sc,
            }
        )
    return in_maps, pos2tok


def kernel(
    x,
    expert_ids,
    smooth_scales,
    expert_scales,
    x_active_mask,
    w1,
    w1_scale,
    w2,
    w2_scale,
    _trace=False,
    _trace_kwargs=None,
):
    in_maps, pos2tok = _prep_inputs(
        x, expert_ids, smooth_scales, expert_scales, w1, w1_scale, w2, w2_scale
    )
    nc = get_program()
    res = run_bass_kernel_spmd(
        nc,
        in_maps,
        core_ids=list(range(NCORES)),
        trace=_trace,
        **(_trace_kwargs or {}),
    )
    y = np.zeros((B, H), np.float32)
    for c, r in enumerate(res.results):
        np.add.at(y, pos2tok[c], r["y"].astype(np.float32))
    y *= np.asarray(x_active_mask).astype(np.float32)[:, None]
    if _trace:
        kernel.last_results = res
    return y


# revision 46
# speedup vs baseline: 1.4486x; 1.0102x over previous
"""Trainium2 Bass kernel for nn_DecodeMoeOps (MoE decode: dispatch-quant,
grouped int8 GEMM1, SwiGLU, requant, grouped int8 GEMM2, weighted combine).

Expert-parallel across 8 NeuronCores: core c owns experts {2c, 2c+1}.

Key design (v2):
- Weights ship to SBUF as RAW INT8 (1 B/weight over HWDGE) and are upcast
  to bf16 on-chip, split across the DVE / ACT / GPSIMD engines. This halves
  the DMA-device byte volume vs casting during the DMA (which is charged at
  bf16 output bytes).
- Both GEMMs run WEIGHT-STATIONARY (weights are the PE's lhsT), so PE time
  scales with the number of routed tokens, not the weight volume. Each
  expert gets a fixed 64-position block of gathered tokens (host routing);
  tokens routed to both of a core's experts appear in both blocks and the
  host scatter-adds per-position outputs back to token rows.
- GEMM1 output lands channel-major [ch, tok]; SwiGLU/requant run in that
  layout (cross-partition absmax via gpsimd.partition_all_reduce), which
  makes the requantized activations directly usable as GEMM2's moving
  operand with no transposes. Final [h, pos] -> [pos, h] via PE transpose.
"""

import os
import sys

for _p in ("/opt/trn_rl_repo", "/root/.axon_site/_ro/trn_rl_repo"):
    if os.path.isdir(_p) and _p not in sys.path:
        sys.path.insert(0, _p)

from contextlib import ExitStack

import ml_dtypes
import numpy as np

import concourse.bass as bass
import concourse.bass_isa as bass_isa
import concourse.mybir as mybir
import concourse.tile as tile
from concourse import bacc
from concourse.bass_utils import run_bass_kernel_spmd
from concourse.masks import make_identity

B, TOPK, H, I, E = 128, 8, 2048, 1408, 16
NCORES = 8
EPC = E // NCORES  # experts per core
KH = H // 128  # 16 contraction tiles for GEMM1
KI = I // 128  # 11 contraction tiles for GEMM2
I2 = 2 * I
CT1 = I2 // 128  # 22 GEMM1 output-channel tiles (gate 0..10, up 11..21)
CT2 = H // 128  # 16 GEMM2 output-channel tiles
NT = 64  # token positions per expert block
NPOS = EPC * NT  # 128 positions per core
F32 = mybir.dt.float32
BF16 = mybir.dt.bfloat16
I8 = mybir.dt.int8
MAGIC = float(3 * 2**22)  # fp32 round-to-nearest-int magic (covers negatives)

# int8 -> bf16 upcast split points (free-dim columns) per engine:
# [0:V) on DVE, [V:A) on ACT, [A:end) on GPSIMD.  Tunables.
W1V, W1A = 1280, 2304  # of I2 = 2816
W2V, W2A = 224, 400  # of 512-wide w2 column chunks

_cache: dict = {}


def _build_program():
    nc = bacc.Bacc(
        "TRN2",
        target_bir_lowering=False,
        debug=False,
        num_devices=NCORES,
    )
    mult = mybir.AluOpType.mult
    opmax = mybir.AluOpType.max

    # --- per-core DRAM I/O ---
    xqT_d = nc.dram_tensor("xqT", [128, KH, NPOS], BF16, kind="ExternalInput").ap()
    sxr_d = nc.dram_tensor("sxrow", [1, NPOS], F32, kind="ExternalInput").ap()
    scr_d = nc.dram_tensor("scrow", [1, NPOS], F32, kind="ExternalInput").ap()
    w1_d = nc.dram_tensor("w1t", [EPC, 128, KH, I2], I8, kind="ExternalInput").ap()
    w2_d = nc.dram_tensor("w2t", [EPC, 128, KI, H], I8, kind="ExternalInput").ap()
    w2b1_d = nc.dram_tensor("w2bf1", [8, 128, KI, 256], BF16, kind="ExternalInput").ap()
    w1sc_d = nc.dram_tensor("w1sc", [EPC, 128, CT1], F32, kind="ExternalInput").ap()
    w2sc_d = nc.dram_tensor("w2sc", [EPC, 128, CT2], F32, kind="ExternalInput").ap()
    y_d = nc.dram_tensor("y", [NPOS, H], F32, kind="ExternalOutput").ap()

    with tile.TileContext(nc) as tc, ExitStack() as ctx:
        consts = ctx.enter_context(tc.tile_pool(name="consts", bufs=1))
        w1i8p = ctx.enter_context(tc.tile_pool(name="w1i8", bufs=3))
        w1bfp = ctx.enter_context(tc.tile_pool(name="w1bf", bufs=2))
        w2i8p = ctx.enter_context(tc.tile_pool(name="w2i8", bufs=2))
        w2bfp = ctx.enter_context(tc.tile_pool(name="w2bf", bufs=2))
        ep = ctx.enter_context(tc.tile_pool(name="ep", bufs=2))
        stats = ctx.enter_context(tc.tile_pool(name="stats", bufs=2))
        aqp = ctx.enter_context(tc.tile_pool(name="aqp", bufs=2))
        yp = ctx.enter_context(tc.tile_pool(name="yp", bufs=1))
        ps1p = ctx.enter_context(tc.tile_pool(name="ps1", bufs=2, space="PSUM"))
        ps2p = ctx.enter_context(tc.tile_pool(name="ps2", bufs=1, space="PSUM"))

        # --- prologue ---
        xqT_s = consts.tile([128, KH, NPOS], BF16, name="xqT_s")
        nc.scalar.dma_start(out=xqT_s[:], in_=xqT_d)
        ident = consts.tile([128, 128], F32, name="ident")
        make_identity(nc, ident[:])
        ones1 = consts.tile([1, 128], F32, name="ones1")
        nc.vector.memset(ones1[:], 1.0)
        sxr_s = consts.tile([1, NPOS], F32, name="sxr_s")
        scr_s = consts.tile([1, NPOS], F32, name="scr_s")
        w1sc_s, w2sc_s = [], []
        for e in range(EPC):
            w1sc_s.append(consts.tile([128, CT1], F32, name=f"w1sc_{e}"))
            w2sc_s.append(consts.tile([128, CT2], F32, name=f"w2sc_{e}"))

        def scale_dmas():
            for e in range(EPC):
                nc.sync.dma_start(out=w1sc_s[e][:], in_=w1sc_d[e])
                nc.sync.dma_start(out=w2sc_s[e][:], in_=w2sc_d[e])

        # broadcast of per-position rows happens in late_prologue (emitted
        # after the first weight chunks so small DMAs don't hog HWDGE early);
        # the psum buffer is reserved here to keep the ps1 tag rotation.
        psb0 = ps1p.tile([128, 24, NT], F32, tag="ps1", name="psb0")
        sxb = consts.tile([128, NPOS], F32, name="sxb")
        scb = consts.tile([128, NPOS], F32, name="scb")

        def late_prologue():
            nc.scalar.dma_start(out=sxr_s[:], in_=sxr_d)
            nc.scalar.dma_start(out=scr_s[:], in_=scr_d)
            bc0 = psb0[:, 0:2, :].rearrange("p a t -> p (a t)")
            nc.tensor.matmul(bc0, lhsT=ones1[:], rhs=sxr_s[:], start=True, stop=True)
            nc.vector.tensor_copy(out=sxb[:], in_=bc0)
            bc1 = psb0[:, 2:4, :].rearrange("p a t -> p (a t)")
            nc.tensor.matmul(bc1, lhsT=ones1[:], rhs=scr_s[:], start=True, stop=True)
            nc.vector.tensor_copy(out=scb[:], in_=bc1)

        # Emission order == per-engine execution order, so the phases below
        # software-pipeline the kernel: each expert's epilogue is emitted in
        # slices spliced between the NEXT phase's chunk pipelines, keeping
        # every engine queue free of long head-of-line dependency waits.
        st = {}  # per-expert tiles carried across phases
        aqs, s2cs = [], []

        def g1_chunk(e, kg):
            if kg == 0:
                # 3 exact PSUM banks; one accumulation group per bank (8
                # chunks share a bank: start on the bank's first chunk, stop
                # on its last -- HW zeroing is lazy per-write in the region).
                st[f"ps1_{e}"] = ps1p.tile([128, 24, NT], F32, tag="ps1", name=f"ps1_{e}")
            ps1 = st[f"ps1_{e}"]
            w1i = w1i8p.tile([128, 4, I2], I8, tag="w1i", name=f"w1i_{e}_{kg}")
            nc.sync.dma_start(out=w1i[:], in_=w1_d[e, :, kg * 4 : (kg + 1) * 4, :])
            w1b = w1bfp.tile([128, 4, I2], BF16, tag="w1b", name=f"w1b_{e}_{kg}")
            for j in (0, 2):
                nc.vector.tensor_copy(
                    out=w1b[:, j : j + 2, 0:W1V], in_=w1i[:, j : j + 2, 0:W1V]
                )
                nc.scalar.copy(
                    out=w1b[:, j : j + 2, W1V:W1A], in_=w1i[:, j : j + 2, W1V:W1A]
                )
                nc.gpsimd.tensor_copy(
                    out=w1b[:, j : j + 2, W1A:I2], in_=w1i[:, j : j + 2, W1A:I2]
                )
            for j in range(4):
                k = kg * 4 + j
                for c in range(CT1):
                    nc.tensor.matmul(
                        ps1[:, c, :],
                        lhsT=w1b[:, j, c * 128 : (c + 1) * 128],
                        rhs=xqT_s[:, k, e * NT : (e + 1) * NT],
                        start=(k == 0 and c % 8 == 0),
                        stop=(k == KH - 1 and (c % 8 == 7 or c == CT1 - 1)),
                    )

        def epi(e, s):
            """Epilogue slice s (1..4) for expert e."""
            if s == 1:
                # dequant: releases ps1
                ps1 = st[f"ps1_{e}"]
                sxb64 = sxb[:, e * NT : (e + 1) * NT]
                gate = st[f"gate_{e}"] = ep.tile(
                    [128, KI, NT], F32, tag="gate", name=f"gate_{e}"
                )
                up = st[f"up_{e}"] = ep.tile(
                    [128, KI, NT], F32, tag="up", name=f"up_{e}"
                )
                # gate = psum * w1sc[ch] * sx[tok]; up = psum * w1sc_up[ch]
                # (sx deferred into s2c: aq is invariant to per-token scale)
                w1g = w1sc_s[e][:, 0:KI].unsqueeze(2).broadcast_to([128, KI, NT])
                w1u = w1sc_s[e][:, KI:CT1].unsqueeze(2).broadcast_to([128, KI, NT])
                sx3 = sxb64.unsqueeze(1).broadcast_to([128, KI, NT])
                nc.vector.tensor_tensor(out=gate[:], in0=ps1[:, 0:KI, :], in1=w1g, op=mult)
                nc.vector.tensor_tensor(out=gate[:], in0=gate[:], in1=sx3, op=mult)
                nc.vector.tensor_tensor(out=up[:], in0=ps1[:, KI : 2 * KI, :], in1=w1u, op=mult)
            elif s == 2:
                gate, up = st[f"gate_{e}"], st[f"up_{e}"]
                sig = ep.tile([128, KI, NT], F32, tag="sig", name=f"sig_{e}")
                nc.scalar.activation(
                    out=sig[:], in_=gate[:], func=mybir.ActivationFunctionType.Sigmoid
                )
                gsig = ep.tile([128, KI, NT], F32, tag="gsig", name=f"gsig_{e}")
                nc.vector.tensor_tensor(out=gsig[:], in0=gate[:], in1=sig[:], op=mult)
                act = st[f"act_{e}"] = ep.tile(
                    [128, KI, NT], F32, tag="act", name=f"act_{e}"
                )
                nc.vector.tensor_tensor(out=act[:], in0=gsig[:], in1=up[:], op=mult)
            elif s == 3:
                act = st[f"act_{e}"]
                # per-token absmax over all I channels (partitions x 11 tiles)
                mall = st[f"mall_{e}"] = ep.tile(
                    [128, KI, NT], F32, tag="mall", name=f"mall_{e}"
                )
                nc.gpsimd.partition_all_reduce(
                    mall[:].rearrange("p j t -> p (j t)"),
                    act[:].rearrange("p j t -> p (j t)"),
                    128,
                    bass_isa.ReduceOp.absmax,
                )
                mfin = stats.tile([128, NT], F32, tag="mfin", name=f"mfin_{e}")
                nc.vector.reduce_max(
                    out=mfin[:].unsqueeze(2),
                    in_=mall[:].rearrange("p j t -> p t j"),
                    axis=mybir.AxisListType.X,
                )
                mc = stats.tile([128, NT], F32, tag="mc", name=f"mc_{e}")
                nc.vector.tensor_scalar_max(out=mc[:], in0=mfin[:], scalar1=1e-12)
                rr = stats.tile([128, NT], F32, tag="rr", name=f"rr_{e}")
                nc.vector.reciprocal(out=rr[:], in_=mc[:])
                r127 = st[f"r127_{e}"] = stats.tile(
                    [128, NT], F32, tag="r127", name=f"r127_{e}"
                )
                nc.vector.tensor_scalar_mul(out=r127[:], in0=rr[:], scalar1=127.0)
                # s2c = (mc/127) * sx[tok] * comb[tok]  (sx folded in scrow)
                s2c = stats.tile([128, NT], F32, tag="s2c", name=f"s2c_{e}")
                nc.vector.scalar_tensor_tensor(
                    out=s2c[:],
                    in0=mc[:],
                    scalar=1.0 / 127.0,
                    in1=scb[:, e * NT : (e + 1) * NT],
                    op0=mult,
                    op1=mult,
                )
                s2cs.append(s2c)
            else:
                act, r127 = st[f"act_{e}"], st[f"r127_{e}"]
                tq = ep.tile([128, KI, NT], F32, tag="gate", name=f"tq_{e}")
                nc.vector.tensor_tensor(
                    out=tq[:],
                    in0=act[:],
                    in1=r127[:].unsqueeze(1).broadcast_to([128, KI, NT]),
                    op=mult,
                )
                aq = aqp.tile([128, KI, NT], BF16, tag="aq", name=f"aq_{e}")
                nc.vector.tensor_scalar(
                    out=aq[:],
                    in0=tq[:],
                    scalar1=MAGIC,
                    scalar2=MAGIC,
                    op0=mybir.AluOpType.add,
                    op1=mybir.AluOpType.subtract,
                )
                aqs.append(aq)

        def w2dma(e, g):
            if e == EPC - 1:
                # expert-1 w2 ships pre-cast as bf16 over plain HWDGE: costs
                # 2 B/weight on the DMA device (idle in this window) but needs
                # zero engine-cast work during the saturated tail
                w2b = st[f"w2b_{e}_{g}"] = w2bfp.tile(
                    [128, KI, 512], BF16, tag="w2be1", name=f"w2b_{e}_{g}"
                )
                nc.sync.dma_start(out=w2b[:, :, 0:256], in_=w2b1_d[2 * g])
                nc.sync.dma_start(out=w2b[:, :, 256:512], in_=w2b1_d[2 * g + 1])
                return
            w2i = st[f"w2i_{e}_{g}"] = w2i8p.tile(
                [128, KI, 512], I8, tag="w2i", name=f"w2i_{e}_{g}"
            )
            nc.sync.dma_start(out=w2i[:], in_=w2_d[e, :, :, g * 512 : (g + 1) * 512])

        def w2cast(e, g):
            w2i = st[f"w2i_{e}_{g}"]
            w2b = st[f"w2b_{e}_{g}"] = w2bfp.tile(
                [128, KI, 512], BF16, tag="w2b", name=f"w2b_{e}_{g}"
            )
            nc.vector.tensor_copy(out=w2b[:, :, 0:W2V], in_=w2i[:, :, 0:W2V])
            nc.scalar.copy(out=w2b[:, :, W2V:W2A], in_=w2i[:, :, W2V:W2A])
            nc.gpsimd.tensor_copy(out=w2b[:, :, W2A:512], in_=w2i[:, :, W2A:512])

        def g2mm(e, g):
            w2b = st[f"w2b_{e}_{g}"]
            # one full bank per tag; single accumulation group per bank
            ps2 = ps2p.tile([128, 8, NT], F32, tag=f"ps2{e}", name=f"ps2_{e}_{g}")
            for cc in range(4):
                for k in range(KI):
                    nc.tensor.matmul(
                        ps2[:, cc, :],
                        lhsT=w2b[:, k, cc * 128 : (cc + 1) * 128],
                        rhs=aqs[e][:, k, :],
                        start=(k == 0 and cc == 0),
                        stop=(k == KI - 1 and cc == 3),
                    )
            # deq2: o * w2sc[h] * (s2 * comb)[tok] -> yT columns
            w2s3 = (
                w2sc_s[e][:, g * 4 : (g + 1) * 4]
                .unsqueeze(2)
                .broadcast_to([128, 4, NT])
            )
            s2c3 = s2cs[e][:].unsqueeze(1).broadcast_to([128, 4, NT])
            dtmp = ep.tile([128, 4, NT], F32, tag="dtmp", name=f"dtmp_{e}_{g}")
            nc.vector.tensor_tensor(out=dtmp[:], in0=ps2[:, 0:4, :], in1=w2s3, op=mult)
            nc.vector.tensor_tensor(
                out=yT[:, g * 4 : (g + 1) * 4, e * NT : (e + 1) * NT],
                in0=dtmp[:],
                in1=s2c3,
                op=mult,
            )

        def ytail(g):
            # both experts done for these channel tiles: emit y rows
            if "pst" not in st:
                st["pst"] = ps1p.tile([128, 24, NT], F32, tag="ps1", name="pst")
            pst = st["pst"]
            views = [
                pst[:, 2 * cc : 2 * cc + 2, :].rearrange("p a t -> p (a t)")
                for cc in range(4)
            ]
            for cc in range(4):
                c = g * 4 + cc
                nc.tensor.transpose(views[cc], yT[:, c, :], ident[:])
            nc.scalar.copy(
                out=ysb[:, g * 512 : (g + 1) * 512],
                in_=pst[:, 0:8, :].rearrange("p a t -> p (a t)"),
            )
            nc.sync.dma_start(
                out=y_d[:, g * 512 : (g + 1) * 512],
                in_=ysb[:, g * 512 : (g + 1) * 512],
            )

        yT = yp.tile([128, CT2, NPOS], F32, name="yT")
        ysb = yp.tile([128, H], F32, name="ysb")

        g1_chunk(0, 0)
        g1_chunk(1, 0)
        scale_dmas()
        late_prologue()
        for kg in range(1, 4):
            g1_chunk(0, kg)
            g1_chunk(1, kg)
        for s in range(1, 5):
            epi(0, s)
            epi(1, s)
        for e in range(EPC):
            for g in range(4):
                w2dma(e, g)
        for g in range(4):
            w2cast(0, g)
            g2mm(0, g)
            g2mm(1, g)
            ytail(g)

    nc.compile()
    return nc


def get_program():
    if "nc" not in _cache:
        _cache["nc"] = _build_program()
    return _cache["nc"]


def _prep_inputs(x, expert_ids, smooth_scales, expert_scales, w1, w1_scale, w2, w2_scale):
    """Host-side dispatch: quantize x, route tokens, shard experts."""
    x = np.asarray(x, np.float32)
    expert_ids = np.asarray(expert_ids)
    smooth_scales = np.asarray(smooth_scales, np.float32)
    expert_scales = np.asarray(expert_scales, np.float32)
    w1_scale = np.asarray(w1_scale, np.float32)
    w2_scale = np.asarray(w2_scale, np.float32)

    # dynamic per-token int8 quantization (exact mirror of reference ops)
    sx = np.maximum(np.max(np.abs(x), axis=-1, keepdims=True), 1e-12) / 127.0
    xq = np.round(np.clip(x / sx, -128.0, 127.0)).astype(np.float32)

    # combine matrix [B, E]: scatter-add expert_scales at expert_ids
    comb = np.zeros((B, E), np.float32)
    np.add.at(comb, (np.arange(B)[:, None], expert_ids), expert_scales)

    w1v = w1.astype(np.int8)
    w2v = w2.astype(np.int8)

    in_maps = []
    pos2tok = np.zeros((NCORES, NPOS), np.int64)
    for c in range(NCORES):
        valid = np.zeros(NPOS, bool)
        ge_of_pos = np.zeros(NPOS, np.int64)
        for le in range(EPC):
            ge = EPC * c + le
            toks = np.nonzero((expert_ids == ge).any(axis=1))[0]
            assert len(toks) <= NT, f"expert {ge}: {len(toks)} tokens > capacity {NT}"
            pos2tok[c, le * NT : le * NT + len(toks)] = toks
            valid[le * NT : le * NT + len(toks)] = True
            ge_of_pos[le * NT : (le + 1) * NT] = ge

        toks_c = pos2tok[c]
        xg = xq[toks_c]  # [NPOS, H]
        xqT = np.ascontiguousarray(
            xg.T.reshape(KH, 128, NPOS).transpose(1, 0, 2)
        ).astype(ml_dtypes.bfloat16)
        sxrow = sx[toks_c, 0][None, :].astype(np.float32)
        combp = comb[toks_c, ge_of_pos] * valid  # zero at padding positions
        scrow = (sx[toks_c, 0] * combp)[None, :].astype(np.float32)

        es = list(range(EPC * c, EPC * (c + 1)))
        w1t = np.ascontiguousarray(
            w1v[es].reshape(EPC, KH, 128, I2).transpose(0, 2, 1, 3)
        )
        w2t = np.ascontiguousarray(
            w2v[es].reshape(EPC, KI, 128, H).transpose(0, 2, 1, 3)
        )
        w2bf1 = np.ascontiguousarray(
            w2t[EPC - 1].reshape(128, KI, 8, 256).transpose(2, 0, 1, 3)
        ).astype(ml_dtypes.bfloat16)
        w1sc = np.empty((EPC, 128, CT1), np.float32)
        w2sc = np.empty((EPC, 128, CT2), np.float32)
        for le, ge in enumerate(es):
            gatesc = w1_scale[ge, :I].reshape(KI, 128).T
            upsc = (w1_scale[ge, I:] * smooth_scales[ge]).reshape(KI, 128).T
            w1sc[le] = np.concatenate([gatesc, upsc], axis=1)
            w2sc[le] = w2_scale[ge].reshape(CT2, 128).T

        in_maps.append(
            {
                "xqT": xqT,
                "sxrow": sxrow,
                "scrow": scrow,
                "w1t": w1t,
                "w2t": w2t,
                "w2bf1": w2bf1,
                "w1sc": w1sc,
                "w2sc": w2sc,
            }
        )
    return in_maps, pos2tok


def kernel(
    x,
    expert_ids,
    smooth_scales,
    expert_scales,
    x_active_mask,
    w1,
    w1_scale,
    w2,
    w2_scale,
    _trace=False,
    _trace_kwargs=None,
):
    in_maps, pos2tok = _prep_inputs(
        x, expert_ids, smooth_scales, expert_scales, w1, w1_scale, w2, w2_scale
    )
    nc = get_program()
    res = run_bass_kernel_spmd(
        nc,
        in_maps,
        core_ids=list(range(NCORES)),
        trace=_trace,
        **(_trace_kwargs or {}),
    )
    y = np.zeros((B, H), np.float32)
    for c, r in enumerate(res.results):
        np.add.at(y, pos2tok[c], r["y"].astype(np.float32))
    y *= np.asarray(x_active_mask).astype(np.float32)[:, None]
    if _trace:
        kernel.last_results = res
    return y


# revision 47
# speedup vs baseline: 1.4964x; 1.0330x over previous
"""Trainium2 Bass kernel for nn_DecodeMoeOps (MoE decode: dispatch-quant,
grouped int8 GEMM1, SwiGLU, requant, grouped int8 GEMM2, weighted combine).

Expert-parallel across 8 NeuronCores: core c owns experts {2c, 2c+1}.

Key design (v2):
- Weights ship to SBUF as RAW INT8 (1 B/weight over HWDGE) and are upcast
  to bf16 on-chip, split across the DVE / ACT / GPSIMD engines. This halves
  the DMA-device byte volume vs casting during the DMA (which is charged at
  bf16 output bytes).
- Both GEMMs run WEIGHT-STATIONARY (weights are the PE's lhsT), so PE time
  scales with the number of routed tokens, not the weight volume. Each
  expert gets a fixed 64-position block of gathered tokens (host routing);
  tokens routed to both of a core's experts appear in both blocks and the
  host scatter-adds per-position outputs back to token rows.
- GEMM1 output lands channel-major [ch, tok]; SwiGLU/requant run in that
  layout (cross-partition absmax via gpsimd.partition_all_reduce), which
  makes the requantized activations directly usable as GEMM2's moving
  operand with no transposes. Final [h, pos] -> [pos, h] via PE transpose.
"""

import os
import sys

for _p in ("/opt/trn_rl_repo", "/root/.axon_site/_ro/trn_rl_repo"):
    if os.path.isdir(_p) and _p not in sys.path:
        sys.path.insert(0, _p)

from contextlib import ExitStack

import ml_dtypes
import numpy as np

import concourse.bass as bass
import concourse.bass_isa as bass_isa
import concourse.mybir as mybir
import concourse.tile as tile
from concourse import bacc
from concourse.bass_utils import run_bass_kernel_spmd
from concourse.masks import make_identity

B, TOPK, H, I, E = 128, 8, 2048, 1408, 16
NCORES = 8
EPC = E // NCORES  # experts per core
KH = H // 128  # 16 contraction tiles for GEMM1
KI = I // 128  # 11 contraction tiles for GEMM2
I2 = 2 * I
CT1 = I2 // 128  # 22 GEMM1 output-channel tiles (gate 0..10, up 11..21)
CT2 = H // 128  # 16 GEMM2 output-channel tiles
NT = 64  # token positions per expert block
NPOS = EPC * NT  # 128 positions per core
F32 = mybir.dt.float32
BF16 = mybir.dt.bfloat16
I8 = mybir.dt.int8
MAGIC = float(3 * 2**22)  # fp32 round-to-nearest-int magic (covers negatives)

# int8 -> bf16 upcast split points (free-dim columns) per engine:
# [0:V) on DVE, [V:A) on ACT, [A:end) on GPSIMD.  Tunables.
W1V, W1A = 1280, 2304  # of I2 = 2816
W2V, W2A = 224, 400  # of 512-wide w2 column chunks

_cache: dict = {}


def _build_program():
    nc = bacc.Bacc(
        "TRN2",
        target_bir_lowering=False,
        debug=False,
        num_devices=NCORES,
    )
    mult = mybir.AluOpType.mult
    opmax = mybir.AluOpType.max

    # --- per-core DRAM I/O ---
    xqT_d = nc.dram_tensor("xqT", [128, KH, NPOS], BF16, kind="ExternalInput").ap()
    sxr_d = nc.dram_tensor("sxrow", [1, NPOS], F32, kind="ExternalInput").ap()
    scr_d = nc.dram_tensor("scrow", [1, NPOS], F32, kind="ExternalInput").ap()
    w1_d = nc.dram_tensor("w1t", [EPC, 128, KH, I2], I8, kind="ExternalInput").ap()
    w2_d = nc.dram_tensor("w2t", [EPC, 128, KI, H], I8, kind="ExternalInput").ap()
    w2b1_d = nc.dram_tensor("w2bf1", [8, 128, KI, 256], BF16, kind="ExternalInput").ap()
    w1sc_d = nc.dram_tensor("w1sc", [EPC, 128, CT1], F32, kind="ExternalInput").ap()
    w2sc_d = nc.dram_tensor("w2sc", [EPC, 128, CT2], F32, kind="ExternalInput").ap()
    y_d = nc.dram_tensor("y", [NPOS, H], F32, kind="ExternalOutput").ap()

    with tile.TileContext(nc) as tc, ExitStack() as ctx:
        consts = ctx.enter_context(tc.tile_pool(name="consts", bufs=1))
        w1i8p = ctx.enter_context(tc.tile_pool(name="w1i8", bufs=3))
        w1bfp = ctx.enter_context(tc.tile_pool(name="w1bf", bufs=2))
        w2i8p = ctx.enter_context(tc.tile_pool(name="w2i8", bufs=2))
        w2bfp = ctx.enter_context(tc.tile_pool(name="w2bf", bufs=2))
        ep = ctx.enter_context(tc.tile_pool(name="ep", bufs=2))
        stats = ctx.enter_context(tc.tile_pool(name="stats", bufs=2))
        aqp = ctx.enter_context(tc.tile_pool(name="aqp", bufs=2))
        yp = ctx.enter_context(tc.tile_pool(name="yp", bufs=1))
        ps1p = ctx.enter_context(tc.tile_pool(name="ps1", bufs=2, space="PSUM"))
        ps2p = ctx.enter_context(tc.tile_pool(name="ps2", bufs=1, space="PSUM"))

        # --- prologue ---
        xqT_s = consts.tile([128, KH, NPOS], BF16, name="xqT_s")
        nc.scalar.dma_start(out=xqT_s[:], in_=xqT_d)
        ident = consts.tile([128, 128], F32, name="ident")
        make_identity(nc, ident[:])
        ones1 = consts.tile([1, 128], F32, name="ones1")
        nc.vector.memset(ones1[:], 1.0)
        sxr_s = consts.tile([1, NPOS], F32, name="sxr_s")
        scr_s = consts.tile([1, NPOS], F32, name="scr_s")
        w1sc_s, w2sc_s = [], []
        for e in range(EPC):
            w1sc_s.append(consts.tile([128, CT1], F32, name=f"w1sc_{e}"))
            w2sc_s.append(consts.tile([128, CT2], F32, name=f"w2sc_{e}"))

        def scale_dmas():
            for e in range(EPC):
                nc.sync.dma_start(out=w1sc_s[e][:], in_=w1sc_d[e])
                nc.sync.dma_start(out=w2sc_s[e][:], in_=w2sc_d[e])

        # broadcast of per-position rows happens in late_prologue (emitted
        # after the first weight chunks so small DMAs don't hog HWDGE early);
        # the psum buffer is reserved here to keep the ps1 tag rotation.
        psb0 = ps1p.tile([128, 24, NT], F32, tag="ps1", name="psb0")
        sxb = consts.tile([128, NPOS], F32, name="sxb")
        scb = consts.tile([128, NPOS], F32, name="scb")

        def late_prologue():
            nc.scalar.dma_start(out=sxr_s[:], in_=sxr_d)
            nc.scalar.dma_start(out=scr_s[:], in_=scr_d)
            bc0 = psb0[:, 0:2, :].rearrange("p a t -> p (a t)")
            nc.tensor.matmul(bc0, lhsT=ones1[:], rhs=sxr_s[:], start=True, stop=True)
            nc.vector.tensor_copy(out=sxb[:], in_=bc0)
            bc1 = psb0[:, 2:4, :].rearrange("p a t -> p (a t)")
            nc.tensor.matmul(bc1, lhsT=ones1[:], rhs=scr_s[:], start=True, stop=True)
            nc.vector.tensor_copy(out=scb[:], in_=bc1)

        # Emission order == per-engine execution order, so the phases below
        # software-pipeline the kernel: each expert's epilogue is emitted in
        # slices spliced between the NEXT phase's chunk pipelines, keeping
        # every engine queue free of long head-of-line dependency waits.
        st = {}  # per-expert tiles carried across phases
        aqs, s2cs = [], []

        def g1_chunk(e, kg):
            if kg == 0:
                # 3 exact PSUM banks; one accumulation group per bank (8
                # chunks share a bank: start on the bank's first chunk, stop
                # on its last -- HW zeroing is lazy per-write in the region).
                st[f"ps1_{e}"] = ps1p.tile([128, 24, NT], F32, tag="ps1", name=f"ps1_{e}")
            ps1 = st[f"ps1_{e}"]
            w1i = w1i8p.tile([128, 4, I2], I8, tag="w1i", name=f"w1i_{e}_{kg}")
            # two half-chunk DMAs (5.6 KB runs, full rate): each k-pair's
            # casts start as soon as its half lands
            nc.sync.dma_start(
                out=w1i[:, 0:2, :], in_=w1_d[e, :, kg * 4 : kg * 4 + 2, :]
            )
            nc.sync.dma_start(
                out=w1i[:, 2:4, :], in_=w1_d[e, :, kg * 4 + 2 : kg * 4 + 4, :]
            )
            w1b = w1bfp.tile([128, 4, I2], BF16, tag="w1b", name=f"w1b_{e}_{kg}")
            for j in (0, 2):
                nc.vector.tensor_copy(
                    out=w1b[:, j : j + 2, 0:W1V], in_=w1i[:, j : j + 2, 0:W1V]
                )
                nc.scalar.copy(
                    out=w1b[:, j : j + 2, W1V:W1A], in_=w1i[:, j : j + 2, W1V:W1A]
                )
                nc.gpsimd.tensor_copy(
                    out=w1b[:, j : j + 2, W1A:I2], in_=w1i[:, j : j + 2, W1A:I2]
                )
            for j in range(4):
                k = kg * 4 + j
                for c in range(CT1):
                    nc.tensor.matmul(
                        ps1[:, c, :],
                        lhsT=w1b[:, j, c * 128 : (c + 1) * 128],
                        rhs=xqT_s[:, k, e * NT : (e + 1) * NT],
                        start=(k == 0 and c % 8 == 0),
                        stop=(k == KH - 1 and (c % 8 == 7 or c == CT1 - 1)),
                    )

        def epi(e, s):
            """Epilogue slice s (1..4) for expert e."""
            if s == 1:
                # dequant: releases ps1
                ps1 = st[f"ps1_{e}"]
                sxb64 = sxb[:, e * NT : (e + 1) * NT]
                gate = st[f"gate_{e}"] = ep.tile(
                    [128, KI, NT], F32, tag="gate", name=f"gate_{e}"
                )
                up = st[f"up_{e}"] = ep.tile(
                    [128, KI, NT], F32, tag="up", name=f"up_{e}"
                )
                # gate = psum * w1sc[ch] * sx[tok]; up = psum * w1sc_up[ch]
                # (sx deferred into s2c: aq is invariant to per-token scale)
                w1g = w1sc_s[e][:, 0:KI].unsqueeze(2).broadcast_to([128, KI, NT])
                w1u = w1sc_s[e][:, KI:CT1].unsqueeze(2).broadcast_to([128, KI, NT])
                sx3 = sxb64.unsqueeze(1).broadcast_to([128, KI, NT])
                nc.vector.tensor_tensor(out=gate[:], in0=ps1[:, 0:KI, :], in1=w1g, op=mult)
                nc.vector.tensor_tensor(out=gate[:], in0=gate[:], in1=sx3, op=mult)
                nc.vector.tensor_tensor(out=up[:], in0=ps1[:, KI : 2 * KI, :], in1=w1u, op=mult)
            elif s == 2:
                gate, up = st[f"gate_{e}"], st[f"up_{e}"]
                sig = ep.tile([128, KI, NT], F32, tag="sig", name=f"sig_{e}")
                nc.scalar.activation(
                    out=sig[:], in_=gate[:], func=mybir.ActivationFunctionType.Sigmoid
                )
                gsig = ep.tile([128, KI, NT], F32, tag="gsig", name=f"gsig_{e}")
                nc.vector.tensor_tensor(out=gsig[:], in0=gate[:], in1=sig[:], op=mult)
                act = st[f"act_{e}"] = ep.tile(
                    [128, KI, NT], F32, tag="act", name=f"act_{e}"
                )
                nc.vector.tensor_tensor(out=act[:], in0=gsig[:], in1=up[:], op=mult)
            elif s == 3:
                act = st[f"act_{e}"]
                # per-token absmax over all I channels (partitions x 11 tiles)
                mall = st[f"mall_{e}"] = ep.tile(
                    [128, KI, NT], F32, tag="mall", name=f"mall_{e}"
                )
                nc.gpsimd.partition_all_reduce(
                    mall[:].rearrange("p j t -> p (j t)"),
                    act[:].rearrange("p j t -> p (j t)"),
                    128,
                    bass_isa.ReduceOp.absmax,
                )
                mfin = stats.tile([128, NT], F32, tag="mfin", name=f"mfin_{e}")
                nc.vector.reduce_max(
                    out=mfin[:].unsqueeze(2),
                    in_=mall[:].rearrange("p j t -> p t j"),
                    axis=mybir.AxisListType.X,
                )
                mc = stats.tile([128, NT], F32, tag="mc", name=f"mc_{e}")
                nc.vector.tensor_scalar_max(out=mc[:], in0=mfin[:], scalar1=1e-12)
                rr = stats.tile([128, NT], F32, tag="rr", name=f"rr_{e}")
                nc.vector.reciprocal(out=rr[:], in_=mc[:])
                r127 = st[f"r127_{e}"] = stats.tile(
                    [128, NT], F32, tag="r127", name=f"r127_{e}"
                )
                nc.vector.tensor_scalar_mul(out=r127[:], in0=rr[:], scalar1=127.0)
                # s2c = (mc/127) * sx[tok] * comb[tok]  (sx folded in scrow)
                s2c = stats.tile([128, NT], F32, tag="s2c", name=f"s2c_{e}")
                nc.vector.scalar_tensor_tensor(
                    out=s2c[:],
                    in0=mc[:],
                    scalar=1.0 / 127.0,
                    in1=scb[:, e * NT : (e + 1) * NT],
                    op0=mult,
                    op1=mult,
                )
                s2cs.append(s2c)
            else:
                act, r127 = st[f"act_{e}"], st[f"r127_{e}"]
                tq = ep.tile([128, KI, NT], F32, tag="gate", name=f"tq_{e}")
                nc.vector.tensor_tensor(
                    out=tq[:],
                    in0=act[:],
                    in1=r127[:].unsqueeze(1).broadcast_to([128, KI, NT]),
                    op=mult,
                )
                aq = aqp.tile([128, KI, NT], BF16, tag="aq", name=f"aq_{e}")
                nc.vector.tensor_scalar(
                    out=aq[:],
                    in0=tq[:],
                    scalar1=MAGIC,
                    scalar2=MAGIC,
                    op0=mybir.AluOpType.add,
                    op1=mybir.AluOpType.subtract,
                )
                aqs.append(aq)

        def w2dma(e, g):
            if e == EPC - 1:
                # expert-1 w2 ships pre-cast as bf16 over plain HWDGE: costs
                # 2 B/weight on the DMA device (idle in this window) but needs
                # zero engine-cast work during the saturated tail
                w2b = st[f"w2b_{e}_{g}"] = w2bfp.tile(
                    [128, KI, 512], BF16, tag="w2be1", name=f"w2b_{e}_{g}"
                )
                nc.sync.dma_start(out=w2b[:, :, 0:256], in_=w2b1_d[2 * g])
                nc.sync.dma_start(out=w2b[:, :, 256:512], in_=w2b1_d[2 * g + 1])
                return
            w2i = st[f"w2i_{e}_{g}"] = w2i8p.tile(
                [128, KI, 512], I8, tag="w2i", name=f"w2i_{e}_{g}"
            )
            nc.sync.dma_start(out=w2i[:], in_=w2_d[e, :, :, g * 512 : (g + 1) * 512])

        def w2cast(e, g):
            w2i = st[f"w2i_{e}_{g}"]
            w2b = st[f"w2b_{e}_{g}"] = w2bfp.tile(
                [128, KI, 512], BF16, tag="w2b", name=f"w2b_{e}_{g}"
            )
            nc.vector.tensor_copy(out=w2b[:, :, 0:W2V], in_=w2i[:, :, 0:W2V])
            nc.scalar.copy(out=w2b[:, :, W2V:W2A], in_=w2i[:, :, W2V:W2A])
            nc.gpsimd.tensor_copy(out=w2b[:, :, W2A:512], in_=w2i[:, :, W2A:512])

        def g2mm(e, g):
            w2b = st[f"w2b_{e}_{g}"]
            # one full bank per tag; single accumulation group per bank
            ps2 = ps2p.tile([128, 8, NT], F32, tag=f"ps2{e}", name=f"ps2_{e}_{g}")
            for cc in range(4):
                for k in range(KI):
                    nc.tensor.matmul(
                        ps2[:, cc, :],
                        lhsT=w2b[:, k, cc * 128 : (cc + 1) * 128],
                        rhs=aqs[e][:, k, :],
                        start=(k == 0 and cc == 0),
                        stop=(k == KI - 1 and cc == 3),
                    )
            # deq2: o * w2sc[h] * (s2 * comb)[tok] -> yT columns
            w2s3 = (
                w2sc_s[e][:, g * 4 : (g + 1) * 4]
                .unsqueeze(2)
                .broadcast_to([128, 4, NT])
            )
            s2c3 = s2cs[e][:].unsqueeze(1).broadcast_to([128, 4, NT])
            dtmp = ep.tile([128, 4, NT], F32, tag="dtmp", name=f"dtmp_{e}_{g}")
            nc.vector.tensor_tensor(out=dtmp[:], in0=ps2[:, 0:4, :], in1=w2s3, op=mult)
            nc.vector.tensor_tensor(
                out=yT[:, g * 4 : (g + 1) * 4, e * NT : (e + 1) * NT],
                in0=dtmp[:],
                in1=s2c3,
                op=mult,
            )

        def ytail(g):
            # both experts done for these channel tiles: emit y rows
            if "pst" not in st:
                st["pst"] = ps1p.tile([128, 24, NT], F32, tag="ps1", name="pst")
            pst = st["pst"]
            views = [
                pst[:, 2 * cc : 2 * cc + 2, :].rearrange("p a t -> p (a t)")
                for cc in range(4)
            ]
            for cc in range(4):
                c = g * 4 + cc
                nc.tensor.transpose(views[cc], yT[:, c, :], ident[:])
            nc.scalar.copy(
                out=ysb[:, g * 512 : (g + 1) * 512],
                in_=pst[:, 0:8, :].rearrange("p a t -> p (a t)"),
            )
            nc.sync.dma_start(
                out=y_d[:, g * 512 : (g + 1) * 512],
                in_=ysb[:, g * 512 : (g + 1) * 512],
            )

        yT = yp.tile([128, CT2, NPOS], F32, name="yT")
        ysb = yp.tile([128, H], F32, name="ysb")

        g1_chunk(0, 0)
        g1_chunk(1, 0)
        scale_dmas()
        late_prologue()
        for kg in range(1, 4):
            g1_chunk(0, kg)
            g1_chunk(1, kg)
        for s in range(1, 5):
            epi(0, s)
            epi(1, s)
        for e in range(EPC):
            for g in range(4):
                w2dma(e, g)
        for g in range(4):
            w2cast(0, g)
            g2mm(0, g)
            g2mm(1, g)
            ytail(g)

    nc.compile()
    return nc


def get_program():
    if "nc" not in _cache:
        _cache["nc"] = _build_program()
    return _cache["nc"]


def _prep_inputs(x, expert_ids, smooth_scales, expert_scales, w1, w1_scale, w2, w2_scale):
    """Host-side dispatch: quantize x, route tokens, shard experts."""
    x = np.asarray(x, np.float32)
    expert_ids = np.asarray(expert_ids)
    smooth_scales = np.asarray(smooth_scales, np.float32)
    expert_scales = np.asarray(expert_scales, np.float32)
    w1_scale = np.asarray(w1_scale, np.float32)
    w2_scale = np.asarray(w2_scale, np.float32)

    # dynamic per-token int8 quantization (exact mirror of reference ops)
    sx = np.maximum(np.max(np.abs(x), axis=-1, keepdims=True), 1e-12) / 127.0
    xq = np.round(np.clip(x / sx, -128.0, 127.0)).astype(np.float32)

    # combine matrix [B, E]: scatter-add expert_scales at expert_ids
    comb = np.zeros((B, E), np.float32)
    np.add.at(comb, (np.arange(B)[:, None], expert_ids), expert_scales)

    w1v = w1.astype(np.int8)
    w2v = w2.astype(np.int8)

    in_maps = []
    pos2tok = np.zeros((NCORES, NPOS), np.int64)
    for c in range(NCORES):
        valid = np.zeros(NPOS, bool)
        ge_of_pos = np.zeros(NPOS, np.int64)
        for le in range(EPC):
            ge = EPC * c + le
            toks = np.nonzero((expert_ids == ge).any(axis=1))[0]
            assert len(toks) <= NT, f"expert {ge}: {len(toks)} tokens > capacity {NT}"
            pos2tok[c, le * NT : le * NT + len(toks)] = toks
            valid[le * NT : le * NT + len(toks)] = True
            ge_of_pos[le * NT : (le + 1) * NT] = ge

        toks_c = pos2tok[c]
        xg = xq[toks_c]  # [NPOS, H]
        xqT = np.ascontiguousarray(
            xg.T.reshape(KH, 128, NPOS).transpose(1, 0, 2)
        ).astype(ml_dtypes.bfloat16)
        sxrow = sx[toks_c, 0][None, :].astype(np.float32)
        combp = comb[toks_c, ge_of_pos] * valid  # zero at padding positions
        scrow = (sx[toks_c, 0] * combp)[None, :].astype(np.float32)

        es = list(range(EPC * c, EPC * (c + 1)))
        w1t = np.ascontiguousarray(
            w1v[es].reshape(EPC, KH, 128, I2).transpose(0, 2, 1, 3)
        )
        w2t = np.ascontiguousarray(
            w2v[es].reshape(EPC, KI, 128, H).transpose(0, 2, 1, 3)
        )
        w2bf1 = np.ascontiguousarray(
            w2t[EPC - 1].reshape(128, KI, 8, 256).transpose(2, 0, 1, 3)
        ).astype(ml_dtypes.bfloat16)
        w1sc = np.empty((EPC, 128, CT1), np.float32)
        w2sc = np.empty((EPC, 128, CT2), np.float32)
        for le, ge in enumerate(es):
            gatesc = w1_scale[ge, :I].reshape(KI, 128).T
            upsc = (w1_scale[ge, I:] * smooth_scales[ge]).reshape(KI, 128).T
            w1sc[le] = np.concatenate([gatesc, upsc], axis=1)
            w2sc[le] = w2_scale[ge].reshape(CT2, 128).T

        in_maps.append(
            {
                "xqT": xqT,
                "sxrow": sxrow,
                "scrow": scrow,
                "w1t": w1t,
                "w2t": w2t,
                "w2bf1": w2bf1,
                "w1sc": w1sc,
                "w2sc": w2sc,
            }
        )
    return in_maps, pos2tok


def kernel(
    x,
    expert_ids,
    smooth_scales,
    expert_scales,
    x_active_mask,
    w1,
    w1_scale,
    w2,
    w2_scale,
    _trace=False,
    _trace_kwargs=None,
):
    in_maps, pos2tok = _prep_inputs(
        x, expert_ids, smooth_scales, expert_scales, w1, w1_scale, w2, w2_scale
    )
    nc = get_program()
    res = run_bass_kernel_spmd(
        nc,
        in_maps,
        core_ids=list(range(NCORES)),
        trace=_trace,
        **(_trace_kwargs or {}),
    )
    y = np.zeros((B, H), np.float32)
    for c, r in enumerate(res.results):
        np.add.at(y, pos2tok[c], r["y"].astype(np.float32))
    y *= np.asarray(x_active_mask).astype(np.float32)[:, None]
    if _trace:
        kernel.last_results = res
    return y
